# revision 9
# baseline (speedup 1.0000x reference)
"""GATv2 + softmax head for nn_GATModel_Softmax_4535485465120 on 8 trn2 NeuronCores.

v2: fp8-DoubleRow projections + xle-based aggregation (no XR-transpose phase).

Strategy (graph/data parallel by dst node, fully local — no collectives):
  - Nodes partitioned into 8 ranges of 1000 dst nodes (one per core).
  - Host preps per core: x.T columns (fp8) for the core's unique src nodes and
    its dst nodes; weights with att-magnitude (clamped at max/64 per head)
    folded into Wl/Wr columns, permuted pos-att-first, scaled 1/S for fp8;
    Wf rows carry the inverse permutation, S/(3*att_eff) un-scaling and the
    head-mean.
  - Device pipeline per core:
      phase 0:  XR'' = xdst8 @ Wr8 via fp8 DoubleRow matmuls -> SBUF fp8
                (resident, pair-packed by dst block for DR reuse)
      phase 0.5: XLu'' = xu8 @ Wl8 (DR) -> HBM bf16 [UP, 3072]
      phase 1 (per dst block, per 128-edge tile): U/S accumulated directly in
                PSUM = DR-selection-matmul of XR'' + identity-matmul of
                gathered XLu'' rows; leaky-relu + sign-segmented accumulation
                on DVE reads PSUM directly; a = exp(S * esc) on ScalarE.
      phase 2:  denom/alpha via selection matmuls; agg^T = sum_e alpha *
                xle (selection matmul, bf16); logits = agg^T-chunks @ Wf_stack
                (+folded bias row); row softmax -> output.

kernel(**inputs) takes FULL inputs, shards internally, returns FULL [8000,460] f32.
"""

import numpy as np
import ml_dtypes

BF16 = ml_dtypes.bfloat16
E4M3 = ml_dtypes.float8_e4m3   # IEEE e4m3: max 240, has inf (matches mybir float8e4)
F8MAX = 240.0

# Problem constants (hardcoded per spec)
N = 8000
DIN = 1024
H = 3
C = 1024
HC = H * C          # 3072
NCLS = 460
NCLS_P = 512
NEG_SLOPE = 0.2
NCORES = 8
ND = N // NCORES    # 1000 dst nodes per core
NDP = 1024          # padded dst count per core
DB = NDP // 128     # 8 dst blocks per core
P = 128
KC = DIN // P       # 8 contraction chunks (4 DoubleRow pairs)
NB = 3              # n-chunks of 1024 in HC (one per head)
HF = 2              # 512-wide matmul halves per 1024 chunk
NAUG = 8            # aug columns: per-head pos/neg sums (6) + pad (2)
HCA = HC + NAUG     # 3080
AUGDIV = 64.0       # aug columns scaled by 1/64 to stay in fp8 range


def _prep(x, edge_index, Wl, bl, Wr, br, att, bias, Wf, bf):
    """Host-side preprocessing -> per-core input maps + static dims."""
    x = np.asarray(x, np.float32)
    ei = np.asarray(edge_index).astype(np.int64)
    Wl = np.asarray(Wl, np.float32)
    Wr = np.asarray(Wr, np.float32)
    bl = np.asarray(bl, np.float32)
    br = np.asarray(br, np.float32)
    att = np.asarray(att, np.float32)
    bias = np.asarray(bias, np.float32)
    Wf = np.asarray(Wf, np.float32)
    bf = np.asarray(bf, np.float32)

    assert np.all(bl == 0) and np.all(br == 0), \
        "nonzero bl/br not supported by this kernel build"

    loops = np.arange(N, dtype=np.int64)
    src_all = np.concatenate([ei[:, 0], loops])
    dst_all = np.concatenate([ei[:, 1], loops])

    # att folding: per head, column scale att_eff (clamped so fp8 columns
    # stay out of the subnormal range) and permutation pos-first
    absatt = np.abs(att)                       # [H, C]
    att_eff = np.maximum(absatt, absatt.max(1, keepdims=True) / 64.0)
    perm = np.zeros((H, C), np.int64)          # perm[h, newc] = origc
    npos = np.zeros(H, np.int64)
    for h in range(H):
        pos = np.nonzero(att[h] > 0)[0]
        neg = np.nonzero(att[h] <= 0)[0]
        perm[h] = np.concatenate([pos, neg])
        npos[h] = len(pos)

    # scaled/permuted projection weights  [DIN, HC]
    Wl_s = np.zeros((DIN, HC), np.float32)
    Wr_s = np.zeros((DIN, HC), np.float32)
    for h in range(H):
        sc = att_eff[h, perm[h]]               # [C]
        Wl_s[:, h * C:(h + 1) * C] = Wl[:, h * C + perm[h]] * sc
        Wr_s[:, h * C:(h + 1) * C] = Wr[:, h * C + perm[h]] * sc

    # global fp8 scale S: covers weight absmax and a 6-sigma bound on the
    # projected activations (xr'' must fit fp8 storage after the matmul)
    colnorm = max(np.linalg.norm(Wl_s, axis=0).max(),
                  np.linalg.norm(Wr_s, axis=0).max())
    S = float(max(np.abs(Wl_s).max() / F8MAX, np.abs(Wr_s).max() / F8MAX,
                  colnorm * 7.0 / F8MAX))

    def with_aug(W):
        W8 = np.clip(W / S, -F8MAX, F8MAX).astype(E4M3)
        Wa = np.zeros((DIN, HCA), E4M3)
        Wa[:, :HC] = W8
        W8f = W8.astype(np.float32)
        for h in range(H):
            np_h = int(npos[h])
            Wa[:, HC + 2 * h] = (W8f[:, h * C:h * C + np_h].sum(1) / AUGDIV).astype(E4M3)
            Wa[:, HC + 2 * h + 1] = (W8f[:, h * C + np_h:(h + 1) * C].sum(1) / AUGDIV).astype(E4M3)
        return Wa
    wl8 = with_aug(Wl_s)
    wr8 = with_aug(Wr_s)

    # final fc stack: logits = sum_h (agg''_h * S/(3*att_eff)) @ Wf  (+ bias@Wf + bf)
    Wfs = np.zeros((HC, NCLS_P), np.float32)
    for h in range(H):
        sc = S / (3.0 * att_eff[h, perm[h]])
        Wfs[h * C:(h + 1) * C, :NCLS] = Wf[perm[h]] * sc[:, None]
    bf2 = np.full((1, NCLS_P), -1e30, np.float32)
    bf2[0, :NCLS] = bias @ Wf + bf

    xT8 = np.clip(np.ascontiguousarray(x.T), -F8MAX, F8MAX).astype(E4M3)     # [DIN, N]

    # per-core edge grouping: edges (incl. self loops) by dst block
    cores = []
    tmax = 1
    for k in range(NCORES):
        lo, hi = k * ND, (k + 1) * ND
        m = (dst_all >= lo) & (dst_all < hi)
        s_k = src_all[m]
        dl_k = (dst_all[m] - lo).astype(np.int64)
        order = np.argsort(dl_k, kind="stable")
        s_k, dl_k = s_k[order], dl_k[order]
        blocks = []
        for db in range(DB):
            bm = (dl_k >= db * 128) & (dl_k < (db + 1) * 128)
            blocks.append((s_k[bm], dl_k[bm]))
            tmax = max(tmax, (len(blocks[-1][0]) + 127) // 128)
        cores.append(blocks)

    T_BLK = tmax
    E1T = DB * T_BLK
    E1P = E1T * 128

    iotaF = np.tile(np.arange(128, dtype=np.float32)[None, :], (128, 1))
    iotaP = np.tile(np.arange(128, dtype=np.float32)[:, None], (1, 128))
    ones1 = np.ones((1, 128), BF16)
    # unique srcs per core -> common padded tile count
    uniq = []
    for k in range(NCORES):
        srcs = np.concatenate([cores[k][db][0] for db in range(DB)])
        uniq.append(np.unique(srcs))
    UT = max((len(u) + 127) // 128 for u in uniq)
    UP = UT * 128
    in_maps = []
    for k in range(NCORES):
        srcslot = np.zeros(E1P, np.int64)
        real = np.zeros(E1P, bool)
        dstloc = np.full(E1P, -1.0, np.float32)
        for db in range(DB):
            s_k, dl_k = cores[k][db]
            base = db * T_BLK * 128
            srcslot[base:base + len(s_k)] = s_k
            real[base:base + len(s_k)] = True
            dstloc[base:base + len(s_k)] = dl_k.astype(np.float32)
        u = uniq[k]
        xuT = np.zeros((DIN, UP), E4M3)
        xuT[:, :len(u)] = xT8[:, u]
        srcloc = np.zeros((E1P, 1), np.int32)
        srcloc[real, 0] = np.searchsorted(u, srcslot[real]).astype(np.int32)
        xdstT = np.zeros((DIN, NDP), E4M3)
        xdstT[:, :ND] = xT8[:, k * ND:(k + 1) * ND]
        dst_col = np.ascontiguousarray(dstloc.reshape(E1T, 128).T)   # [128, E1T]
        dst_row = np.tile(dstloc[None, :], (128, 1))                 # [128, E1P]
        in_maps.append({
            "xuT": xuT,
            "srcloc": srcloc,
            "xdstT": xdstT,
            "wl": wl8,
            "wr": wr8,
            "wfs": Wfs.astype(BF16),
            "bf2": bf2.astype(BF16),
            "dstcp": dst_col,
            "dstrow": dst_row,
            "iotaF": iotaF,
            "iotaP": iotaP,
            "ones1": ones1,
        })
    dims = {"T_BLK": T_BLK, "E1T": E1T, "E1P": E1P, "UT": UT,
            "npos": [int(v) for v in npos], "S": S}
    return in_maps, dims


def _build(dims, debug=False):
    """Trace the Bass/Tile program (identical for all cores)."""
    import concourse.bass as bass
    import concourse.mybir as mybir
    import concourse.tile as tile
    from concourse import bacc

    T_BLK, E1T, E1P = dims["T_BLK"], dims["E1T"], dims["E1P"]
    UT = dims["UT"]
    npos = dims["npos"]
    S = dims["S"]
    UP = UT * 128
    fp32 = mybir.dt.float32
    bf16 = mybir.dt.bfloat16
    fp8 = mybir.dt.float8e4
    AT = mybir.AluOpType
    AF = mybir.ActivationFunctionType
    DRM = mybir.MatmulPerfMode.DoubleRow

    nc = bacc.Bacc("TRN2", target_bir_lowering=False, debug=False)

    with tile.TileContext(nc) as tc:
        with tc.tile_pool(name="dram", bufs=1, space="DRAM") as dram:
            d_xuT = dram.tile([DIN, UP], fp8, kind="ExternalInput", name="xuT", uniquify=False)
            d_srcloc = dram.tile([E1P, 1], mybir.dt.int32, kind="ExternalInput", name="srcloc", uniquify=False)
            d_xdstT = dram.tile([DIN, NDP], fp8, kind="ExternalInput", name="xdstT", uniquify=False)
            d_wl = dram.tile([DIN, HCA], fp8, kind="ExternalInput", name="wl", uniquify=False)
            d_wr = dram.tile([DIN, HCA], fp8, kind="ExternalInput", name="wr", uniquify=False)
            d_wfs = dram.tile([HC, NCLS_P], bf16, kind="ExternalInput", name="wfs", uniquify=False)
            d_bf2 = dram.tile([1, NCLS_P], bf16, kind="ExternalInput", name="bf2", uniquify=False)
            d_dstcp = dram.tile([128, E1T], fp32, kind="ExternalInput", name="dstcp", uniquify=False)
            d_dstrow = dram.tile([128, E1P], fp32, kind="ExternalInput", name="dstrow", uniquify=False)
            d_iotaF = dram.tile([128, 128], fp32, kind="ExternalInput", name="iotaF", uniquify=False)
            d_iotaP = dram.tile([128, 128], fp32, kind="ExternalInput", name="iotaP", uniquify=False)
            d_ones1 = dram.tile([1, 128], bf16, kind="ExternalInput", name="ones1", uniquify=False)
            d_out = dram.tile([NDP, NCLS_P], fp32, kind="ExternalOutput", name="out", uniquify=False)
            d_xlu = dram.tile([UP, HCA], bf16, name="xlu_i",
                              kind="ExternalOutput" if debug else "Internal",
                              uniquify=False)
            if debug:
                d_dbga = dram.tile([128, E1T * H], fp32, kind="ExternalOutput", name="dbg_a", uniquify=False)
                d_dbgx = dram.tile([128, DB, HCA], fp8, kind="ExternalOutput", name="dbg_xrd", uniquify=False)

            with tc.tile_pool(name="gsb", bufs=1) as gsb:
                # resident tensors
                dstcp = gsb.tile([128, E1T], fp32, name="dstcp_r")
                nc.sync.dma_start(out=dstcp[:], in_=d_dstcp[:])
                iotaF = gsb.tile([128, 128], fp32, name="iotaF_r")
                nc.sync.dma_start(out=iotaF[:], in_=d_iotaF[:])
                iotaP = gsb.tile([128, 128], fp32, name="iotaP_r")
                nc.sync.dma_start(out=iotaP[:], in_=d_iotaP[:])
                ones1 = gsb.tile([1, 128], bf16, name="ones1_r")
                nc.sync.dma_start(out=ones1[:], in_=d_ones1[:])
                bf2 = gsb.tile([1, NCLS_P], bf16, name="bf2_r")
                nc.sync.dma_start(out=bf2[:], in_=d_bf2[:])
                a_all = gsb.tile([128, E1T * H], fp32, name="a_all_r")
                denr = gsb.tile([128, DB * H], fp32, name="denr_r")
                xrdb8 = gsb.tile([128, DB, HCA], fp8, name="xrdb8_r")
                idm = gsb.tile([128, 128], bf16, name="idm_r")
                nc.vector.scalar_tensor_tensor(
                    out=idm[:], in0=iotaP[:], scalar=0.0, in1=iotaF[:],
                    op0=AT.add, op1=AT.is_equal)

                # ---------------- phase 0: XR'' projection (fp8 DR) ----------------
                with tc.tile_pool(name="p01", bufs=1, space="PSUM") as ps1:
                    with tc.tile_pool(name="wpool", bufs=1) as wpool:
                        wmat = wpool.tile([128, KC, HCA], fp8, tag="wmat", bufs=1, name="wmat_r")
                        nc.sync.dma_start(out=wmat[:], in_=d_wr[:].rearrange("(kc p) n -> p kc n", p=128))
                        xdst8 = wpool.tile([128, KC, NDP], fp8, tag="xdst8", bufs=1, name="xdst8_r")
                        nc.sync.dma_start(out=xdst8[:], in_=d_xdstT[:].rearrange("(kc p) n -> p kc n", p=128))
                        for dc in range(DB):
                            for nb in range(NB):
                                pp = ps1.tile([128, C], fp32, tag="pp", bufs=2, name=f"pp{nb}_c{dc}")
                                for kp in range(KC // 2):
                                    for hf in range(HF):
                                        nc.tensor.matmul(
                                            pp[:, hf * 512:(hf + 1) * 512],
                                            xdst8[:, 2 * kp:2 * kp + 2, dc * 128:(dc + 1) * 128],
                                            wmat[:, 2 * kp:2 * kp + 2, nb * C + hf * 512:nb * C + (hf + 1) * 512],
                                            start=(kp == 0), stop=(kp == KC // 2 - 1),
                                            perf_mode=DRM)
                                nc.scalar.activation(out=xrdb8[:, dc, nb * C:(nb + 1) * C],
                                                     in_=pp[:], func=AF.Copy)
                            ppa = ps1.tile([128, NAUG], fp32, tag="ppaug", bufs=1, name=f"ppa_c{dc}")
                            for kp in range(KC // 2):
                                nc.tensor.matmul(
                                    ppa[:], xdst8[:, 2 * kp:2 * kp + 2, dc * 128:(dc + 1) * 128],
                                    wmat[:, 2 * kp:2 * kp + 2, HC:HCA],
                                    start=(kp == 0), stop=(kp == KC // 2 - 1),
                                    perf_mode=DRM)
                            nc.scalar.activation(out=xrdb8[:, dc, HC:HCA], in_=ppa[:], func=AF.Copy)

                        # ------- phase 0.5: XLu'' projection of unique srcs (fp8 DR) -------
                        wmat2 = wpool.tile([128, KC, HCA], fp8, tag="wmat", bufs=1, name="wmat_r2")
                        nc.sync.dma_start(out=wmat2[:], in_=d_wl[:].rearrange("(kc p) n -> p kc n", p=128))
                        xu8 = wpool.tile([128, KC, UP], fp8, tag="xu8", bufs=1, name="xu8_r")
                        nc.sync.dma_start(out=xu8[:], in_=d_xuT[:].rearrange("(kc p) n -> p kc n", p=128))
                        with tc.tile_pool(name="sb0", bufs=2) as sb0:
                            for ut in range(UT):
                                xl_sb = sb0.tile([128, HCA], bf16, tag="xl_sb")
                                for nb in range(NB):
                                    pp = ps1.tile([128, C], fp32, tag="pp", bufs=2, name=f"ppu{nb}_{ut}")
                                    for kp in range(KC // 2):
                                        for hf in range(HF):
                                            nc.tensor.matmul(
                                                pp[:, hf * 512:(hf + 1) * 512],
                                                xu8[:, 2 * kp:2 * kp + 2, ut * 128:(ut + 1) * 128],
                                                wmat2[:, 2 * kp:2 * kp + 2, nb * C + hf * 512:nb * C + (hf + 1) * 512],
                                                start=(kp == 0), stop=(kp == KC // 2 - 1),
                                                perf_mode=DRM)
                                    if nb == 0:
                                        nc.vector.tensor_copy(out=xl_sb[:, nb * C:(nb + 1) * C], in_=pp[:])
                                    else:
                                        nc.scalar.activation(out=xl_sb[:, nb * C:(nb + 1) * C],
                                                             in_=pp[:], func=AF.Copy)
                                ppa = ps1.tile([128, NAUG], fp32, tag="ppaug", bufs=1, name=f"ppa_u{ut}")
                                for kp in range(KC // 2):
                                    nc.tensor.matmul(
                                        ppa[:], xu8[:, 2 * kp:2 * kp + 2, ut * 128:(ut + 1) * 128],
                                        wmat2[:, 2 * kp:2 * kp + 2, HC:HCA],
                                        start=(kp == 0), stop=(kp == KC // 2 - 1),
                                        perf_mode=DRM)
                                nc.scalar.activation(out=xl_sb[:, HC:HCA], in_=ppa[:], func=AF.Copy)
                                nc.sync.dma_start(out=d_xlu[ut * 128:(ut + 1) * 128, :], in_=xl_sb[:])

                    # wfs resident (phase-0 weight slots are free now)
                    wfs_r = gsb.tile([128, HC // 128, NCLS_P], bf16, name="wfs_r")
                    nc.sync.dma_start(out=wfs_r[:], in_=d_wfs[:].rearrange("(cc p) n -> p cc n", p=128))

                    # ------- fused phases 1+2: per dst block -------
                    with (
                        tc.tile_pool(name="sb1", bufs=3) as sb1,
                        tc.tile_pool(name="sbe", bufs=T_BLK + 4) as sbe,
                        tc.tile_pool(name="p2", bufs=1, space="PSUM") as ps2,
                        tc.tile_pool(name="sb2", bufs=3) as sb2,
                    ):
                        for db in range(DB):
                            pb = db // 2
                            jact = db % 2
                            xles, esels, sals_all, drows = [], [], [], []
                            for t2 in range(T_BLK):
                                t = db * T_BLK + t2
                                sidx = sb1.tile([128, 1], mybir.dt.int32, tag="sidx")
                                nc.sync.dma_start(out=sidx[:], in_=d_srcloc[t * 128:(t + 1) * 128, :])
                                drow = sbe.tile([128, 128], fp32, tag="drow", name=f"drow_{db}_{t2}")
                                nc.sync.dma_start(out=drow[:], in_=d_dstrow[:, t * 128:(t + 1) * 128])
                                drows.append(drow)
                                # gather XL''[src] rows (bf16)
                                xle = sbe.tile([128, HCA], bf16, tag="xle", name=f"xle_{db}_{t2}")
                                nc.gpsimd.indirect_dma_start(
                                    out=xle[:], out_offset=None, in_=d_xlu[:],
                                    in_offset=bass.IndirectOffsetOnAxis(ap=sidx[:, :1], axis=0))
                                xles.append(xle)
                                # pair-packed 0/1 dst-selection for DoubleRow U-expansion
                                eselw = sb1.tile([128, 2, 128], fp8, tag="eselw")
                                nc.gpsimd.memset(eselw[:, 1 - jact, :], 0.0)
                                nc.vector.scalar_tensor_tensor(
                                    out=eselw[:, jact, :], in0=drow[:],
                                    scalar=float(-db * 128), in1=iotaP[:],
                                    op0=AT.add, op1=AT.is_equal)
                                eacc = sb1.tile([128, 2 * H], fp32, tag="eacc")
                                scrD = sb1.tile([128, C], fp32, tag="scrD", bufs=2)
                                scrA = sb1.tile([128, C], fp32, tag="scrA", bufs=2)
                                use_act = (t2 % 2 == 1)
                                for h in range(H):
                                    # U/S for head h directly in PSUM:
                                    #   DR dst-selection matmul of XR'' + identity matmul of xle
                                    pp = ps1.tile([128, C], fp32, tag="pp", bufs=2, name=f"ppe{h}_t{t}")
                                    for hf in range(HF):
                                        cl = h * C + hf * 512
                                        nc.tensor.matmul(
                                            pp[:, hf * 512:(hf + 1) * 512], eselw[:],
                                            xrdb8[:, 2 * pb:2 * pb + 2, cl:cl + 512],
                                            start=True, stop=False, perf_mode=DRM)
                                        nc.tensor.matmul(
                                            pp[:, hf * 512:(hf + 1) * 512], idm[:],
                                            xle[:, cl:cl + 512],
                                            start=False, stop=True)
                                    # scores: pos/neg relu accumulation off PSUM
                                    # (leaky = 0.8*relu(u) + 0.2*u; sum(u) rides the aug cols)
                                    np_h = npos[h]
                                    segs = [(0, np_h, 2 * h), (np_h, C - np_h, 2 * h + 1)]
                                    for off, ln, j in segs:
                                        if ln == 0:
                                            nc.vector.memset(eacc[:, j:j + 1], 0.0)
                                            continue
                                        if use_act:
                                            nc.scalar.activation(
                                                out=scrA[:, :ln], in_=pp[:, off:off + ln],
                                                func=AF.Relu, accum_out=eacc[:, j:j + 1])
                                        else:
                                            nc.vector.tensor_scalar(
                                                out=scrD[:, :ln], in0=pp[:, off:off + ln],
                                                scalar1=0.0, scalar2=0.0, op0=AT.max,
                                                op1=AT.add, accum_out=eacc[:, j:j + 1])
                                # aug sums: U aug cols in a small PSUM tile
                                ppa = ps1.tile([128, NAUG], fp32, tag="ppaug", bufs=1, name=f"ppa_t{t}")
                                nc.tensor.matmul(
                                    ppa[:], eselw[:], xrdb8[:, 2 * pb:2 * pb + 2, HC:HCA],
                                    start=True, stop=False, perf_mode=DRM)
                                nc.tensor.matmul(
                                    ppa[:], idm[:], xle[:, HC:HCA],
                                    start=False, stop=True)
                                aug8 = sb1.tile([128, NAUG], fp32, tag="aug8")
                                nc.scalar.activation(out=aug8[:], in_=ppa[:], func=AF.Copy)
                                rdif = sb1.tile([128, H], fp32, tag="rdif")
                                nc.vector.tensor_tensor(
                                    out=rdif[:], in0=eacc[:, 0:2 * H:2],
                                    in1=eacc[:, 1:2 * H:2], op=AT.subtract)
                                adif = sb1.tile([128, H], fp32, tag="adif")
                                nc.vector.tensor_tensor(
                                    out=adif[:], in0=aug8[:, 0:2 * H:2],
                                    in1=aug8[:, 1:2 * H:2], op=AT.subtract)
                                esc = sb1.tile([128, H], fp32, tag="esc")
                                nc.vector.scalar_tensor_tensor(
                                    out=esc[:], in0=adif[:],
                                    scalar=float(AUGDIV * NEG_SLOPE / (1.0 - NEG_SLOPE)),
                                    in1=rdif[:], op0=AT.mult, op1=AT.add)
                                nc.scalar.activation(
                                    out=a_all[:, t * H:(t + 1) * H], in_=esc[:],
                                    func=AF.Exp, scale=float(S * (1.0 - NEG_SLOPE)))
                            # pass A: denominators
                            pden = ps2.tile([128, H], fp32, tag="psmall", bufs=2, name=f"pden_{db}")
                            for t2 in range(T_BLK):
                                t = db * T_BLK + t2
                                ee = sbe.tile([128, 128], fp32, tag="esel_et", name=f"eet_{db}_{t2}")
                                nc.vector.scalar_tensor_tensor(
                                    out=ee[:], in0=dstcp[:, t:t + 1].to_broadcast([128, 128]),
                                    scalar=float(-db * 128), in1=iotaF[:],
                                    op0=AT.add, op1=AT.is_equal)
                                esels.append(ee)
                                nc.tensor.matmul(
                                    pden[:], ee[:], a_all[:, t * H:(t + 1) * H],
                                    start=(t2 == 0), stop=(t2 == T_BLK - 1))
                            dtmp = sb2.tile([128, H], fp32, tag="dtmp")
                            nc.vector.tensor_scalar_add(out=dtmp[:], in0=pden[:], scalar1=1e-16)
                            nc.vector.reciprocal(out=denr[:, db * H:(db + 1) * H], in_=dtmp[:])
                            # pass B: alpha and selection weights
                            for t2 in range(T_BLK):
                                t = db * T_BLK + t2
                                esde = sb2.tile([128, 128], fp32, tag="esde", bufs=4)
                                nc.vector.scalar_tensor_tensor(
                                    out=esde[:], in0=drows[t2][:],
                                    scalar=float(-db * 128), in1=iotaP[:],
                                    op0=AT.add, op1=AT.is_equal)
                                pad = ps2.tile([128, H], fp32, tag="psmall", bufs=2, name=f"pad_{db}_{t2}")
                                nc.tensor.matmul(pad[:], esde[:], denr[:, db * H:(db + 1) * H],
                                                 start=True, stop=True)
                                alpha = sb2.tile([128, H], fp32, tag="alpha")
                                nc.vector.tensor_tensor(out=alpha[:], in0=a_all[:, t * H:(t + 1) * H],
                                                        in1=pad[:], op=AT.mult)
                                hsal = []
                                for h in range(H):
                                    sal = sb2.tile([128, 128], bf16, tag=f"sal{h}",
                                                   bufs=T_BLK + 4, name=f"sal{h}_{db}_{t2}")
                                    nc.vector.tensor_tensor(
                                        out=sal[:], in0=esels[t2][:],
                                        in1=alpha[:, h:h + 1].to_broadcast([128, 128]),
                                        op=AT.mult)
                                    hsal.append(sal)
                                sals_all.append(hsal)
                            # transposed aggregation (xle-based: out = sum alpha*xl) + fc
                            NGRP = 4            # cc chunks per PSUM group (1 bank)
                            oaggs = []
                            for g in range(HC // 128 // NGRP):
                                pagg = ps2.tile([128, NGRP * 128], fp32, tag="pagg",
                                                bufs=1, name=f"pagg_{db}_{g}")
                                oagg = sb2.tile([128, NGRP * 128], bf16, tag="oagg", bufs=6,
                                                name=f"oagg_{db}_{g}")
                                for j in range(NGRP):
                                    cc = g * NGRP + j
                                    h = cc // (HC // 128 // H)
                                    for t2 in range(T_BLK):
                                        nc.tensor.matmul(
                                            pagg[:, j * 128:(j + 1) * 128],
                                            xles[t2][:, cc * 128:(cc + 1) * 128],
                                            sals_all[t2][h][:],
                                            start=(t2 == 0), stop=(t2 == T_BLK - 1))
                                nc.scalar.activation(out=oagg[:], in_=pagg[:], func=AF.Copy)
                                oaggs.append(oagg)
                            pfc = ps2.tile([128, NCLS_P], fp32, tag="psmall", bufs=2, name=f"pfc_{db}")
                            for cc in range(HC // 128):
                                nc.tensor.matmul(
                                    pfc[:], oaggs[cc // NGRP][:, (cc % NGRP) * 128:(cc % NGRP + 1) * 128],
                                    wfs_r[:, cc, :],
                                    start=(cc == 0), stop=False)
                            nc.tensor.matmul(pfc[:], ones1[:], bf2[:], start=False, stop=True)
                            # softmax
                            negmax = sb2.tile([128, 1], fp32, tag="negmax")
                            nc.vector.tensor_reduce(out=negmax[:], in_=pfc[:],
                                                    axis=mybir.AxisListType.X,
                                                    op=AT.max, negate=True)
                            pexp = sb2.tile([128, NCLS_P], fp32, tag="pexp", bufs=2)
                            nc.scalar.activation(out=pexp[:], in_=pfc[:], func=AF.Exp,
                                                 bias=negmax[:, 0:1], scale=1.0)
                            ssum = sb2.tile([128, 1], fp32, tag="ssum")
                            nc.vector.tensor_reduce(out=ssum[:], in_=pexp[:],
                                                    axis=mybir.AxisListType.X, op=AT.add)
                            rs = sb2.tile([128, 1], fp32, tag="rs")
                            nc.vector.reciprocal(out=rs[:], in_=ssum[:])
                            hout = sb2.tile([128, NCLS_P], fp32, tag="hout", bufs=2)
                            nc.vector.scalar_tensor_tensor(
                                out=hout[:], in0=pexp[:], scalar=rs[:, 0:1], in1=pexp[:],
                                op0=AT.mult, op1=AT.bypass)
                            nc.sync.dma_start(out=d_out[db * 128:(db + 1) * 128, :], in_=hout[:])
                        if debug:
                            nc.sync.dma_start(out=d_dbga[:], in_=a_all[:])
                            nc.sync.dma_start(out=d_dbgx[:], in_=xrdb8[:])

    nc.compile()
    return nc


def kernel(**inputs):
    out_full = np.zeros((N, NCLS), np.float32)
    in_maps, dims = _prep(
        inputs["x"], inputs["edge_index"], inputs["Wl"], inputs["bl"],
        inputs["Wr"], inputs["br"], inputs["att"], inputs["bias"],
        inputs["Wf"], inputs["bf"])
    nc = _build(dims)
    from concourse.bass_utils import run_bass_kernel_spmd
    res = run_bass_kernel_spmd(nc, in_maps, core_ids=list(range(NCORES)))
    for k in range(NCORES):
        out_full[k * ND:(k + 1) * ND, :] = res.results[k]["out"][:ND, :NCLS]
    return out_full


# revision 12
# speedup vs baseline: 5.4256x; 5.4256x over previous
"""GATv2 + softmax head for nn_GATModel_Softmax_4535485465120 on 8 trn2 NeuronCores.

v2: fp8-DoubleRow projections + xle-based aggregation (no XR-transpose phase).

Strategy (graph/data parallel by dst node, fully local — no collectives):
  - Nodes partitioned into 8 ranges of 1000 dst nodes (one per core).
  - Host preps per core: x.T columns (fp8) for the core's unique src nodes and
    its dst nodes; weights with att-magnitude (clamped at max/64 per head)
    folded into Wl/Wr columns, permuted pos-att-first, scaled 1/S for fp8;
    Wf rows carry the inverse permutation, S/(3*att_eff) un-scaling and the
    head-mean.
  - Device pipeline per core:
      phase 0:  XR'' = xdst8 @ Wr8 via fp8 DoubleRow matmuls -> SBUF fp8
                (resident, pair-packed by dst block for DR reuse)
      phase 0.5: XLu'' = xu8 @ Wl8 (DR) -> HBM bf16 [UP, 3072]
      phase 1 (per dst block, per 128-edge tile): U/S accumulated directly in
                PSUM = DR-selection-matmul of XR'' + identity-matmul of
                gathered XLu'' rows; leaky-relu + sign-segmented accumulation
                on DVE reads PSUM directly; a = exp(S * esc) on ScalarE.
      phase 2:  denom/alpha via selection matmuls; agg^T = sum_e alpha *
                xle (selection matmul, bf16); logits = agg^T-chunks @ Wf_stack
                (+folded bias row); row softmax -> output.

kernel(**inputs) takes FULL inputs, shards internally, returns FULL [8000,460] f32.
"""

import numpy as np
import ml_dtypes

BF16 = ml_dtypes.bfloat16
E4M3 = ml_dtypes.float8_e4m3   # IEEE e4m3: max 240, has inf (matches mybir float8e4)
F8MAX = 240.0

# Problem constants (hardcoded per spec)
N = 8000
DIN = 1024
H = 3
C = 1024
HC = H * C          # 3072
NCLS = 460
NCLS_P = 512
NEG_SLOPE = 0.2
NCORES = 8
ND = N // NCORES    # 1000 dst nodes per core
NDP = 1024          # padded dst count per core
DB = NDP // 128     # 8 dst blocks per core
P = 128
KC = DIN // P       # 8 contraction chunks (4 DoubleRow pairs)
NB = 3              # n-chunks of 1024 in HC (one per head)
HF = 2              # 512-wide matmul halves per 1024 chunk
NAUG = 8            # aug columns: per-head pos/neg sums (6) + pad (2)
HCA = HC + NAUG     # 3080
AUGDIV = 64.0       # aug columns scaled by 1/64 to stay in fp8 range


def _prep(x, edge_index, Wl, bl, Wr, br, att, bias, Wf, bf):
    """Host-side preprocessing -> per-core input maps + static dims."""
    x = np.asarray(x, np.float32)
    ei = np.asarray(edge_index).astype(np.int64)
    Wl = np.asarray(Wl, np.float32)
    Wr = np.asarray(Wr, np.float32)
    bl = np.asarray(bl, np.float32)
    br = np.asarray(br, np.float32)
    att = np.asarray(att, np.float32)
    bias = np.asarray(bias, np.float32)
    Wf = np.asarray(Wf, np.float32)
    bf = np.asarray(bf, np.float32)

    assert np.all(bl == 0) and np.all(br == 0), \
        "nonzero bl/br not supported by this kernel build"

    loops = np.arange(N, dtype=np.int64)
    src_all = np.concatenate([ei[:, 0], loops])
    dst_all = np.concatenate([ei[:, 1], loops])

    # att folding: per head, column scale att_eff (clamped so fp8 columns
    # stay out of the subnormal range) and permutation pos-first
    absatt = np.abs(att)                       # [H, C]
    att_eff = np.maximum(absatt, absatt.max(1, keepdims=True) / 64.0)
    perm = np.zeros((H, C), np.int64)          # perm[h, newc] = origc
    npos = np.zeros(H, np.int64)
    for h in range(H):
        pos = np.nonzero(att[h] > 0)[0]
        neg = np.nonzero(att[h] <= 0)[0]
        perm[h] = np.concatenate([pos, neg])
        npos[h] = len(pos)

    # scaled/permuted projection weights  [DIN, HC]
    Wl_s = np.zeros((DIN, HC), np.float32)
    Wr_s = np.zeros((DIN, HC), np.float32)
    for h in range(H):
        sc = att_eff[h, perm[h]]               # [C]
        Wl_s[:, h * C:(h + 1) * C] = Wl[:, h * C + perm[h]] * sc
        Wr_s[:, h * C:(h + 1) * C] = Wr[:, h * C + perm[h]] * sc

    # global fp8 scale S: covers weight absmax and a 6-sigma bound on the
    # projected activations (xr'' must fit fp8 storage after the matmul)
    colnorm = max(np.linalg.norm(Wl_s, axis=0).max(),
                  np.linalg.norm(Wr_s, axis=0).max())
    S = float(max(np.abs(Wl_s).max() / F8MAX, np.abs(Wr_s).max() / F8MAX,
                  colnorm * 7.0 / F8MAX))

    def with_aug(W):
        W8 = np.clip(W / S, -F8MAX, F8MAX).astype(E4M3)
        Wa = np.zeros((DIN, HCA), E4M3)
        Wa[:, :HC] = W8
        W8f = W8.astype(np.float32)
        for h in range(H):
            np_h = int(npos[h])
            Wa[:, HC + 2 * h] = (W8f[:, h * C:h * C + np_h].sum(1) / AUGDIV).astype(E4M3)
            Wa[:, HC + 2 * h + 1] = (W8f[:, h * C + np_h:(h + 1) * C].sum(1) / AUGDIV).astype(E4M3)
        return Wa
    wl8 = with_aug(Wl_s)
    wr8 = with_aug(Wr_s)

    # final fc stack: logits = sum_h (agg''_h * S/(3*att_eff)) @ Wf  (+ bias@Wf + bf)
    Wfs = np.zeros((HC, NCLS_P), np.float32)
    for h in range(H):
        sc = S / (3.0 * att_eff[h, perm[h]])
        Wfs[h * C:(h + 1) * C, :NCLS] = Wf[perm[h]] * sc[:, None]
    bf2 = np.full((1, NCLS_P), -1e30, np.float32)
    bf2[0, :NCLS] = bias @ Wf + bf

    xT8 = np.clip(np.ascontiguousarray(x.T), -F8MAX, F8MAX).astype(E4M3)     # [DIN, N]

    # per-core edge grouping: edges (incl. self loops) by dst block
    cores = []
    tmax = 1
    for k in range(NCORES):
        lo, hi = k * ND, (k + 1) * ND
        m = (dst_all >= lo) & (dst_all < hi)
        s_k = src_all[m]
        dl_k = (dst_all[m] - lo).astype(np.int64)
        order = np.argsort(dl_k, kind="stable")
        s_k, dl_k = s_k[order], dl_k[order]
        blocks = []
        for db in range(DB):
            bm = (dl_k >= db * 128) & (dl_k < (db + 1) * 128)
            blocks.append((s_k[bm], dl_k[bm]))
            tmax = max(tmax, (len(blocks[-1][0]) + 127) // 128)
        cores.append(blocks)

    T_BLK = tmax
    E1T = DB * T_BLK
    E1P = E1T * 128

    iotaF = np.tile(np.arange(128, dtype=np.float32)[None, :], (128, 1))
    iotaP = np.tile(np.arange(128, dtype=np.float32)[:, None], (1, 128))
    ones1 = np.ones((1, 128), BF16)
    # unique srcs per core -> common padded tile count
    uniq = []
    for k in range(NCORES):
        srcs = np.concatenate([cores[k][db][0] for db in range(DB)])
        uniq.append(np.unique(srcs))
    UT = max((len(u) + 127) // 128 for u in uniq)
    UP = UT * 128
    in_maps = []
    for k in range(NCORES):
        srcslot = np.zeros(E1P, np.int64)
        real = np.zeros(E1P, bool)
        dstloc = np.full(E1P, -1.0, np.float32)
        for db in range(DB):
            s_k, dl_k = cores[k][db]
            base = db * T_BLK * 128
            srcslot[base:base + len(s_k)] = s_k
            real[base:base + len(s_k)] = True
            dstloc[base:base + len(s_k)] = dl_k.astype(np.float32)
        u = uniq[k]
        xuT = np.zeros((DIN, UP), E4M3)
        xuT[:, :len(u)] = xT8[:, u]
        srcloc = np.zeros((E1P, 1), np.int32)
        srcloc[real, 0] = np.searchsorted(u, srcslot[real]).astype(np.int32)
        xdstT = np.zeros((DIN, NDP), E4M3)
        xdstT[:, :ND] = xT8[:, k * ND:(k + 1) * ND]
        dst_col = np.ascontiguousarray(dstloc.reshape(E1T, 128).T)   # [128, E1T]
        dst_row = np.tile(dstloc[None, :], (128, 1))                 # [128, E1P]
        in_maps.append({
            "xuT": xuT,
            "srcloc": srcloc,
            "xdstT": xdstT,
            "wl": wl8,
            "wr": wr8,
            "wfs": Wfs.astype(BF16),
            "bf2": bf2.astype(BF16),
            "dstcp": dst_col,
            "dstrow": dst_row,
            "iotaF": iotaF,
            "iotaP": iotaP,
            "ones1": ones1,
        })
    dims = {"T_BLK": T_BLK, "E1T": E1T, "E1P": E1P, "UT": UT,
            "npos": [int(v) for v in npos], "S": S}
    return in_maps, dims


def _build(dims, debug=False):
    """Trace the Bass/Tile program (identical for all cores)."""
    import concourse.bass as bass
    import concourse.mybir as mybir
    import concourse.tile as tile
    from concourse import bacc

    T_BLK, E1T, E1P = dims["T_BLK"], dims["E1T"], dims["E1P"]
    UT = dims["UT"]
    npos = dims["npos"]
    S = dims["S"]
    UP = UT * 128
    fp32 = mybir.dt.float32
    bf16 = mybir.dt.bfloat16
    fp8 = mybir.dt.float8e4
    AT = mybir.AluOpType
    AF = mybir.ActivationFunctionType
    DRM = mybir.MatmulPerfMode.DoubleRow

    nc = bacc.Bacc("TRN2", target_bir_lowering=False, debug=False)

    with tile.TileContext(nc) as tc:
        with tc.tile_pool(name="dram", bufs=1, space="DRAM") as dram:
            d_xuT = dram.tile([DIN, UP], fp8, kind="ExternalInput", name="xuT", uniquify=False)
            d_srcloc = dram.tile([E1P, 1], mybir.dt.int32, kind="ExternalInput", name="srcloc", uniquify=False)
            d_xdstT = dram.tile([DIN, NDP], fp8, kind="ExternalInput", name="xdstT", uniquify=False)
            d_wl = dram.tile([DIN, HCA], fp8, kind="ExternalInput", name="wl", uniquify=False)
            d_wr = dram.tile([DIN, HCA], fp8, kind="ExternalInput", name="wr", uniquify=False)
            d_wfs = dram.tile([HC, NCLS_P], bf16, kind="ExternalInput", name="wfs", uniquify=False)
            d_bf2 = dram.tile([1, NCLS_P], bf16, kind="ExternalInput", name="bf2", uniquify=False)
            d_dstcp = dram.tile([128, E1T], fp32, kind="ExternalInput", name="dstcp", uniquify=False)
            d_dstrow = dram.tile([128, E1P], fp32, kind="ExternalInput", name="dstrow", uniquify=False)
            d_iotaF = dram.tile([128, 128], fp32, kind="ExternalInput", name="iotaF", uniquify=False)
            d_iotaP = dram.tile([128, 128], fp32, kind="ExternalInput", name="iotaP", uniquify=False)
            d_ones1 = dram.tile([1, 128], bf16, kind="ExternalInput", name="ones1", uniquify=False)
            d_out = dram.tile([NDP, NCLS_P], fp32, kind="ExternalOutput", name="out", uniquify=False)
            d_xlu = dram.tile([UP, HCA], bf16, name="xlu_i",
                              kind="ExternalOutput" if debug else "Internal",
                              uniquify=False)
            if debug:
                d_dbga = dram.tile([128, E1T * H], fp32, kind="ExternalOutput", name="dbg_a", uniquify=False)
                d_dbgx = dram.tile([128, DB, HCA], fp8, kind="ExternalOutput", name="dbg_xrd", uniquify=False)

            with tc.tile_pool(name="gsb", bufs=1) as gsb:
                # resident tensors
                dstcp = gsb.tile([128, E1T], fp32, name="dstcp_r")
                nc.sync.dma_start(out=dstcp[:], in_=d_dstcp[:])
                iotaF = gsb.tile([128, 128], fp32, name="iotaF_r")
                nc.sync.dma_start(out=iotaF[:], in_=d_iotaF[:])
                iotaP = gsb.tile([128, 128], fp32, name="iotaP_r")
                nc.sync.dma_start(out=iotaP[:], in_=d_iotaP[:])
                ones1 = gsb.tile([1, 128], bf16, name="ones1_r")
                nc.sync.dma_start(out=ones1[:], in_=d_ones1[:])
                bf2 = gsb.tile([1, NCLS_P], bf16, name="bf2_r")
                nc.sync.dma_start(out=bf2[:], in_=d_bf2[:])
                a_all = gsb.tile([128, E1T * H], fp32, name="a_all_r")
                denr = gsb.tile([128, DB * H], fp32, name="denr_r")
                xrdb8 = gsb.tile([128, DB, HCA], fp8, name="xrdb8_r")
                idm = gsb.tile([128, 128], bf16, name="idm_r")
                nc.vector.scalar_tensor_tensor(
                    out=idm[:], in0=iotaP[:], scalar=0.0, in1=iotaF[:],
                    op0=AT.add, op1=AT.is_equal)

                # ---------------- phase 0: XR'' projection (fp8 DR) ----------------
                with tc.tile_pool(name="p01", bufs=1, space="PSUM") as ps1:
                    with tc.tile_pool(name="wpool", bufs=1) as wpool:
                        wmat = wpool.tile([128, KC, HCA], fp8, tag="wmat", bufs=1, name="wmat_r")
                        nc.sync.dma_start(out=wmat[:], in_=d_wr[:].rearrange("(kc p) n -> p kc n", p=128))
                        xdst8 = wpool.tile([128, KC, NDP], fp8, tag="xdst8", bufs=1, name="xdst8_r")
                        nc.sync.dma_start(out=xdst8[:], in_=d_xdstT[:].rearrange("(kc p) n -> p kc n", p=128))
                        for dc in range(DB):
                            for nb in range(NB):
                                pp = ps1.tile([128, C], fp32, tag="pp", bufs=2, name=f"pp{nb}_c{dc}")
                                for kp in range(KC // 2):
                                    for hf in range(HF):
                                        nc.tensor.matmul(
                                            pp[:, hf * 512:(hf + 1) * 512],
                                            xdst8[:, 2 * kp:2 * kp + 2, dc * 128:(dc + 1) * 128],
                                            wmat[:, 2 * kp:2 * kp + 2, nb * C + hf * 512:nb * C + (hf + 1) * 512],
                                            start=(kp == 0), stop=(kp == KC // 2 - 1),
                                            perf_mode=DRM)
                                nc.vector.tensor_copy(out=xrdb8[:, dc, nb * C:(nb + 1) * C],
                                                      in_=pp[:])
                            ppa = ps1.tile([128, NAUG], fp32, tag="ppaug", bufs=1, name=f"ppa_c{dc}")
                            for kp in range(KC // 2):
                                nc.tensor.matmul(
                                    ppa[:], xdst8[:, 2 * kp:2 * kp + 2, dc * 128:(dc + 1) * 128],
                                    wmat[:, 2 * kp:2 * kp + 2, HC:HCA],
                                    start=(kp == 0), stop=(kp == KC // 2 - 1),
                                    perf_mode=DRM)
                            nc.vector.tensor_copy(out=xrdb8[:, dc, HC:HCA], in_=ppa[:])

                        # ------- phase 0.5: XLu'' projection of unique srcs (fp8 DR) -------
                        wmat2 = wpool.tile([128, KC, HCA], fp8, tag="wmat2", bufs=1, name="wmat_r2")
                        nc.sync.dma_start(out=wmat2[:], in_=d_wl[:].rearrange("(kc p) n -> p kc n", p=128))
                        xu8 = wpool.tile([128, KC, UP], fp8, tag="xu8", bufs=1, name="xu8_r")
                        nc.sync.dma_start(out=xu8[:], in_=d_xuT[:].rearrange("(kc p) n -> p kc n", p=128))
                        with tc.tile_pool(name="sb0", bufs=2) as sb0:
                            for ut in range(UT):
                                xl_sb = sb0.tile([128, HCA], bf16, tag="xl_sb", bufs=3)
                                for nb in range(NB):
                                    pp = ps1.tile([128, C], fp32, tag="pp", bufs=2, name=f"ppu{nb}_{ut}")
                                    for kp in range(KC // 2):
                                        for hf in range(HF):
                                            nc.tensor.matmul(
                                                pp[:, hf * 512:(hf + 1) * 512],
                                                xu8[:, 2 * kp:2 * kp + 2, ut * 128:(ut + 1) * 128],
                                                wmat2[:, 2 * kp:2 * kp + 2, nb * C + hf * 512:nb * C + (hf + 1) * 512],
                                                start=(kp == 0), stop=(kp == KC // 2 - 1),
                                                perf_mode=DRM)
                                    if nb == 0:
                                        nc.vector.tensor_copy(out=xl_sb[:, nb * C:(nb + 1) * C], in_=pp[:])
                                    else:
                                        nc.scalar.activation(out=xl_sb[:, nb * C:(nb + 1) * C],
                                                             in_=pp[:], func=AF.Copy)
                                ppa = ps1.tile([128, NAUG], fp32, tag="ppaug", bufs=1, name=f"ppa_u{ut}")
                                for kp in range(KC // 2):
                                    nc.tensor.matmul(
                                        ppa[:], xu8[:, 2 * kp:2 * kp + 2, ut * 128:(ut + 1) * 128],
                                        wmat2[:, 2 * kp:2 * kp + 2, HC:HCA],
                                        start=(kp == 0), stop=(kp == KC // 2 - 1),
                                        perf_mode=DRM)
                                nc.scalar.activation(out=xl_sb[:, HC:HCA], in_=ppa[:], func=AF.Copy)
                                nc.sync.dma_start(out=d_xlu[ut * 128:(ut + 1) * 128, :], in_=xl_sb[:])

                    # wfs resident (phase-0 weight slots are free now)
                    wfs_r = gsb.tile([128, HC // 128, NCLS_P], bf16, name="wfs_r")
                    nc.sync.dma_start(out=wfs_r[:], in_=d_wfs[:].rearrange("(cc p) n -> p cc n", p=128))

                    # ------- fused phases 1+2: per dst block -------
                    with (
                        tc.tile_pool(name="sb1", bufs=3) as sb1,
                        tc.tile_pool(name="sbe", bufs=T_BLK + 4) as sbe,
                        tc.tile_pool(name="p2", bufs=1, space="PSUM") as ps2,
                        tc.tile_pool(name="sb2", bufs=3) as sb2,
                    ):
                        for db in range(DB):
                            pb = db // 2
                            jact = db % 2
                            xles, esels, sals_all, drows = [], [], [], []
                            for t2 in range(T_BLK):
                                t = db * T_BLK + t2
                                sidx = sb1.tile([128, 1], mybir.dt.int32, tag="sidx")
                                nc.sync.dma_start(out=sidx[:], in_=d_srcloc[t * 128:(t + 1) * 128, :])
                                drow = sbe.tile([128, 128], fp32, tag="drow", name=f"drow_{db}_{t2}")
                                nc.sync.dma_start(out=drow[:], in_=d_dstrow[:, t * 128:(t + 1) * 128])
                                drows.append(drow)
                                # gather XL''[src] rows (bf16)
                                xle = sbe.tile([128, HCA], bf16, tag="xle", bufs=2 * T_BLK + 1, name=f"xle_{db}_{t2}")
                                nc.gpsimd.indirect_dma_start(
                                    out=xle[:], out_offset=None, in_=d_xlu[:],
                                    in_offset=bass.IndirectOffsetOnAxis(ap=sidx[:, :1], axis=0))
                                xles.append(xle)
                                # pair-packed 0/1 dst-selection for DoubleRow U-expansion
                                eselw = sb1.tile([128, 2, 128], fp8, tag="eselw")
                                nc.gpsimd.memset(eselw[:, 1 - jact, :], 0.0)
                                nc.vector.scalar_tensor_tensor(
                                    out=eselw[:, jact, :], in0=drow[:],
                                    scalar=float(-db * 128), in1=iotaP[:],
                                    op0=AT.add, op1=AT.is_equal)
                                eacc = sb1.tile([128, 2 * H], fp32, tag="eacc")
                                scrD = sb1.tile([128, C], fp32, tag="scrD", bufs=2)
                                scrA = sb1.tile([128, C], fp32, tag="scrA", bufs=2)
                                use_act = (t2 % 2 == 0)
                                for h in range(H):
                                    # U/S for head h directly in PSUM:
                                    #   DR dst-selection matmul of XR'' + identity matmul of xle
                                    pp = ps1.tile([128, C], fp32, tag="pp", bufs=2, name=f"ppe{h}_t{t}")
                                    for hf in range(HF):
                                        cl = h * C + hf * 512
                                        nc.tensor.matmul(
                                            pp[:, hf * 512:(hf + 1) * 512], eselw[:],
                                            xrdb8[:, 2 * pb:2 * pb + 2, cl:cl + 512],
                                            start=True, stop=False, perf_mode=DRM)
                                        nc.tensor.matmul(
                                            pp[:, hf * 512:(hf + 1) * 512], idm[:],
                                            xle[:, cl:cl + 512],
                                            start=False, stop=True)
                                    # scores: pos/neg relu accumulation off PSUM
                                    # (leaky = 0.8*relu(u) + 0.2*u; sum(u) rides the aug cols)
                                    np_h = npos[h]
                                    segs = [(0, np_h, 2 * h), (np_h, C - np_h, 2 * h + 1)]
                                    for off, ln, j in segs:
                                        if ln == 0:
                                            nc.vector.memset(eacc[:, j:j + 1], 0.0)
                                            continue
                                        if use_act:
                                            nc.scalar.activation(
                                                out=scrA[:, :ln], in_=pp[:, off:off + ln],
                                                func=AF.Relu, accum_out=eacc[:, j:j + 1])
                                        else:
                                            nc.vector.tensor_scalar(
                                                out=scrD[:, :ln], in0=pp[:, off:off + ln],
                                                scalar1=0.0, scalar2=0.0, op0=AT.max,
                                                op1=AT.add, accum_out=eacc[:, j:j + 1])
                                # aug sums: U aug cols in a small PSUM tile
                                ppa = ps1.tile([128, NAUG], fp32, tag="ppaug", bufs=1, name=f"ppa_t{t}")
                                nc.tensor.matmul(
                                    ppa[:], eselw[:], xrdb8[:, 2 * pb:2 * pb + 2, HC:HCA],
                                    start=True, stop=False, perf_mode=DRM)
                                nc.tensor.matmul(
                                    ppa[:], idm[:], xle[:, HC:HCA],
                                    start=False, stop=True)
                                aug8 = sb1.tile([128, NAUG], fp32, tag="aug8")
                                nc.scalar.activation(out=aug8[:], in_=ppa[:], func=AF.Copy)
                                rdif = sb1.tile([128, H], fp32, tag="rdif")
                                nc.vector.tensor_tensor(
                                    out=rdif[:], in0=eacc[:, 0:2 * H:2],
                                    in1=eacc[:, 1:2 * H:2], op=AT.subtract)
                                adif = sb1.tile([128, H], fp32, tag="adif")
                                nc.vector.tensor_tensor(
                                    out=adif[:], in0=aug8[:, 0:2 * H:2],
                                    in1=aug8[:, 1:2 * H:2], op=AT.subtract)
                                esc = sb1.tile([128, H], fp32, tag="esc")
                                nc.vector.scalar_tensor_tensor(
                                    out=esc[:], in0=adif[:],
                                    scalar=float(AUGDIV * NEG_SLOPE / (1.0 - NEG_SLOPE)),
                                    in1=rdif[:], op0=AT.mult, op1=AT.add)
                                nc.scalar.activation(
                                    out=a_all[:, t * H:(t + 1) * H], in_=esc[:],
                                    func=AF.Exp, scale=float(S * (1.0 - NEG_SLOPE)))
                            # pass A: denominators
                            pden = ps2.tile([128, H], fp32, tag="psmall", bufs=2, name=f"pden_{db}")
                            for t2 in range(T_BLK):
                                t = db * T_BLK + t2
                                ee = sbe.tile([128, 128], fp32, tag="esel_et", name=f"eet_{db}_{t2}")
                                nc.vector.scalar_tensor_tensor(
                                    out=ee[:], in0=dstcp[:, t:t + 1].to_broadcast([128, 128]),
                                    scalar=float(-db * 128), in1=iotaF[:],
                                    op0=AT.add, op1=AT.is_equal)
                                esels.append(ee)
                                nc.tensor.matmul(
                                    pden[:], ee[:], a_all[:, t * H:(t + 1) * H],
                                    start=(t2 == 0), stop=(t2 == T_BLK - 1))
                            dtmp = sb2.tile([128, H], fp32, tag="dtmp")
                            nc.vector.tensor_scalar_add(out=dtmp[:], in0=pden[:], scalar1=1e-16)
                            nc.vector.reciprocal(out=denr[:, db * H:(db + 1) * H], in_=dtmp[:])
                            # pass B: alpha and selection weights
                            for t2 in range(T_BLK):
                                t = db * T_BLK + t2
                                esde = sb2.tile([128, 128], fp32, tag="esde", bufs=4)
                                nc.vector.scalar_tensor_tensor(
                                    out=esde[:], in0=drows[t2][:],
                                    scalar=float(-db * 128), in1=iotaP[:],
                                    op0=AT.add, op1=AT.is_equal)
                                pad = ps2.tile([128, H], fp32, tag="psmall", bufs=2, name=f"pad_{db}_{t2}")
                                nc.tensor.matmul(pad[:], esde[:], denr[:, db * H:(db + 1) * H],
                                                 start=True, stop=True)
                                alpha = sb2.tile([128, H], fp32, tag="alpha")
                                nc.vector.tensor_tensor(out=alpha[:], in0=a_all[:, t * H:(t + 1) * H],
                                                        in1=pad[:], op=AT.mult)
                                hsal = []
                                for h in range(H):
                                    sal = sb2.tile([128, 128], bf16, tag=f"sal{h}",
                                                   bufs=2 * T_BLK + 1, name=f"sal{h}_{db}_{t2}")
                                    nc.gpsimd.tensor_tensor(
                                        out=sal[:], in0=esels[t2][:],
                                        in1=alpha[:, h:h + 1].to_broadcast([128, 128]),
                                        op=AT.mult)
                                    hsal.append(sal)
                                sals_all.append(hsal)
                            # transposed aggregation (xle-based: out = sum alpha*xl) + fc
                            NGRP = 4            # cc chunks per PSUM group (1 bank)
                            oaggs = []
                            for g in range(HC // 128 // NGRP):
                                pagg = ps2.tile([128, NGRP * 128], fp32, tag="pagg",
                                                bufs=1, name=f"pagg_{db}_{g}")
                                oagg = sb2.tile([128, NGRP * 128], bf16, tag="oagg", bufs=6,
                                                name=f"oagg_{db}_{g}")
                                for j in range(NGRP):
                                    cc = g * NGRP + j
                                    h = cc // (HC // 128 // H)
                                    for t2 in range(T_BLK):
                                        nc.tensor.matmul(
                                            pagg[:, j * 128:(j + 1) * 128],
                                            xles[t2][:, cc * 128:(cc + 1) * 128],
                                            sals_all[t2][h][:],
                                            start=(t2 == 0), stop=(t2 == T_BLK - 1))
                                if g % 2 == 0:
                                    nc.scalar.activation(out=oagg[:], in_=pagg[:], func=AF.Copy)
                                else:
                                    nc.vector.tensor_copy(out=oagg[:], in_=pagg[:])
                                oaggs.append(oagg)
                            pfc = ps2.tile([128, NCLS_P], fp32, tag="psmall", bufs=2, name=f"pfc_{db}")
                            for cc in range(HC // 128):
                                nc.tensor.matmul(
                                    pfc[:], oaggs[cc // NGRP][:, (cc % NGRP) * 128:(cc % NGRP + 1) * 128],
                                    wfs_r[:, cc, :],
                                    start=(cc == 0), stop=False)
                            nc.tensor.matmul(pfc[:], ones1[:], bf2[:], start=False, stop=True)
                            # softmax
                            negmax = sb2.tile([128, 1], fp32, tag="negmax")
                            nc.vector.tensor_reduce(out=negmax[:], in_=pfc[:],
                                                    axis=mybir.AxisListType.X,
                                                    op=AT.max, negate=True)
                            pexp = sb2.tile([128, NCLS_P], fp32, tag="pexp", bufs=2)
                            nc.scalar.activation(out=pexp[:], in_=pfc[:], func=AF.Exp,
                                                 bias=negmax[:, 0:1], scale=1.0)
                            ssum = sb2.tile([128, 1], fp32, tag="ssum")
                            nc.vector.tensor_reduce(out=ssum[:], in_=pexp[:],
                                                    axis=mybir.AxisListType.X, op=AT.add)
                            rs = sb2.tile([128, 1], fp32, tag="rs")
                            nc.vector.reciprocal(out=rs[:], in_=ssum[:])
                            hout = sb2.tile([128, NCLS_P], fp32, tag="hout", bufs=2)
                            nc.vector.scalar_tensor_tensor(
                                out=hout[:], in0=pexp[:], scalar=rs[:, 0:1], in1=pexp[:],
                                op0=AT.mult, op1=AT.bypass)
                            nc.sync.dma_start(out=d_out[db * 128:(db + 1) * 128, :], in_=hout[:])
                        if debug:
                            nc.sync.dma_start(out=d_dbga[:], in_=a_all[:])
                            nc.sync.dma_start(out=d_dbgx[:], in_=xrdb8[:])

    nc.compile()
    return nc


def kernel(**inputs):
    out_full = np.zeros((N, NCLS), np.float32)
    in_maps, dims = _prep(
        inputs["x"], inputs["edge_index"], inputs["Wl"], inputs["bl"],
        inputs["Wr"], inputs["br"], inputs["att"], inputs["bias"],
        inputs["Wf"], inputs["bf"])
    nc = _build(dims)
    from concourse.bass_utils import run_bass_kernel_spmd
    res = run_bass_kernel_spmd(nc, in_maps, core_ids=list(range(NCORES)))
    for k in range(NCORES):
        out_full[k * ND:(k + 1) * ND, :] = res.results[k]["out"][:ND, :NCLS]
    return out_full


# revision 16
# speedup vs baseline: 5.6783x; 1.0466x over previous
"""GATv2 + softmax head for nn_GATModel_Softmax_4535485465120 on 8 trn2 NeuronCores.

v2: fp8-DoubleRow projections + xle-based aggregation (no XR-transpose phase).

Strategy (graph/data parallel by dst node, fully local — no collectives):
  - Nodes partitioned into 8 ranges of 1000 dst nodes (one per core).
  - Host preps per core: x.T columns (fp8) for the core's unique src nodes and
    its dst nodes; weights with att-magnitude (clamped at max/64 per head)
    folded into Wl/Wr columns, permuted pos-att-first, scaled 1/S for fp8;
    Wf rows carry the inverse permutation, S/(3*att_eff) un-scaling and the
    head-mean.
  - Device pipeline per core:
      phase 0:  XR'' = xdst8 @ Wr8 via fp8 DoubleRow matmuls -> SBUF fp8
                (resident, pair-packed by dst block for DR reuse)
      phase 0.5: XLu'' = xu8 @ Wl8 (DR) -> HBM bf16 [UP, 3072]
      phase 1 (per dst block, per 128-edge tile): U/S accumulated directly in
                PSUM = DR-selection-matmul of XR'' + identity-matmul of
                gathered XLu'' rows; leaky-relu + sign-segmented accumulation
                on DVE reads PSUM directly; a = exp(S * esc) on ScalarE.
      phase 2:  denom/alpha via selection matmuls; agg^T = sum_e alpha *
                xle (selection matmul, bf16); logits = agg^T-chunks @ Wf_stack
                (+folded bias row); row softmax -> output.

kernel(**inputs) takes FULL inputs, shards internally, returns FULL [8000,460] f32.
"""

import numpy as np
import ml_dtypes

BF16 = ml_dtypes.bfloat16
E4M3 = ml_dtypes.float8_e4m3   # IEEE e4m3: max 240, has inf (matches mybir float8e4)
F8MAX = 240.0

# Problem constants (hardcoded per spec)
N = 8000
DIN = 1024
H = 3
C = 1024
HC = H * C          # 3072
NCLS = 460
NCLS_P = 512
NEG_SLOPE = 0.2
NCORES = 8
ND = N // NCORES    # 1000 dst nodes per core
NDP = 1024          # padded dst count per core
DB = NDP // 128     # 8 dst blocks per core
P = 128
KC = DIN // P       # 8 contraction chunks (4 DoubleRow pairs)
NB = 3              # n-chunks of 1024 in HC (one per head)
HF = 2              # 512-wide matmul halves per 1024 chunk
NAUG = 8            # aug columns: per-head pos/neg sums (6) + pad (2)
HCA = HC + NAUG     # 3080
AUGDIV = 64.0       # aug columns scaled by 1/64 to stay in fp8 range


def _prep(x, edge_index, Wl, bl, Wr, br, att, bias, Wf, bf):
    """Host-side preprocessing -> per-core input maps + static dims."""
    x = np.asarray(x, np.float32)
    ei = np.asarray(edge_index).astype(np.int64)
    Wl = np.asarray(Wl, np.float32)
    Wr = np.asarray(Wr, np.float32)
    bl = np.asarray(bl, np.float32)
    br = np.asarray(br, np.float32)
    att = np.asarray(att, np.float32)
    bias = np.asarray(bias, np.float32)
    Wf = np.asarray(Wf, np.float32)
    bf = np.asarray(bf, np.float32)

    assert np.all(bl == 0) and np.all(br == 0), \
        "nonzero bl/br not supported by this kernel build"

    loops = np.arange(N, dtype=np.int64)
    src_all = np.concatenate([ei[:, 0], loops])
    dst_all = np.concatenate([ei[:, 1], loops])

    # att folding: per head, column scale att_eff (clamped so fp8 columns
    # stay out of the subnormal range) and permutation pos-first
    absatt = np.abs(att)                       # [H, C]
    att_eff = np.maximum(absatt, absatt.max(1, keepdims=True) / 64.0)
    perm = np.zeros((H, C), np.int64)          # perm[h, newc] = origc
    npos = np.zeros(H, np.int64)
    for h in range(H):
        pos = np.nonzero(att[h] > 0)[0]
        neg = np.nonzero(att[h] <= 0)[0]
        perm[h] = np.concatenate([pos, neg])
        npos[h] = len(pos)

    # scaled/permuted projection weights  [DIN, HC]
    Wl_s = np.zeros((DIN, HC), np.float32)
    Wr_s = np.zeros((DIN, HC), np.float32)
    for h in range(H):
        sc = att_eff[h, perm[h]]               # [C]
        Wl_s[:, h * C:(h + 1) * C] = Wl[:, h * C + perm[h]] * sc
        Wr_s[:, h * C:(h + 1) * C] = Wr[:, h * C + perm[h]] * sc

    # global fp8 scale S: covers weight absmax and a 6-sigma bound on the
    # projected activations (xr'' must fit fp8 storage after the matmul)
    colnorm = max(np.linalg.norm(Wl_s, axis=0).max(),
                  np.linalg.norm(Wr_s, axis=0).max())
    S = float(max(np.abs(Wl_s).max() / F8MAX, np.abs(Wr_s).max() / F8MAX,
                  colnorm * 7.0 / F8MAX))

    def with_aug(W):
        W8 = np.clip(W / S, -F8MAX, F8MAX).astype(E4M3)
        Wa = np.zeros((DIN, HCA), E4M3)
        Wa[:, :HC] = W8
        W8f = W8.astype(np.float32)
        for h in range(H):
            np_h = int(npos[h])
            Wa[:, HC + 2 * h] = (W8f[:, h * C:h * C + np_h].sum(1) / AUGDIV).astype(E4M3)
            Wa[:, HC + 2 * h + 1] = (W8f[:, h * C + np_h:(h + 1) * C].sum(1) / AUGDIV).astype(E4M3)
        return Wa
    wl8 = with_aug(Wl_s)
    wr8 = with_aug(Wr_s)

    # final fc stack: logits = sum_h (agg''_h * S/(3*att_eff)) @ Wf  (+ bias@Wf + bf)
    Wfs = np.zeros((HC, NCLS_P), np.float32)
    for h in range(H):
        sc = S / (3.0 * att_eff[h, perm[h]])
        Wfs[h * C:(h + 1) * C, :NCLS] = Wf[perm[h]] * sc[:, None]
    bf2 = np.full((1, NCLS_P), -1e30, np.float32)
    bf2[0, :NCLS] = bias @ Wf + bf

    xT8 = np.clip(np.ascontiguousarray(x.T), -F8MAX, F8MAX).astype(E4M3)     # [DIN, N]

    # per-core edge grouping: edges (incl. self loops) by dst block
    cores = []
    tmax = 1
    for k in range(NCORES):
        lo, hi = k * ND, (k + 1) * ND
        m = (dst_all >= lo) & (dst_all < hi)
        s_k = src_all[m]
        dl_k = (dst_all[m] - lo).astype(np.int64)
        order = np.argsort(dl_k, kind="stable")
        s_k, dl_k = s_k[order], dl_k[order]
        blocks = []
        for db in range(DB):
            bm = (dl_k >= db * 128) & (dl_k < (db + 1) * 128)
            blocks.append((s_k[bm], dl_k[bm]))
            tmax = max(tmax, (len(blocks[-1][0]) + 127) // 128)
        cores.append(blocks)

    T_BLK = tmax
    E1T = DB * T_BLK
    E1P = E1T * 128

    iotaF = np.tile(np.arange(128, dtype=np.float32)[None, :], (128, 1))
    iotaP = np.tile(np.arange(128, dtype=np.float32)[:, None], (1, 128))
    ones1 = np.ones((1, 128), BF16)
    # unique srcs per core -> common padded tile count
    uniq = []
    for k in range(NCORES):
        srcs = np.concatenate([cores[k][db][0] for db in range(DB)])
        uniq.append(np.unique(srcs))
    UT = max((len(u) + 127) // 128 for u in uniq)
    UP = UT * 128
    in_maps = []
    for k in range(NCORES):
        srcslot = np.zeros(E1P, np.int64)
        real = np.zeros(E1P, bool)
        dstloc = np.full(E1P, -1.0, np.float32)
        for db in range(DB):
            s_k, dl_k = cores[k][db]
            base = db * T_BLK * 128
            srcslot[base:base + len(s_k)] = s_k
            real[base:base + len(s_k)] = True
            dstloc[base:base + len(s_k)] = dl_k.astype(np.float32)
        u = uniq[k]
        xuT = np.zeros((DIN, UP), E4M3)
        xuT[:, :len(u)] = xT8[:, u]
        srcloc = np.zeros((E1P, 1), np.int32)
        srcloc[real, 0] = np.searchsorted(u, srcslot[real]).astype(np.int32)
        xdstT = np.zeros((DIN, NDP), E4M3)
        xdstT[:, :ND] = xT8[:, k * ND:(k + 1) * ND]
        dst_col = np.ascontiguousarray(dstloc.reshape(E1T, 128).T)   # [128, E1T]
        dst_row = np.tile(dstloc[None, :], (128, 1))                 # [128, E1P]
        in_maps.append({
            "xuT": xuT,
            "srcloc": srcloc,
            "xdstT": xdstT,
            "wl": wl8,
            "wr": wr8,
            "wfs": Wfs.astype(BF16),
            "bf2": bf2.astype(BF16),
            "dstcp": dst_col,
            "dstrow": dst_row,
            "iotaF": iotaF,
            "iotaP": iotaP,
            "ones1": ones1,
        })
    dims = {"T_BLK": T_BLK, "E1T": E1T, "E1P": E1P, "UT": UT,
            "npos": [int(v) for v in npos], "S": S}
    return in_maps, dims


def _build(dims, debug=False):
    """Trace the Bass/Tile program (identical for all cores)."""
    import concourse.bass as bass
    import concourse.mybir as mybir
    import concourse.tile as tile
    from concourse import bacc

    T_BLK, E1T, E1P = dims["T_BLK"], dims["E1T"], dims["E1P"]
    UT = dims["UT"]
    npos = dims["npos"]
    S = dims["S"]
    UP = UT * 128
    fp32 = mybir.dt.float32
    bf16 = mybir.dt.bfloat16
    fp8 = mybir.dt.float8e4
    AT = mybir.AluOpType
    AF = mybir.ActivationFunctionType
    DRM = mybir.MatmulPerfMode.DoubleRow

    nc = bacc.Bacc("TRN2", target_bir_lowering=False, debug=False)

    with tile.TileContext(nc) as tc:
        with tc.tile_pool(name="dram", bufs=1, space="DRAM") as dram:
            d_xuT = dram.tile([DIN, UP], fp8, kind="ExternalInput", name="xuT", uniquify=False)
            d_srcloc = dram.tile([E1P, 1], mybir.dt.int32, kind="ExternalInput", name="srcloc", uniquify=False)
            d_xdstT = dram.tile([DIN, NDP], fp8, kind="ExternalInput", name="xdstT", uniquify=False)
            d_wl = dram.tile([DIN, HCA], fp8, kind="ExternalInput", name="wl", uniquify=False)
            d_wr = dram.tile([DIN, HCA], fp8, kind="ExternalInput", name="wr", uniquify=False)
            d_wfs = dram.tile([HC, NCLS_P], bf16, kind="ExternalInput", name="wfs", uniquify=False)
            d_bf2 = dram.tile([1, NCLS_P], bf16, kind="ExternalInput", name="bf2", uniquify=False)
            d_dstcp = dram.tile([128, E1T], fp32, kind="ExternalInput", name="dstcp", uniquify=False)
            d_dstrow = dram.tile([128, E1P], fp32, kind="ExternalInput", name="dstrow", uniquify=False)
            d_iotaF = dram.tile([128, 128], fp32, kind="ExternalInput", name="iotaF", uniquify=False)
            d_iotaP = dram.tile([128, 128], fp32, kind="ExternalInput", name="iotaP", uniquify=False)
            d_ones1 = dram.tile([1, 128], bf16, kind="ExternalInput", name="ones1", uniquify=False)
            d_out = dram.tile([NDP, NCLS_P], fp32, kind="ExternalOutput", name="out", uniquify=False)
            d_xlu = dram.tile([UP, HCA], bf16, name="xlu_i",
                              kind="ExternalOutput" if debug else "Internal",
                              uniquify=False)
            if debug:
                d_dbga = dram.tile([128, E1T * H], fp32, kind="ExternalOutput", name="dbg_a", uniquify=False)
                d_dbgx = dram.tile([128, DB, HCA], fp8, kind="ExternalOutput", name="dbg_xrd", uniquify=False)

            with tc.tile_pool(name="gsb", bufs=1) as gsb:
                # resident tensors
                dstcp = gsb.tile([128, E1T], fp32, name="dstcp_r")
                nc.sync.dma_start(out=dstcp[:], in_=d_dstcp[:])
                iotaF = gsb.tile([128, 128], fp32, name="iotaF_r")
                nc.sync.dma_start(out=iotaF[:], in_=d_iotaF[:])
                iotaP = gsb.tile([128, 128], fp32, name="iotaP_r")
                nc.sync.dma_start(out=iotaP[:], in_=d_iotaP[:])
                ones1 = gsb.tile([1, 128], bf16, name="ones1_r")
                nc.sync.dma_start(out=ones1[:], in_=d_ones1[:])
                bf2 = gsb.tile([1, NCLS_P], bf16, name="bf2_r")
                nc.sync.dma_start(out=bf2[:], in_=d_bf2[:])
                a_all = gsb.tile([128, E1T * H], fp32, name="a_all_r")
                denr = gsb.tile([128, DB * H], fp32, name="denr_r")
                xrdb8 = gsb.tile([128, DB, HCA], fp8, name="xrdb8_r")
                idm = gsb.tile([128, 128], bf16, name="idm_r")
                nc.vector.scalar_tensor_tensor(
                    out=idm[:], in0=iotaP[:], scalar=0.0, in1=iotaF[:],
                    op0=AT.add, op1=AT.is_equal)

                # ---------------- phase 0: XR'' projection (fp8 DR) ----------------
                with tc.tile_pool(name="p01", bufs=1, space="PSUM") as ps1:
                    with tc.tile_pool(name="wpool", bufs=1) as wpool:
                        wmat = wpool.tile([128, KC, HCA], fp8, tag="wmat", bufs=1, name="wmat_r")
                        nc.sync.dma_start(out=wmat[:], in_=d_wr[:].rearrange("(kc p) n -> p kc n", p=128))
                        xdst8 = wpool.tile([128, KC, NDP], fp8, tag="xdst8", bufs=1, name="xdst8_r")
                        nc.sync.dma_start(out=xdst8[:], in_=d_xdstT[:].rearrange("(kc p) n -> p kc n", p=128))
                        # phase 0 + 0.5 interleaved: XR'' block and XLu'' tiles
                        # alternate so PE fills each other's copy/DMA stalls
                        wmat2 = wpool.tile([128, KC, HCA], fp8, tag="wmat2", bufs=1, name="wmat_r2")
                        nc.sync.dma_start(out=wmat2[:], in_=d_wl[:].rearrange("(kc p) n -> p kc n", p=128))
                        xu8 = wpool.tile([128, KC, UP], fp8, tag="xu8", bufs=1, name="xu8_r")
                        nc.sync.dma_start(out=xu8[:], in_=d_xuT[:].rearrange("(kc p) n -> p kc n", p=128))

                        def emit_xr(dc):
                            for nb in range(NB):
                                pp = ps1.tile([128, C], fp32, tag="pp", bufs=2, name=f"pp{nb}_c{dc}")
                                for kp in range(KC // 2):
                                    for hf in range(HF):
                                        nc.tensor.matmul(
                                            pp[:, hf * 512:(hf + 1) * 512],
                                            xdst8[:, 2 * kp:2 * kp + 2, dc * 128:(dc + 1) * 128],
                                            wmat[:, 2 * kp:2 * kp + 2, nb * C + hf * 512:nb * C + (hf + 1) * 512],
                                            start=(kp == 0), stop=(kp == KC // 2 - 1),
                                            perf_mode=DRM)
                                nc.vector.tensor_copy(out=xrdb8[:, dc, nb * C:(nb + 1) * C],
                                                      in_=pp[:])
                            ppa = ps1.tile([128, NAUG], fp32, tag="ppaug", bufs=1, name=f"ppa_c{dc}")
                            for kp in range(KC // 2):
                                nc.tensor.matmul(
                                    ppa[:], xdst8[:, 2 * kp:2 * kp + 2, dc * 128:(dc + 1) * 128],
                                    wmat[:, 2 * kp:2 * kp + 2, HC:HCA],
                                    start=(kp == 0), stop=(kp == KC // 2 - 1),
                                    perf_mode=DRM)
                            nc.vector.tensor_copy(out=xrdb8[:, dc, HC:HCA], in_=ppa[:])

                        with tc.tile_pool(name="sb0", bufs=2) as sb0:
                            def emit_xlu(ut):
                                xl_sb = sb0.tile([128, HCA], bf16, tag="xl_sb", bufs=3)
                                for nb in range(NB):
                                    pp = ps1.tile([128, C], fp32, tag="pp", bufs=2, name=f"ppu{nb}_{ut}")
                                    for kp in range(KC // 2):
                                        for hf in range(HF):
                                            nc.tensor.matmul(
                                                pp[:, hf * 512:(hf + 1) * 512],
                                                xu8[:, 2 * kp:2 * kp + 2, ut * 128:(ut + 1) * 128],
                                                wmat2[:, 2 * kp:2 * kp + 2, nb * C + hf * 512:nb * C + (hf + 1) * 512],
                                                start=(kp == 0), stop=(kp == KC // 2 - 1),
                                                perf_mode=DRM)
                                    if nb == 0:
                                        nc.vector.tensor_copy(out=xl_sb[:, nb * C:(nb + 1) * C], in_=pp[:])
                                    else:
                                        nc.scalar.activation(out=xl_sb[:, nb * C:(nb + 1) * C],
                                                             in_=pp[:], func=AF.Copy)
                                ppa = ps1.tile([128, NAUG], fp32, tag="ppaug", bufs=1, name=f"ppa_u{ut}")
                                for kp in range(KC // 2):
                                    nc.tensor.matmul(
                                        ppa[:], xu8[:, 2 * kp:2 * kp + 2, ut * 128:(ut + 1) * 128],
                                        wmat2[:, 2 * kp:2 * kp + 2, HC:HCA],
                                        start=(kp == 0), stop=(kp == KC // 2 - 1),
                                        perf_mode=DRM)
                                nc.scalar.activation(out=xl_sb[:, HC:HCA], in_=ppa[:], func=AF.Copy)
                                nc.sync.dma_start(out=d_xlu[ut * 128:(ut + 1) * 128, :], in_=xl_sb[:])

                            for dc in range(DB):
                                emit_xr(dc)
                            for ut_i in range(UT):
                                emit_xlu(ut_i)

                    # wfs resident (phase-0 weight slots are free now)
                    wfs_r = gsb.tile([128, HC // 128, NCLS_P], bf16, name="wfs_r")
                    nc.sync.dma_start(out=wfs_r[:], in_=d_wfs[:].rearrange("(cc p) n -> p cc n", p=128))

                    # ------- fused phases 1+2: per dst block -------
                    with (
                        tc.tile_pool(name="sb1", bufs=3) as sb1,
                        tc.tile_pool(name="sbe", bufs=T_BLK + 4) as sbe,
                        tc.tile_pool(name="p2", bufs=1, space="PSUM") as ps2,
                        tc.tile_pool(name="sb2", bufs=3) as sb2,
                    ):
                        for db in range(DB):
                            pb = db // 2
                            jact = db % 2
                            xles, esels, sals_all, drows = [], [], [], []
                            esc_all = sb1.tile([128, T_BLK * H], fp32, tag="esc_all",
                                               bufs=2, name=f"escall_{db}")
                            for t2 in range(T_BLK):
                                t = db * T_BLK + t2
                                sidx = sb1.tile([128, 1], mybir.dt.int32, tag="sidx")
                                nc.sync.dma_start(out=sidx[:], in_=d_srcloc[t * 128:(t + 1) * 128, :])
                                drow = sbe.tile([128, 128], fp32, tag="drow", name=f"drow_{db}_{t2}")
                                nc.sync.dma_start(out=drow[:], in_=d_dstrow[:, t * 128:(t + 1) * 128])
                                drows.append(drow)
                                # gather XL''[src] rows (bf16)
                                xle = sbe.tile([128, HCA], bf16, tag="xle", name=f"xle_{db}_{t2}")
                                nc.gpsimd.indirect_dma_start(
                                    out=xle[:], out_offset=None, in_=d_xlu[:],
                                    in_offset=bass.IndirectOffsetOnAxis(ap=sidx[:, :1], axis=0))
                                xles.append(xle)
                                # pair-packed 0/1 dst-selection for DoubleRow U-expansion
                                eselw = sb1.tile([128, 2, 128], fp8, tag="eselw")
                                nc.gpsimd.memset(eselw[:, 1 - jact, :], 0.0)
                                nc.vector.scalar_tensor_tensor(
                                    out=eselw[:, jact, :], in0=drow[:],
                                    scalar=float(-db * 128), in1=iotaP[:],
                                    op0=AT.add, op1=AT.is_equal)
                                eacc = sb1.tile([128, 2 * H], fp32, tag="eacc")
                                scrD = sb1.tile([128, C], fp32, tag="scrD", bufs=2)
                                scrA = sb1.tile([128, C], fp32, tag="scrA", bufs=2)
                                use_act = (t2 % 2 == 1)
                                for h in range(H):
                                    # U/S for head h directly in PSUM:
                                    #   DR dst-selection matmul of XR'' + identity matmul of xle
                                    pp = ps1.tile([128, C], fp32, tag="pp", bufs=2, name=f"ppe{h}_t{t}")
                                    for hf in range(HF):
                                        cl = h * C + hf * 512
                                        nc.tensor.matmul(
                                            pp[:, hf * 512:(hf + 1) * 512], eselw[:],
                                            xrdb8[:, 2 * pb:2 * pb + 2, cl:cl + 512],
                                            start=True, stop=False, perf_mode=DRM)
                                        nc.tensor.matmul(
                                            pp[:, hf * 512:(hf + 1) * 512], idm[:],
                                            xle[:, cl:cl + 512],
                                            start=False, stop=True)
                                    # scores: pos/neg relu accumulation off PSUM
                                    # (leaky = 0.8*relu(u) + 0.2*u; sum(u) rides the aug cols)
                                    np_h = npos[h]
                                    segs = [(0, np_h, 2 * h), (np_h, C - np_h, 2 * h + 1)]
                                    for off, ln, j in segs:
                                        if ln == 0:
                                            nc.vector.memset(eacc[:, j:j + 1], 0.0)
                                            continue
                                        if use_act:
                                            nc.scalar.activation(
                                                out=scrA[:, :ln], in_=pp[:, off:off + ln],
                                                func=AF.Relu, accum_out=eacc[:, j:j + 1])
                                        else:
                                            nc.vector.tensor_scalar(
                                                out=scrD[:, :ln], in0=pp[:, off:off + ln],
                                                scalar1=0.0, scalar2=0.0, op0=AT.max,
                                                op1=AT.add, accum_out=eacc[:, j:j + 1])
                                # aug sums: U aug cols in a small PSUM tile
                                ppa = ps1.tile([128, NAUG], fp32, tag="ppaug", bufs=1, name=f"ppa_t{t}")
                                nc.tensor.matmul(
                                    ppa[:], eselw[:], xrdb8[:, 2 * pb:2 * pb + 2, HC:HCA],
                                    start=True, stop=False, perf_mode=DRM)
                                nc.tensor.matmul(
                                    ppa[:], idm[:], xle[:, HC:HCA],
                                    start=False, stop=True)
                                aug8 = sb1.tile([128, NAUG], fp32, tag="aug8")
                                nc.scalar.activation(out=aug8[:], in_=ppa[:], func=AF.Copy)
                                rdif = sb1.tile([128, H], fp32, tag="rdif")
                                nc.vector.tensor_tensor(
                                    out=rdif[:], in0=eacc[:, 0:2 * H:2],
                                    in1=eacc[:, 1:2 * H:2], op=AT.subtract)
                                adif = sb1.tile([128, H], fp32, tag="adif")
                                nc.vector.tensor_tensor(
                                    out=adif[:], in0=aug8[:, 0:2 * H:2],
                                    in1=aug8[:, 1:2 * H:2], op=AT.subtract)
                                nc.vector.scalar_tensor_tensor(
                                    out=esc_all[:, t2 * H:(t2 + 1) * H], in0=adif[:],
                                    scalar=float(AUGDIV * NEG_SLOPE / (1.0 - NEG_SLOPE)),
                                    in1=rdif[:], op0=AT.mult, op1=AT.add)
                            # one Exp per dst block (avoids per-tile act-table swaps)
                            nc.scalar.activation(
                                out=a_all[:, db * T_BLK * H:(db + 1) * T_BLK * H],
                                in_=esc_all[:], func=AF.Exp,
                                scale=float(S * (1.0 - NEG_SLOPE)))
                            # pass A: denominators
                            pden = ps2.tile([128, H], fp32, tag="psmall", bufs=2, name=f"pden_{db}")
                            for t2 in range(T_BLK):
                                t = db * T_BLK + t2
                                ee = sbe.tile([128, 128], fp32, tag="esel_et", name=f"eet_{db}_{t2}")
                                nc.vector.scalar_tensor_tensor(
                                    out=ee[:], in0=dstcp[:, t:t + 1].to_broadcast([128, 128]),
                                    scalar=float(-db * 128), in1=iotaF[:],
                                    op0=AT.add, op1=AT.is_equal)
                                esels.append(ee)
                                nc.tensor.matmul(
                                    pden[:], ee[:], a_all[:, t * H:(t + 1) * H],
                                    start=(t2 == 0), stop=(t2 == T_BLK - 1))
                            dtmp = sb2.tile([128, H], fp32, tag="dtmp")
                            nc.vector.tensor_scalar_add(out=dtmp[:], in0=pden[:], scalar1=1e-16)
                            nc.vector.reciprocal(out=denr[:, db * H:(db + 1) * H], in_=dtmp[:])
                            # pass B: alpha and selection weights
                            for t2 in range(T_BLK):
                                t = db * T_BLK + t2
                                esde = sb2.tile([128, 128], fp32, tag="esde", bufs=4)
                                nc.vector.scalar_tensor_tensor(
                                    out=esde[:], in0=drows[t2][:],
                                    scalar=float(-db * 128), in1=iotaP[:],
                                    op0=AT.add, op1=AT.is_equal)
                                pad = ps2.tile([128, H], fp32, tag="psmall", bufs=2, name=f"pad_{db}_{t2}")
                                nc.tensor.matmul(pad[:], esde[:], denr[:, db * H:(db + 1) * H],
                                                 start=True, stop=True)
                                alpha = sb2.tile([128, H], fp32, tag="alpha")
                                nc.vector.tensor_tensor(out=alpha[:], in0=a_all[:, t * H:(t + 1) * H],
                                                        in1=pad[:], op=AT.mult)
                                hsal = []
                                for h in range(H):
                                    sal = sb2.tile([128, 128], bf16, tag=f"sal{h}",
                                                   bufs=T_BLK + 4, name=f"sal{h}_{db}_{t2}")
                                    nc.vector.tensor_tensor(
                                        out=sal[:], in0=esels[t2][:],
                                        in1=alpha[:, h:h + 1].to_broadcast([128, 128]),
                                        op=AT.mult)
                                    hsal.append(sal)
                                sals_all.append(hsal)
                            # transposed aggregation (xle-based: out = sum alpha*xl) + fc
                            NGRP = 4            # cc chunks per PSUM group (1 bank)
                            oaggs = []
                            for g in range(HC // 128 // NGRP):
                                pagg = ps2.tile([128, NGRP * 128], fp32, tag="pagg",
                                                bufs=1, name=f"pagg_{db}_{g}")
                                oagg = sb2.tile([128, NGRP * 128], bf16, tag="oagg", bufs=6,
                                                name=f"oagg_{db}_{g}")
                                for j in range(NGRP):
                                    cc = g * NGRP + j
                                    h = cc // (HC // 128 // H)
                                    for t2 in range(T_BLK):
                                        nc.tensor.matmul(
                                            pagg[:, j * 128:(j + 1) * 128],
                                            xles[t2][:, cc * 128:(cc + 1) * 128],
                                            sals_all[t2][h][:],
                                            start=(t2 == 0), stop=(t2 == T_BLK - 1))
                                if g % 2 == 0:
                                    nc.scalar.activation(out=oagg[:], in_=pagg[:], func=AF.Copy)
                                else:
                                    nc.vector.tensor_copy(out=oagg[:], in_=pagg[:])
                                oaggs.append(oagg)
                            pfc = ps2.tile([128, NCLS_P], fp32, tag="psmall", bufs=2, name=f"pfc_{db}")
                            for cc in range(HC // 128):
                                nc.tensor.matmul(
                                    pfc[:], oaggs[cc // NGRP][:, (cc % NGRP) * 128:(cc % NGRP + 1) * 128],
                                    wfs_r[:, cc, :],
                                    start=(cc == 0), stop=False)
                            nc.tensor.matmul(pfc[:], ones1[:], bf2[:], start=False, stop=True)
                            # softmax
                            negmax = sb2.tile([128, 1], fp32, tag="negmax")
                            nc.vector.tensor_reduce(out=negmax[:], in_=pfc[:],
                                                    axis=mybir.AxisListType.X,
                                                    op=AT.max, negate=True)
                            pexp = sb2.tile([128, NCLS_P], fp32, tag="pexp", bufs=2)
                            nc.scalar.activation(out=pexp[:], in_=pfc[:], func=AF.Exp,
                                                 bias=negmax[:, 0:1], scale=1.0)
                            ssum = sb2.tile([128, 1], fp32, tag="ssum")
                            nc.vector.tensor_reduce(out=ssum[:], in_=pexp[:],
                                                    axis=mybir.AxisListType.X, op=AT.add)
                            rs = sb2.tile([128, 1], fp32, tag="rs")
                            nc.vector.reciprocal(out=rs[:], in_=ssum[:])
                            hout = sb2.tile([128, NCLS_P], fp32, tag="hout", bufs=2)
                            nc.vector.scalar_tensor_tensor(
                                out=hout[:], in0=pexp[:], scalar=rs[:, 0:1], in1=pexp[:],
                                op0=AT.mult, op1=AT.bypass)
                            nc.sync.dma_start(out=d_out[db * 128:(db + 1) * 128, :], in_=hout[:])
                        if debug:
                            nc.sync.dma_start(out=d_dbga[:], in_=a_all[:])
                            nc.sync.dma_start(out=d_dbgx[:], in_=xrdb8[:])

    nc.compile()
    return nc


def kernel(**inputs):
    out_full = np.zeros((N, NCLS), np.float32)
    in_maps, dims = _prep(
        inputs["x"], inputs["edge_index"], inputs["Wl"], inputs["bl"],
        inputs["Wr"], inputs["br"], inputs["att"], inputs["bias"],
        inputs["Wf"], inputs["bf"])
    nc = _build(dims)
    from concourse.bass_utils import run_bass_kernel_spmd
    res = run_bass_kernel_spmd(nc, in_maps, core_ids=list(range(NCORES)))
    for k in range(NCORES):
        out_full[k * ND:(k + 1) * ND, :] = res.results[k]["out"][:ND, :NCLS]
    return out_full


# revision 18
# speedup vs baseline: 5.8922x; 1.0377x over previous
"""GATv2 + softmax head for nn_GATModel_Softmax_4535485465120 on 8 trn2 NeuronCores.

v2: fp8-DoubleRow projections + xle-based aggregation (no XR-transpose phase).

Strategy (graph/data parallel by dst node, fully local — no collectives):
  - Nodes partitioned into 8 ranges of 1000 dst nodes (one per core).
  - Host preps per core: x.T columns (fp8) for the core's unique src nodes and
    its dst nodes; weights with att-magnitude (clamped at max/64 per head)
    folded into Wl/Wr columns, permuted pos-att-first, scaled 1/S for fp8;
    Wf rows carry the inverse permutation, S/(3*att_eff) un-scaling and the
    head-mean.
  - Device pipeline per core:
      phase 0:  XR'' = xdst8 @ Wr8 via fp8 DoubleRow matmuls -> SBUF fp8
                (resident, pair-packed by dst block for DR reuse)
      phase 0.5: XLu'' = xu8 @ Wl8 (DR) -> HBM bf16 [UP, 3072]
      phase 1 (per dst block, per 128-edge tile): U/S accumulated directly in
                PSUM = DR-selection-matmul of XR'' + identity-matmul of
                gathered XLu'' rows; leaky-relu + sign-segmented accumulation
                on DVE reads PSUM directly; a = exp(S * esc) on ScalarE.
      phase 2:  denom/alpha via selection matmuls; agg^T = sum_e alpha *
                xle (selection matmul, bf16); logits = agg^T-chunks @ Wf_stack
                (+folded bias row); row softmax -> output.

kernel(**inputs) takes FULL inputs, shards internally, returns FULL [8000,460] f32.
"""

import numpy as np
import ml_dtypes

BF16 = ml_dtypes.bfloat16
E4M3 = ml_dtypes.float8_e4m3   # IEEE e4m3: max 240, has inf (matches mybir float8e4)
F8MAX = 240.0

# Problem constants (hardcoded per spec)
N = 8000
DIN = 1024
H = 3
C = 1024
HC = H * C          # 3072
NCLS = 460
NCLS_P = 512
NEG_SLOPE = 0.2
NCORES = 8
ND = N // NCORES    # 1000 dst nodes per core
NDP = 1024          # padded dst count per core
DB = NDP // 128     # 8 dst blocks per core
P = 128
KC = DIN // P       # 8 contraction chunks (4 DoubleRow pairs)
NB = 3              # n-chunks of 1024 in HC (one per head)
HF = 2              # 512-wide matmul halves per 1024 chunk
NAUG = 8            # aug columns: per-head pos/neg sums (6) + pad (2)
HCA = HC + NAUG     # 3080
AUGDIV = 64.0       # aug columns scaled by 1/64 to stay in fp8 range


def _prep(x, edge_index, Wl, bl, Wr, br, att, bias, Wf, bf):
    """Host-side preprocessing -> per-core input maps + static dims."""
    x = np.asarray(x, np.float32)
    ei = np.asarray(edge_index).astype(np.int64)
    Wl = np.asarray(Wl, np.float32)
    Wr = np.asarray(Wr, np.float32)
    bl = np.asarray(bl, np.float32)
    br = np.asarray(br, np.float32)
    att = np.asarray(att, np.float32)
    bias = np.asarray(bias, np.float32)
    Wf = np.asarray(Wf, np.float32)
    bf = np.asarray(bf, np.float32)

    assert np.all(bl == 0) and np.all(br == 0), \
        "nonzero bl/br not supported by this kernel build"

    loops = np.arange(N, dtype=np.int64)
    src_all = np.concatenate([ei[:, 0], loops])
    dst_all = np.concatenate([ei[:, 1], loops])

    # att folding: per head, column scale att_eff (clamped so fp8 columns
    # stay out of the subnormal range) and permutation pos-first
    absatt = np.abs(att)                       # [H, C]
    att_eff = np.maximum(absatt, absatt.max(1, keepdims=True) / 64.0)
    perm = np.zeros((H, C), np.int64)          # perm[h, newc] = origc
    npos = np.zeros(H, np.int64)
    for h in range(H):
        pos = np.nonzero(att[h] > 0)[0]
        neg = np.nonzero(att[h] <= 0)[0]
        perm[h] = np.concatenate([pos, neg])
        npos[h] = len(pos)

    # scaled/permuted projection weights  [DIN, HC]
    Wl_s = np.zeros((DIN, HC), np.float32)
    Wr_s = np.zeros((DIN, HC), np.float32)
    for h in range(H):
        sc = att_eff[h, perm[h]]               # [C]
        Wl_s[:, h * C:(h + 1) * C] = Wl[:, h * C + perm[h]] * sc
        Wr_s[:, h * C:(h + 1) * C] = Wr[:, h * C + perm[h]] * sc

    # global fp8 scale S: covers weight absmax and a 6-sigma bound on the
    # projected activations (xr'' must fit fp8 storage after the matmul)
    colnorm = max(np.linalg.norm(Wl_s, axis=0).max(),
                  np.linalg.norm(Wr_s, axis=0).max())
    S = float(max(np.abs(Wl_s).max() / F8MAX, np.abs(Wr_s).max() / F8MAX,
                  colnorm * 7.0 / F8MAX))

    def with_aug(W):
        W8 = np.clip(W / S, -F8MAX, F8MAX).astype(E4M3)
        Wa = np.zeros((DIN, HCA), E4M3)
        Wa[:, :HC] = W8
        W8f = W8.astype(np.float32)
        for h in range(H):
            np_h = int(npos[h])
            Wa[:, HC + 2 * h] = (W8f[:, h * C:h * C + np_h].sum(1) / AUGDIV).astype(E4M3)
            Wa[:, HC + 2 * h + 1] = (W8f[:, h * C + np_h:(h + 1) * C].sum(1) / AUGDIV).astype(E4M3)
        return Wa
    wl8 = with_aug(Wl_s)
    wr8 = with_aug(Wr_s)

    # final fc stack: logits = sum_h (agg''_h * S/(3*att_eff)) @ Wf  (+ bias@Wf + bf)
    Wfs = np.zeros((HC, NCLS_P), np.float32)
    for h in range(H):
        sc = S / (3.0 * att_eff[h, perm[h]])
        Wfs[h * C:(h + 1) * C, :NCLS] = Wf[perm[h]] * sc[:, None]
    bf2 = np.full((1, NCLS_P), -1e30, np.float32)
    bf2[0, :NCLS] = bias @ Wf + bf

    xT8 = np.clip(np.ascontiguousarray(x.T), -F8MAX, F8MAX).astype(E4M3)     # [DIN, N]

    # per-core edge grouping: edges (incl. self loops) by dst block
    cores = []
    tmax = 1
    for k in range(NCORES):
        lo, hi = k * ND, (k + 1) * ND
        m = (dst_all >= lo) & (dst_all < hi)
        s_k = src_all[m]
        dl_k = (dst_all[m] - lo).astype(np.int64)
        order = np.argsort(dl_k, kind="stable")
        s_k, dl_k = s_k[order], dl_k[order]
        blocks = []
        for db in range(DB):
            bm = (dl_k >= db * 128) & (dl_k < (db + 1) * 128)
            blocks.append((s_k[bm], dl_k[bm]))
            tmax = max(tmax, (len(blocks[-1][0]) + 127) // 128)
        cores.append(blocks)

    T_BLK = tmax
    E1T = DB * T_BLK
    E1P = E1T * 128

    iotaF = np.tile(np.arange(128, dtype=np.float32)[None, :], (128, 1))
    iotaP = np.tile(np.arange(128, dtype=np.float32)[:, None], (1, 128))
    ones1 = np.ones((1, 128), BF16)
    # unique srcs per core -> common padded tile count
    uniq = []
    for k in range(NCORES):
        srcs = np.concatenate([cores[k][db][0] for db in range(DB)])
        uniq.append(np.unique(srcs))
    UT = max((len(u) + 127) // 128 for u in uniq)
    UP = UT * 128
    in_maps = []
    for k in range(NCORES):
        srcslot = np.zeros(E1P, np.int64)
        real = np.zeros(E1P, bool)
        dstloc = np.full(E1P, -1.0, np.float32)
        for db in range(DB):
            s_k, dl_k = cores[k][db]
            base = db * T_BLK * 128
            srcslot[base:base + len(s_k)] = s_k
            real[base:base + len(s_k)] = True
            dstloc[base:base + len(s_k)] = dl_k.astype(np.float32)
        u = uniq[k]
        xuT = np.zeros((DIN, UP), E4M3)
        xuT[:, :len(u)] = xT8[:, u]
        srcloc = np.zeros((E1P, 1), np.int32)
        srcloc[real, 0] = np.searchsorted(u, srcslot[real]).astype(np.int32)
        xdstT = np.zeros((DIN, NDP), E4M3)
        xdstT[:, :ND] = xT8[:, k * ND:(k + 1) * ND]
        dst_col = np.ascontiguousarray(dstloc.reshape(E1T, 128).T)   # [128, E1T]
        src_col = np.ascontiguousarray(srcloc.reshape(E1T, 128).T)   # [128, E1T] int32
        dst_row = np.tile(dstloc[None, :], (128, 1))                 # [128, E1P]
        in_maps.append({
            "xuT": xuT,
            "srcloc": srcloc,
            "srccp": src_col,
            "xdstT": xdstT,
            "wl": wl8,
            "wr": wr8,
            "wfs": Wfs.astype(BF16),
            "bf2": bf2.astype(BF16),
            "dstcp": dst_col,
            "dstrow": dst_row,
            "iotaF": iotaF,
            "iotaP": iotaP,
            "ones1": ones1,
        })
    dims = {"T_BLK": T_BLK, "E1T": E1T, "E1P": E1P, "UT": UT,
            "npos": [int(v) for v in npos], "S": S}
    return in_maps, dims


def _build(dims, debug=False):
    """Trace the Bass/Tile program (identical for all cores)."""
    import concourse.bass as bass
    import concourse.mybir as mybir
    import concourse.tile as tile
    from concourse import bacc

    T_BLK, E1T, E1P = dims["T_BLK"], dims["E1T"], dims["E1P"]
    UT = dims["UT"]
    npos = dims["npos"]
    S = dims["S"]
    UP = UT * 128
    fp32 = mybir.dt.float32
    bf16 = mybir.dt.bfloat16
    fp8 = mybir.dt.float8e4
    AT = mybir.AluOpType
    AF = mybir.ActivationFunctionType
    DRM = mybir.MatmulPerfMode.DoubleRow

    nc = bacc.Bacc("TRN2", target_bir_lowering=False, debug=False)

    with tile.TileContext(nc) as tc:
        with tc.tile_pool(name="dram", bufs=1, space="DRAM") as dram:
            d_xuT = dram.tile([DIN, UP], fp8, kind="ExternalInput", name="xuT", uniquify=False)
            d_srcloc = dram.tile([E1P, 1], mybir.dt.int32, kind="ExternalInput", name="srcloc", uniquify=False)
            d_srccp = dram.tile([128, E1T], mybir.dt.int32, kind="ExternalInput", name="srccp", uniquify=False)
            d_xdstT = dram.tile([DIN, NDP], fp8, kind="ExternalInput", name="xdstT", uniquify=False)
            d_wl = dram.tile([DIN, HCA], fp8, kind="ExternalInput", name="wl", uniquify=False)
            d_wr = dram.tile([DIN, HCA], fp8, kind="ExternalInput", name="wr", uniquify=False)
            d_wfs = dram.tile([HC, NCLS_P], bf16, kind="ExternalInput", name="wfs", uniquify=False)
            d_bf2 = dram.tile([1, NCLS_P], bf16, kind="ExternalInput", name="bf2", uniquify=False)
            d_dstcp = dram.tile([128, E1T], fp32, kind="ExternalInput", name="dstcp", uniquify=False)
            d_dstrow = dram.tile([128, E1P], fp32, kind="ExternalInput", name="dstrow", uniquify=False)
            d_iotaF = dram.tile([128, 128], fp32, kind="ExternalInput", name="iotaF", uniquify=False)
            d_iotaP = dram.tile([128, 128], fp32, kind="ExternalInput", name="iotaP", uniquify=False)
            d_ones1 = dram.tile([1, 128], bf16, kind="ExternalInput", name="ones1", uniquify=False)
            d_out = dram.tile([NDP, NCLS_P], fp32, kind="ExternalOutput", name="out", uniquify=False)
            d_xlu = dram.tile([UP, HCA], bf16, name="xlu_i",
                              kind="ExternalOutput" if debug else "Internal",
                              uniquify=False)
            if debug:
                d_dbga = dram.tile([128, E1T * H], fp32, kind="ExternalOutput", name="dbg_a", uniquify=False)
                d_dbgx = dram.tile([128, DB, HCA], fp8, kind="ExternalOutput", name="dbg_xrd", uniquify=False)

            with tc.tile_pool(name="gsb", bufs=1) as gsb:
                # resident tensors
                dstcp = gsb.tile([128, E1T], fp32, name="dstcp_r")
                nc.sync.dma_start(out=dstcp[:], in_=d_dstcp[:])
                srccp = gsb.tile([128, E1T], mybir.dt.int32, name="srccp_r")
                nc.sync.dma_start(out=srccp[:], in_=d_srccp[:])
                iotaF = gsb.tile([128, 128], fp32, name="iotaF_r")
                nc.sync.dma_start(out=iotaF[:], in_=d_iotaF[:])
                iotaP = gsb.tile([128, 128], fp32, name="iotaP_r")
                nc.sync.dma_start(out=iotaP[:], in_=d_iotaP[:])
                ones1 = gsb.tile([1, 128], bf16, name="ones1_r")
                nc.sync.dma_start(out=ones1[:], in_=d_ones1[:])
                bf2 = gsb.tile([1, NCLS_P], bf16, name="bf2_r")
                nc.sync.dma_start(out=bf2[:], in_=d_bf2[:])
                a_all = gsb.tile([128, E1T * H], fp32, name="a_all_r")
                denr = gsb.tile([128, DB * H], fp32, name="denr_r")
                xrdb8 = gsb.tile([128, DB, HCA], fp8, name="xrdb8_r")
                idm = gsb.tile([128, 128], bf16, name="idm_r")
                nc.vector.scalar_tensor_tensor(
                    out=idm[:], in0=iotaP[:], scalar=0.0, in1=iotaF[:],
                    op0=AT.add, op1=AT.is_equal)

                # ---------------- phase 0: XR'' projection (fp8 DR) ----------------
                with tc.tile_pool(name="p01", bufs=1, space="PSUM") as ps1:
                    with tc.tile_pool(name="wpool", bufs=1) as wpool:
                        wmat = wpool.tile([128, KC, HCA], fp8, tag="wmat", bufs=1, name="wmat_r")
                        xdst8 = wpool.tile([128, KC, NDP], fp8, tag="xdst8", bufs=1, name="xdst8_r")
                        for kp in range(KC // 2):
                            nc.sync.dma_start(
                                out=xdst8[:, 2 * kp:2 * kp + 2, :],
                                in_=d_xdstT[kp * 256:(kp + 1) * 256, :]
                                    .rearrange("(kc p) n -> p kc n", p=128))
                            nc.sync.dma_start(
                                out=wmat[:, 2 * kp:2 * kp + 2, :],
                                in_=d_wr[kp * 256:(kp + 1) * 256, :]
                                    .rearrange("(kc p) n -> p kc n", p=128))
                        # phase 0 + 0.5 interleaved: XR'' block and XLu'' tiles
                        # alternate so PE fills each other's copy/DMA stalls
                        wmat2 = wpool.tile([128, KC, HCA], fp8, tag="wmat2", bufs=1, name="wmat_r2")
                        xu8 = wpool.tile([128, KC, UP], fp8, tag="xu8", bufs=1, name="xu8_r")
                        for kp in range(KC // 2):
                            nc.sync.dma_start(
                                out=wmat2[:, 2 * kp:2 * kp + 2, :],
                                in_=d_wl[kp * 256:(kp + 1) * 256, :]
                                    .rearrange("(kc p) n -> p kc n", p=128))
                            nc.sync.dma_start(
                                out=xu8[:, 2 * kp:2 * kp + 2, :],
                                in_=d_xuT[kp * 256:(kp + 1) * 256, :]
                                    .rearrange("(kc p) n -> p kc n", p=128))

                        def emit_xr(dc):
                            for nb in range(NB):
                                pp = ps1.tile([128, C], fp32, tag="pp", bufs=2, name=f"pp{nb}_c{dc}")
                                for kp in range(KC // 2):
                                    for hf in range(HF):
                                        nc.tensor.matmul(
                                            pp[:, hf * 512:(hf + 1) * 512],
                                            xdst8[:, 2 * kp:2 * kp + 2, dc * 128:(dc + 1) * 128],
                                            wmat[:, 2 * kp:2 * kp + 2, nb * C + hf * 512:nb * C + (hf + 1) * 512],
                                            start=(kp == 0), stop=(kp == KC // 2 - 1),
                                            perf_mode=DRM)
                                nc.vector.tensor_copy(out=xrdb8[:, dc, nb * C:(nb + 1) * C],
                                                      in_=pp[:])
                            ppa = ps1.tile([128, NAUG], fp32, tag="ppaug", bufs=1, name=f"ppa_c{dc}")
                            for kp in range(KC // 2):
                                nc.tensor.matmul(
                                    ppa[:], xdst8[:, 2 * kp:2 * kp + 2, dc * 128:(dc + 1) * 128],
                                    wmat[:, 2 * kp:2 * kp + 2, HC:HCA],
                                    start=(kp == 0), stop=(kp == KC // 2 - 1),
                                    perf_mode=DRM)
                            nc.vector.tensor_copy(out=xrdb8[:, dc, HC:HCA], in_=ppa[:])

                        with tc.tile_pool(name="sb0", bufs=2) as sb0:
                            def emit_xlu(ut):
                                xl_sb = sb0.tile([128, HCA], bf16, tag="xl_sb", bufs=3)
                                for nb in range(NB):
                                    pp = ps1.tile([128, C], fp32, tag="pp", bufs=2, name=f"ppu{nb}_{ut}")
                                    for kp in range(KC // 2):
                                        for hf in range(HF):
                                            nc.tensor.matmul(
                                                pp[:, hf * 512:(hf + 1) * 512],
                                                xu8[:, 2 * kp:2 * kp + 2, ut * 128:(ut + 1) * 128],
                                                wmat2[:, 2 * kp:2 * kp + 2, nb * C + hf * 512:nb * C + (hf + 1) * 512],
                                                start=(kp == 0), stop=(kp == KC // 2 - 1),
                                                perf_mode=DRM)
                                    if nb == 0:
                                        nc.vector.tensor_copy(out=xl_sb[:, nb * C:(nb + 1) * C], in_=pp[:])
                                    else:
                                        nc.scalar.activation(out=xl_sb[:, nb * C:(nb + 1) * C],
                                                             in_=pp[:], func=AF.Copy)
                                ppa = ps1.tile([128, NAUG], fp32, tag="ppaug", bufs=1, name=f"ppa_u{ut}")
                                for kp in range(KC // 2):
                                    nc.tensor.matmul(
                                        ppa[:], xu8[:, 2 * kp:2 * kp + 2, ut * 128:(ut + 1) * 128],
                                        wmat2[:, 2 * kp:2 * kp + 2, HC:HCA],
                                        start=(kp == 0), stop=(kp == KC // 2 - 1),
                                        perf_mode=DRM)
                                nc.scalar.activation(out=xl_sb[:, HC:HCA], in_=ppa[:], func=AF.Copy)
                                nc.sync.dma_start(out=d_xlu[ut * 128:(ut + 1) * 128, :], in_=xl_sb[:])

                            for dc in range(DB):
                                emit_xr(dc)
                            for ut_i in range(UT):
                                emit_xlu(ut_i)

                    # wfs resident (phase-0 weight slots are free now)
                    wfs_r = gsb.tile([128, HC // 128, NCLS_P], bf16, name="wfs_r")
                    nc.sync.dma_start(out=wfs_r[:], in_=d_wfs[:].rearrange("(cc p) n -> p cc n", p=128))

                    # ------- fused phases 1+2: per dst block -------
                    with (
                        tc.tile_pool(name="sb1", bufs=3) as sb1,
                        tc.tile_pool(name="sbe", bufs=T_BLK + 4) as sbe,
                        tc.tile_pool(name="p2", bufs=1, space="PSUM") as ps2,
                        tc.tile_pool(name="sb2", bufs=3) as sb2,
                    ):
                        for db in range(DB):
                            pb = db // 2
                            jact = db % 2
                            xles, esels, sals_all, drows = [], [], [], []
                            esc_all = sb1.tile([128, T_BLK * H], fp32, tag="esc_all",
                                               bufs=2, name=f"escall_{db}")
                            drow_db = sbe.tile([128, T_BLK * 128], fp32, tag="drow",
                                               bufs=2, name=f"drow_{db}")
                            nc.sync.dma_start(
                                out=drow_db[:],
                                in_=d_dstrow[:, db * T_BLK * 128:(db + 1) * T_BLK * 128])
                            for t2 in range(T_BLK):
                                t = db * T_BLK + t2
                                drow = drow_db[:, t2 * 128:(t2 + 1) * 128]
                                drows.append(drow)
                                # gather XL''[src] rows (bf16)
                                xle = sbe.tile([128, HCA], bf16, tag="xle", name=f"xle_{db}_{t2}")
                                nc.gpsimd.indirect_dma_start(
                                    out=xle[:], out_offset=None, in_=d_xlu[:],
                                    in_offset=bass.IndirectOffsetOnAxis(ap=srccp[:, t:t + 1], axis=0))
                                xles.append(xle)
                                # pair-packed 0/1 dst-selection for DoubleRow U-expansion
                                eselw = sb1.tile([128, 2, 128], fp8, tag="eselw")
                                nc.gpsimd.memset(eselw[:, 1 - jact, :], 0.0)
                                nc.vector.scalar_tensor_tensor(
                                    out=eselw[:, jact, :], in0=drow,
                                    scalar=float(-db * 128), in1=iotaP[:],
                                    op0=AT.add, op1=AT.is_equal)
                                eacc = sb1.tile([128, 2 * H], fp32, tag="eacc")
                                scrD = sb1.tile([128, C], fp32, tag="scrD", bufs=2)
                                scrA = sb1.tile([128, C], fp32, tag="scrA", bufs=2)
                                for h in range(H):
                                    use_act = (h == 1) or (h == 2 and t2 % 2 == 0)
                                    # U/S for head h directly in PSUM:
                                    #   DR dst-selection matmul of XR'' + identity matmul of xle
                                    pp = ps1.tile([128, C], fp32, tag="pp", bufs=2, name=f"ppe{h}_t{t}")
                                    for hf in range(HF):
                                        cl = h * C + hf * 512
                                        nc.tensor.matmul(
                                            pp[:, hf * 512:(hf + 1) * 512], eselw[:],
                                            xrdb8[:, 2 * pb:2 * pb + 2, cl:cl + 512],
                                            start=True, stop=False, perf_mode=DRM)
                                        nc.tensor.matmul(
                                            pp[:, hf * 512:(hf + 1) * 512], idm[:],
                                            xle[:, cl:cl + 512],
                                            start=False, stop=True)
                                    # scores: pos/neg relu accumulation off PSUM
                                    # (leaky = 0.8*relu(u) + 0.2*u; sum(u) rides the aug cols)
                                    np_h = npos[h]
                                    segs = [(0, np_h, 2 * h), (np_h, C - np_h, 2 * h + 1)]
                                    for off, ln, j in segs:
                                        if ln == 0:
                                            nc.vector.memset(eacc[:, j:j + 1], 0.0)
                                            continue
                                        if use_act:
                                            nc.scalar.activation(
                                                out=scrA[:, :ln], in_=pp[:, off:off + ln],
                                                func=AF.Relu, accum_out=eacc[:, j:j + 1])
                                        else:
                                            nc.vector.tensor_scalar(
                                                out=scrD[:, :ln], in0=pp[:, off:off + ln],
                                                scalar1=0.0, scalar2=0.0, op0=AT.max,
                                                op1=AT.add, accum_out=eacc[:, j:j + 1])
                                # aug sums: U aug cols in a small PSUM tile
                                ppa = ps1.tile([128, NAUG], fp32, tag="ppaug", bufs=1, name=f"ppa_t{t}")
                                nc.tensor.matmul(
                                    ppa[:], eselw[:], xrdb8[:, 2 * pb:2 * pb + 2, HC:HCA],
                                    start=True, stop=False, perf_mode=DRM)
                                nc.tensor.matmul(
                                    ppa[:], idm[:], xle[:, HC:HCA],
                                    start=False, stop=True)
                                aug8 = sb1.tile([128, NAUG], fp32, tag="aug8")
                                nc.scalar.activation(out=aug8[:], in_=ppa[:], func=AF.Copy)
                                rdif = sb1.tile([128, H], fp32, tag="rdif")
                                nc.vector.tensor_tensor(
                                    out=rdif[:], in0=eacc[:, 0:2 * H:2],
                                    in1=eacc[:, 1:2 * H:2], op=AT.subtract)
                                adif = sb1.tile([128, H], fp32, tag="adif")
                                nc.vector.tensor_tensor(
                                    out=adif[:], in0=aug8[:, 0:2 * H:2],
                                    in1=aug8[:, 1:2 * H:2], op=AT.subtract)
                                nc.vector.scalar_tensor_tensor(
                                    out=esc_all[:, t2 * H:(t2 + 1) * H], in0=adif[:],
                                    scalar=float(AUGDIV * NEG_SLOPE / (1.0 - NEG_SLOPE)),
                                    in1=rdif[:], op0=AT.mult, op1=AT.add)
                            # one Exp per dst block (avoids per-tile act-table swaps)
                            nc.scalar.activation(
                                out=a_all[:, db * T_BLK * H:(db + 1) * T_BLK * H],
                                in_=esc_all[:], func=AF.Exp,
                                scale=float(S * (1.0 - NEG_SLOPE)))
                            # pass A: denominators
                            pden = ps2.tile([128, H], fp32, tag="psmall", bufs=2, name=f"pden_{db}")
                            for t2 in range(T_BLK):
                                t = db * T_BLK + t2
                                ee = sbe.tile([128, 128], fp32, tag="esel_et", name=f"eet_{db}_{t2}")
                                nc.vector.scalar_tensor_tensor(
                                    out=ee[:], in0=dstcp[:, t:t + 1].to_broadcast([128, 128]),
                                    scalar=float(-db * 128), in1=iotaF[:],
                                    op0=AT.add, op1=AT.is_equal)
                                esels.append(ee)
                                nc.tensor.matmul(
                                    pden[:], ee[:], a_all[:, t * H:(t + 1) * H],
                                    start=(t2 == 0), stop=(t2 == T_BLK - 1))
                            dtmp = sb2.tile([128, H], fp32, tag="dtmp")
                            nc.vector.tensor_scalar_add(out=dtmp[:], in0=pden[:], scalar1=1e-16)
                            nc.vector.reciprocal(out=denr[:, db * H:(db + 1) * H], in_=dtmp[:])
                            # pass B: alpha and selection weights
                            for t2 in range(T_BLK):
                                t = db * T_BLK + t2
                                esde = sb2.tile([128, 128], fp32, tag="esde", bufs=4)
                                nc.vector.scalar_tensor_tensor(
                                    out=esde[:], in0=drows[t2],
                                    scalar=float(-db * 128), in1=iotaP[:],
                                    op0=AT.add, op1=AT.is_equal)
                                pad = ps2.tile([128, H], fp32, tag="psmall", bufs=2, name=f"pad_{db}_{t2}")
                                nc.tensor.matmul(pad[:], esde[:], denr[:, db * H:(db + 1) * H],
                                                 start=True, stop=True)
                                alpha = sb2.tile([128, H], fp32, tag="alpha")
                                nc.vector.tensor_tensor(out=alpha[:], in0=a_all[:, t * H:(t + 1) * H],
                                                        in1=pad[:], op=AT.mult)
                                hsal = []
                                for h in range(H):
                                    sal = sb2.tile([128, 128], bf16, tag=f"sal{h}",
                                                   bufs=T_BLK + 4, name=f"sal{h}_{db}_{t2}")
                                    nc.vector.tensor_tensor(
                                        out=sal[:], in0=esels[t2][:],
                                        in1=alpha[:, h:h + 1].to_broadcast([128, 128]),
                                        op=AT.mult)
                                    hsal.append(sal)
                                sals_all.append(hsal)
                            # transposed aggregation (xle-based: out = sum alpha*xl) + fc
                            NGRP = 4            # cc chunks per PSUM group (1 bank)
                            oaggs = []
                            for g in range(HC // 128 // NGRP):
                                pagg = ps2.tile([128, NGRP * 128], fp32, tag="pagg",
                                                bufs=1, name=f"pagg_{db}_{g}")
                                oagg = sb2.tile([128, NGRP * 128], bf16, tag="oagg", bufs=6,
                                                name=f"oagg_{db}_{g}")
                                for j in range(NGRP):
                                    cc = g * NGRP + j
                                    h = cc // (HC // 128 // H)
                                    for t2 in range(T_BLK):
                                        nc.tensor.matmul(
                                            pagg[:, j * 128:(j + 1) * 128],
                                            xles[t2][:, cc * 128:(cc + 1) * 128],
                                            sals_all[t2][h][:],
                                            start=(t2 == 0), stop=(t2 == T_BLK - 1))
                                if g % 2 == 0:
                                    nc.scalar.activation(out=oagg[:], in_=pagg[:], func=AF.Copy)
                                else:
                                    nc.vector.tensor_copy(out=oagg[:], in_=pagg[:])
                                oaggs.append(oagg)
                            pfc = ps2.tile([128, NCLS_P], fp32, tag="psmall", bufs=2, name=f"pfc_{db}")
                            for cc in range(HC // 128):
                                nc.tensor.matmul(
                                    pfc[:], oaggs[cc // NGRP][:, (cc % NGRP) * 128:(cc % NGRP + 1) * 128],
                                    wfs_r[:, cc, :],
                                    start=(cc == 0), stop=False)
                            nc.tensor.matmul(pfc[:], ones1[:], bf2[:], start=False, stop=True)
                            # softmax
                            negmax = sb2.tile([128, 1], fp32, tag="negmax")
                            nc.vector.tensor_reduce(out=negmax[:], in_=pfc[:],
                                                    axis=mybir.AxisListType.X,
                                                    op=AT.max, negate=True)
                            pexp = sb2.tile([128, NCLS_P], fp32, tag="pexp", bufs=2)
                            nc.scalar.activation(out=pexp[:], in_=pfc[:], func=AF.Exp,
                                                 bias=negmax[:, 0:1], scale=1.0)
                            ssum = sb2.tile([128, 1], fp32, tag="ssum")
                            nc.vector.tensor_reduce(out=ssum[:], in_=pexp[:],
                                                    axis=mybir.AxisListType.X, op=AT.add)
                            rs = sb2.tile([128, 1], fp32, tag="rs")
                            nc.vector.reciprocal(out=rs[:], in_=ssum[:])
                            hout = sb2.tile([128, NCLS_P], fp32, tag="hout", bufs=2)
                            nc.vector.scalar_tensor_tensor(
                                out=hout[:], in0=pexp[:], scalar=rs[:, 0:1], in1=pexp[:],
                                op0=AT.mult, op1=AT.bypass)
                            nc.sync.dma_start(out=d_out[db * 128:(db + 1) * 128, :], in_=hout[:])
                        if debug:
                            nc.sync.dma_start(out=d_dbga[:], in_=a_all[:])
                            nc.sync.dma_start(out=d_dbgx[:], in_=xrdb8[:])

    nc.compile()
    return nc


def kernel(**inputs):
    out_full = np.zeros((N, NCLS), np.float32)
    in_maps, dims = _prep(
        inputs["x"], inputs["edge_index"], inputs["Wl"], inputs["bl"],
        inputs["Wr"], inputs["br"], inputs["att"], inputs["bias"],
        inputs["Wf"], inputs["bf"])
    nc = _build(dims)
    from concourse.bass_utils import run_bass_kernel_spmd
    res = run_bass_kernel_spmd(nc, in_maps, core_ids=list(range(NCORES)))
    for k in range(NCORES):
        out_full[k * ND:(k + 1) * ND, :] = res.results[k]["out"][:ND, :NCLS]
    return out_full


# revision 19
# speedup vs baseline: 5.9270x; 1.0059x over previous
"""GATv2 + softmax head for nn_GATModel_Softmax_4535485465120 on 8 trn2 NeuronCores.

v2: fp8-DoubleRow projections + xle-based aggregation (no XR-transpose phase).

Strategy (graph/data parallel by dst node, fully local — no collectives):
  - Nodes partitioned into 8 ranges of 1000 dst nodes (one per core).
  - Host preps per core: x.T columns (fp8) for the core's unique src nodes and
    its dst nodes; weights with att-magnitude (clamped at max/64 per head)
    folded into Wl/Wr columns, permuted pos-att-first, scaled 1/S for fp8;
    Wf rows carry the inverse permutation, S/(3*att_eff) un-scaling and the
    head-mean.
  - Device pipeline per core:
      phase 0:  XR'' = xdst8 @ Wr8 via fp8 DoubleRow matmuls -> SBUF fp8
                (resident, pair-packed by dst block for DR reuse)
      phase 0.5: XLu'' = xu8 @ Wl8 (DR) -> HBM bf16 [UP, 3072]
      phase 1 (per dst block, per 128-edge tile): U/S accumulated directly in
                PSUM = DR-selection-matmul of XR'' + identity-matmul of
                gathered XLu'' rows; leaky-relu + sign-segmented accumulation
                on DVE reads PSUM directly; a = exp(S * esc) on ScalarE.
      phase 2:  denom/alpha via selection matmuls; agg^T = sum_e alpha *
                xle (selection matmul, bf16); logits = agg^T-chunks @ Wf_stack
                (+folded bias row); row softmax -> output.

kernel(**inputs) takes FULL inputs, shards internally, returns FULL [8000,460] f32.
"""

import numpy as np
import ml_dtypes

BF16 = ml_dtypes.bfloat16
E4M3 = ml_dtypes.float8_e4m3   # IEEE e4m3: max 240, has inf (matches mybir float8e4)
F8MAX = 240.0

# Problem constants (hardcoded per spec)
N = 8000
DIN = 1024
H = 3
C = 1024
HC = H * C          # 3072
NCLS = 460
NCLS_P = 512
NEG_SLOPE = 0.2
NCORES = 8
ND = N // NCORES    # 1000 dst nodes per core
NDP = 1024          # padded dst count per core
DB = NDP // 128     # 8 dst blocks per core
P = 128
KC = DIN // P       # 8 contraction chunks (4 DoubleRow pairs)
NB = 3              # n-chunks of 1024 in HC (one per head)
HF = 2              # 512-wide matmul halves per 1024 chunk
NAUG = 8            # aug columns: per-head pos/neg sums (6) + pad (2)
HCA = HC + NAUG     # 3080
AUGDIV = 64.0       # aug columns scaled by 1/64 to stay in fp8 range


def _prep(x, edge_index, Wl, bl, Wr, br, att, bias, Wf, bf):
    """Host-side preprocessing -> per-core input maps + static dims."""
    x = np.asarray(x, np.float32)
    ei = np.asarray(edge_index).astype(np.int64)
    Wl = np.asarray(Wl, np.float32)
    Wr = np.asarray(Wr, np.float32)
    bl = np.asarray(bl, np.float32)
    br = np.asarray(br, np.float32)
    att = np.asarray(att, np.float32)
    bias = np.asarray(bias, np.float32)
    Wf = np.asarray(Wf, np.float32)
    bf = np.asarray(bf, np.float32)

    assert np.all(bl == 0) and np.all(br == 0), \
        "nonzero bl/br not supported by this kernel build"

    loops = np.arange(N, dtype=np.int64)
    src_all = np.concatenate([ei[:, 0], loops])
    dst_all = np.concatenate([ei[:, 1], loops])

    # att folding: per head, column scale att_eff (clamped so fp8 columns
    # stay out of the subnormal range) and permutation pos-first
    absatt = np.abs(att)                       # [H, C]
    att_eff = np.maximum(absatt, absatt.max(1, keepdims=True) / 64.0)
    perm = np.zeros((H, C), np.int64)          # perm[h, newc] = origc
    npos = np.zeros(H, np.int64)
    for h in range(H):
        pos = np.nonzero(att[h] > 0)[0]
        neg = np.nonzero(att[h] <= 0)[0]
        perm[h] = np.concatenate([pos, neg])
        npos[h] = len(pos)

    # scaled/permuted projection weights  [DIN, HC]
    Wl_s = np.zeros((DIN, HC), np.float32)
    Wr_s = np.zeros((DIN, HC), np.float32)
    for h in range(H):
        sc = att_eff[h, perm[h]]               # [C]
        Wl_s[:, h * C:(h + 1) * C] = Wl[:, h * C + perm[h]] * sc
        Wr_s[:, h * C:(h + 1) * C] = Wr[:, h * C + perm[h]] * sc

    # global fp8 scale S: covers weight absmax and a 6-sigma bound on the
    # projected activations (xr'' must fit fp8 storage after the matmul)
    colnorm = max(np.linalg.norm(Wl_s, axis=0).max(),
                  np.linalg.norm(Wr_s, axis=0).max())
    S = float(max(np.abs(Wl_s).max() / F8MAX, np.abs(Wr_s).max() / F8MAX,
                  colnorm * 7.0 / F8MAX))

    def with_aug(W):
        W8 = np.clip(W / S, -F8MAX, F8MAX).astype(E4M3)
        Wa = np.zeros((DIN, HCA), E4M3)
        Wa[:, :HC] = W8
        W8f = W8.astype(np.float32)
        for h in range(H):
            np_h = int(npos[h])
            Wa[:, HC + 2 * h] = (W8f[:, h * C:h * C + np_h].sum(1) / AUGDIV).astype(E4M3)
            Wa[:, HC + 2 * h + 1] = (W8f[:, h * C + np_h:(h + 1) * C].sum(1) / AUGDIV).astype(E4M3)
        return Wa
    wl8 = with_aug(Wl_s)
    wr8 = with_aug(Wr_s)

    # final fc stack: logits = sum_h (agg''_h * S/(3*att_eff)) @ Wf  (+ bias@Wf + bf)
    Wfs = np.zeros((HC, NCLS_P), np.float32)
    for h in range(H):
        sc = S / (3.0 * att_eff[h, perm[h]])
        Wfs[h * C:(h + 1) * C, :NCLS] = Wf[perm[h]] * sc[:, None]
    bf2 = np.full((1, NCLS_P), -1e30, np.float32)
    bf2[0, :NCLS] = bias @ Wf + bf

    xT8 = np.clip(np.ascontiguousarray(x.T), -F8MAX, F8MAX).astype(E4M3)     # [DIN, N]

    # per-core edge grouping: edges (incl. self loops) by dst block
    cores = []
    tmax = 1
    for k in range(NCORES):
        lo, hi = k * ND, (k + 1) * ND
        m = (dst_all >= lo) & (dst_all < hi)
        s_k = src_all[m]
        dl_k = (dst_all[m] - lo).astype(np.int64)
        order = np.argsort(dl_k, kind="stable")
        s_k, dl_k = s_k[order], dl_k[order]
        blocks = []
        for db in range(DB):
            bm = (dl_k >= db * 128) & (dl_k < (db + 1) * 128)
            blocks.append((s_k[bm], dl_k[bm]))
            tmax = max(tmax, (len(blocks[-1][0]) + 127) // 128)
        cores.append(blocks)

    T_BLK = tmax
    E1T = DB * T_BLK
    E1P = E1T * 128

    iotaF = np.tile(np.arange(128, dtype=np.float32)[None, :], (128, 1))
    iotaP = np.tile(np.arange(128, dtype=np.float32)[:, None], (1, 128))
    ones1 = np.ones((1, 128), BF16)
    # unique srcs per core -> common padded tile count
    uniq = []
    for k in range(NCORES):
        srcs = np.concatenate([cores[k][db][0] for db in range(DB)])
        uniq.append(np.unique(srcs))
    UT = max((len(u) + 127) // 128 for u in uniq)
    UP = UT * 128
    in_maps = []
    for k in range(NCORES):
        srcslot = np.zeros(E1P, np.int64)
        real = np.zeros(E1P, bool)
        dstloc = np.full(E1P, -1.0, np.float32)
        for db in range(DB):
            s_k, dl_k = cores[k][db]
            base = db * T_BLK * 128
            srcslot[base:base + len(s_k)] = s_k
            real[base:base + len(s_k)] = True
            dstloc[base:base + len(s_k)] = dl_k.astype(np.float32)
        u = uniq[k]
        xuT = np.zeros((DIN, UP), E4M3)
        xuT[:, :len(u)] = xT8[:, u]
        srcloc = np.zeros((E1P, 1), np.int32)
        srcloc[real, 0] = np.searchsorted(u, srcslot[real]).astype(np.int32)
        xdstT = np.zeros((DIN, NDP), E4M3)
        xdstT[:, :ND] = xT8[:, k * ND:(k + 1) * ND]
        dst_col = np.ascontiguousarray(dstloc.reshape(E1T, 128).T)   # [128, E1T]
        src_col = np.ascontiguousarray(srcloc.reshape(E1T, 128).T)   # [128, E1T] int32
        dst_row = np.tile(dstloc[None, :], (128, 1))                 # [128, E1P]
        in_maps.append({
            "xuT": xuT,
            "srcloc": srcloc,
            "srccp": src_col,
            "xdstT": xdstT,
            "wl": wl8,
            "wr": wr8,
            "wfs": Wfs.astype(BF16),
            "bf2": bf2.astype(BF16),
            "dstcp": dst_col,
            "dstrow": dst_row,
            "iotaF": iotaF,
            "iotaP": iotaP,
            "ones1": ones1,
        })
    dims = {"T_BLK": T_BLK, "E1T": E1T, "E1P": E1P, "UT": UT,
            "npos": [int(v) for v in npos], "S": S}
    return in_maps, dims


def _build(dims, debug=False):
    """Trace the Bass/Tile program (identical for all cores)."""
    import concourse.bass as bass
    import concourse.mybir as mybir
    import concourse.tile as tile
    from concourse import bacc

    T_BLK, E1T, E1P = dims["T_BLK"], dims["E1T"], dims["E1P"]
    UT = dims["UT"]
    npos = dims["npos"]
    S = dims["S"]
    UP = UT * 128
    fp32 = mybir.dt.float32
    bf16 = mybir.dt.bfloat16
    fp8 = mybir.dt.float8e4
    AT = mybir.AluOpType
    AF = mybir.ActivationFunctionType
    DRM = mybir.MatmulPerfMode.DoubleRow

    nc = bacc.Bacc("TRN2", target_bir_lowering=False, debug=False)

    with tile.TileContext(nc) as tc:
        with tc.tile_pool(name="dram", bufs=1, space="DRAM") as dram:
            d_xuT = dram.tile([DIN, UP], fp8, kind="ExternalInput", name="xuT", uniquify=False)
            d_srcloc = dram.tile([E1P, 1], mybir.dt.int32, kind="ExternalInput", name="srcloc", uniquify=False)
            d_srccp = dram.tile([128, E1T], mybir.dt.int32, kind="ExternalInput", name="srccp", uniquify=False)
            d_xdstT = dram.tile([DIN, NDP], fp8, kind="ExternalInput", name="xdstT", uniquify=False)
            d_wl = dram.tile([DIN, HCA], fp8, kind="ExternalInput", name="wl", uniquify=False)
            d_wr = dram.tile([DIN, HCA], fp8, kind="ExternalInput", name="wr", uniquify=False)
            d_wfs = dram.tile([HC, NCLS_P], bf16, kind="ExternalInput", name="wfs", uniquify=False)
            d_bf2 = dram.tile([1, NCLS_P], bf16, kind="ExternalInput", name="bf2", uniquify=False)
            d_dstcp = dram.tile([128, E1T], fp32, kind="ExternalInput", name="dstcp", uniquify=False)
            d_dstrow = dram.tile([128, E1P], fp32, kind="ExternalInput", name="dstrow", uniquify=False)
            d_iotaF = dram.tile([128, 128], fp32, kind="ExternalInput", name="iotaF", uniquify=False)
            d_iotaP = dram.tile([128, 128], fp32, kind="ExternalInput", name="iotaP", uniquify=False)
            d_ones1 = dram.tile([1, 128], bf16, kind="ExternalInput", name="ones1", uniquify=False)
            d_out = dram.tile([NDP, NCLS_P], fp32, kind="ExternalOutput", name="out", uniquify=False)
            d_xlu = dram.tile([UP, HCA], bf16, name="xlu_i",
                              kind="ExternalOutput" if debug else "Internal",
                              uniquify=False)
            if debug:
                d_dbga = dram.tile([128, E1T * H], fp32, kind="ExternalOutput", name="dbg_a", uniquify=False)
                d_dbgx = dram.tile([128, DB, HCA], fp8, kind="ExternalOutput", name="dbg_xrd", uniquify=False)

            with tc.tile_pool(name="gsb", bufs=1) as gsb:
                # resident tensors
                dstcp = gsb.tile([128, E1T], fp32, name="dstcp_r")
                nc.sync.dma_start(out=dstcp[:], in_=d_dstcp[:])
                srccp = gsb.tile([128, E1T], mybir.dt.int32, name="srccp_r")
                nc.sync.dma_start(out=srccp[:], in_=d_srccp[:])
                iotaF = gsb.tile([128, 128], fp32, name="iotaF_r")
                nc.sync.dma_start(out=iotaF[:], in_=d_iotaF[:])
                iotaP = gsb.tile([128, 128], fp32, name="iotaP_r")
                nc.sync.dma_start(out=iotaP[:], in_=d_iotaP[:])
                ones1 = gsb.tile([1, 128], bf16, name="ones1_r")
                nc.sync.dma_start(out=ones1[:], in_=d_ones1[:])
                bf2 = gsb.tile([1, NCLS_P], bf16, name="bf2_r")
                nc.sync.dma_start(out=bf2[:], in_=d_bf2[:])
                a_all = gsb.tile([128, E1T * H], fp32, name="a_all_r")
                denr = gsb.tile([128, DB * H], fp32, name="denr_r")
                xrdb8 = gsb.tile([128, DB, HCA], fp8, name="xrdb8_r")
                idm = gsb.tile([128, 128], bf16, name="idm_r")
                nc.vector.scalar_tensor_tensor(
                    out=idm[:], in0=iotaP[:], scalar=0.0, in1=iotaF[:],
                    op0=AT.add, op1=AT.is_equal)

                # ---------------- phase 0: XR'' projection (fp8 DR) ----------------
                with tc.tile_pool(name="p01", bufs=1, space="PSUM") as ps1:
                    with tc.tile_pool(name="wpool", bufs=1) as wpool:
                        wmat = wpool.tile([128, KC, HCA], fp8, tag="wmat", bufs=1, name="wmat_r")
                        xdst8 = wpool.tile([128, KC, NDP], fp8, tag="xdst8", bufs=1, name="xdst8_r")
                        for kp in range(KC // 2):
                            nc.sync.dma_start(
                                out=xdst8[:, 2 * kp:2 * kp + 2, :],
                                in_=d_xdstT[kp * 256:(kp + 1) * 256, :]
                                    .rearrange("(kc p) n -> p kc n", p=128))
                            nc.sync.dma_start(
                                out=wmat[:, 2 * kp:2 * kp + 2, :],
                                in_=d_wr[kp * 256:(kp + 1) * 256, :]
                                    .rearrange("(kc p) n -> p kc n", p=128))
                        # phase 0 + 0.5 interleaved: XR'' block and XLu'' tiles
                        # alternate so PE fills each other's copy/DMA stalls
                        wmat2 = wpool.tile([128, KC, HCA], fp8, tag="wmat2", bufs=1, name="wmat_r2")
                        xu8 = wpool.tile([128, KC, UP], fp8, tag="xu8", bufs=1, name="xu8_r")
                        for kp in range(KC // 2):
                            nc.sync.dma_start(
                                out=wmat2[:, 2 * kp:2 * kp + 2, :],
                                in_=d_wl[kp * 256:(kp + 1) * 256, :]
                                    .rearrange("(kc p) n -> p kc n", p=128))
                            nc.sync.dma_start(
                                out=xu8[:, 2 * kp:2 * kp + 2, :],
                                in_=d_xuT[kp * 256:(kp + 1) * 256, :]
                                    .rearrange("(kc p) n -> p kc n", p=128))

                        def emit_xr(dc):
                            for nb in range(NB):
                                pp = ps1.tile([128, C], fp32, tag="pp", bufs=2, name=f"pp{nb}_c{dc}")
                                for kp in range(KC // 2):
                                    for hf in range(HF):
                                        nc.tensor.matmul(
                                            pp[:, hf * 512:(hf + 1) * 512],
                                            xdst8[:, 2 * kp:2 * kp + 2, dc * 128:(dc + 1) * 128],
                                            wmat[:, 2 * kp:2 * kp + 2, nb * C + hf * 512:nb * C + (hf + 1) * 512],
                                            start=(kp == 0), stop=(kp == KC // 2 - 1),
                                            perf_mode=DRM)
                                nc.vector.tensor_copy(out=xrdb8[:, dc, nb * C:(nb + 1) * C],
                                                      in_=pp[:])
                            ppa = ps1.tile([128, NAUG], fp32, tag="ppaug", bufs=1, name=f"ppa_c{dc}")
                            for kp in range(KC // 2):
                                nc.tensor.matmul(
                                    ppa[:], xdst8[:, 2 * kp:2 * kp + 2, dc * 128:(dc + 1) * 128],
                                    wmat[:, 2 * kp:2 * kp + 2, HC:HCA],
                                    start=(kp == 0), stop=(kp == KC // 2 - 1),
                                    perf_mode=DRM)
                            nc.vector.tensor_copy(out=xrdb8[:, dc, HC:HCA], in_=ppa[:])

                        with tc.tile_pool(name="sb0", bufs=2) as sb0:
                            def emit_xlu(ut):
                                xl_sb = sb0.tile([128, HCA], bf16, tag="xl_sb", bufs=3)
                                for nb in range(NB):
                                    pp = ps1.tile([128, C], fp32, tag="pp", bufs=2, name=f"ppu{nb}_{ut}")
                                    for kp in range(KC // 2):
                                        for hf in range(HF):
                                            nc.tensor.matmul(
                                                pp[:, hf * 512:(hf + 1) * 512],
                                                xu8[:, 2 * kp:2 * kp + 2, ut * 128:(ut + 1) * 128],
                                                wmat2[:, 2 * kp:2 * kp + 2, nb * C + hf * 512:nb * C + (hf + 1) * 512],
                                                start=(kp == 0), stop=(kp == KC // 2 - 1),
                                                perf_mode=DRM)
                                    if nb == 0:
                                        nc.vector.tensor_copy(out=xl_sb[:, nb * C:(nb + 1) * C], in_=pp[:])
                                    else:
                                        nc.scalar.activation(out=xl_sb[:, nb * C:(nb + 1) * C],
                                                             in_=pp[:], func=AF.Copy)
                                ppa = ps1.tile([128, NAUG], fp32, tag="ppaug", bufs=1, name=f"ppa_u{ut}")
                                for kp in range(KC // 2):
                                    nc.tensor.matmul(
                                        ppa[:], xu8[:, 2 * kp:2 * kp + 2, ut * 128:(ut + 1) * 128],
                                        wmat2[:, 2 * kp:2 * kp + 2, HC:HCA],
                                        start=(kp == 0), stop=(kp == KC // 2 - 1),
                                        perf_mode=DRM)
                                nc.scalar.activation(out=xl_sb[:, HC:HCA], in_=ppa[:], func=AF.Copy)
                                nc.sync.dma_start(out=d_xlu[ut * 128:(ut + 1) * 128, :], in_=xl_sb[:])

                            for dc in range(DB):
                                emit_xr(dc)
                            for ut_i in range(UT):
                                emit_xlu(ut_i)

                    # wfs resident (phase-0 weight slots are free now)
                    wfs_r = gsb.tile([128, HC // 128, NCLS_P], bf16, name="wfs_r")
                    nc.sync.dma_start(out=wfs_r[:], in_=d_wfs[:].rearrange("(cc p) n -> p cc n", p=128))

                    # ------- fused phases 1+2: per dst block -------
                    with (
                        tc.tile_pool(name="sb1", bufs=3) as sb1,
                        tc.tile_pool(name="sbe", bufs=T_BLK + 4) as sbe,
                        tc.tile_pool(name="p2", bufs=1, space="PSUM") as ps2,
                        tc.tile_pool(name="sb2", bufs=3) as sb2,
                    ):
                        for db in range(DB):
                            pb = db // 2
                            jact = db % 2
                            xles, esels, sals_all, drows = [], [], [], []
                            esc_all = sb1.tile([128, T_BLK * H], fp32, tag="esc_all",
                                               bufs=2, name=f"escall_{db}")
                            drow_db = sbe.tile([128, T_BLK * 128], fp32, tag="drow",
                                               bufs=2, name=f"drow_{db}")
                            nc.sync.dma_start(
                                out=drow_db[:],
                                in_=d_dstrow[:, db * T_BLK * 128:(db + 1) * T_BLK * 128])
                            for t2 in range(T_BLK):
                                t = db * T_BLK + t2
                                drow = drow_db[:, t2 * 128:(t2 + 1) * 128]
                                drows.append(drow)
                                # gather XL''[src] rows (bf16)
                                xle = sbe.tile([128, HCA], bf16, tag="xle", bufs=2 * T_BLK + 1, name=f"xle_{db}_{t2}")
                                nc.gpsimd.indirect_dma_start(
                                    out=xle[:], out_offset=None, in_=d_xlu[:],
                                    in_offset=bass.IndirectOffsetOnAxis(ap=srccp[:, t:t + 1], axis=0))
                                xles.append(xle)
                                # pair-packed 0/1 dst-selection for DoubleRow U-expansion
                                eselw = sb1.tile([128, 2, 128], fp8, tag="eselw")
                                nc.gpsimd.memset(eselw[:, 1 - jact, :], 0.0)
                                nc.vector.scalar_tensor_tensor(
                                    out=eselw[:, jact, :], in0=drow,
                                    scalar=float(-db * 128), in1=iotaP[:],
                                    op0=AT.add, op1=AT.is_equal)
                                eacc = sb1.tile([128, 2 * H], fp32, tag="eacc")
                                scrD = sb1.tile([128, C], fp32, tag="scrD", bufs=2)
                                scrA = sb1.tile([128, C], fp32, tag="scrA", bufs=2)
                                # aug sums first: frees the single-buffered aug bank
                                # early and overlaps the aug copy with head scores
                                ppa = ps1.tile([128, NAUG], fp32, tag="ppaug", bufs=1, name=f"ppa_t{t}")
                                nc.tensor.matmul(
                                    ppa[:], eselw[:], xrdb8[:, 2 * pb:2 * pb + 2, HC:HCA],
                                    start=True, stop=False, perf_mode=DRM)
                                nc.tensor.matmul(
                                    ppa[:], idm[:], xle[:, HC:HCA],
                                    start=False, stop=True)
                                aug8 = sb1.tile([128, NAUG], fp32, tag="aug8")
                                nc.scalar.activation(out=aug8[:], in_=ppa[:], func=AF.Copy)
                                for h in range(H):
                                    use_act = (h == 1) or (h == 2 and t2 % 2 == 0)
                                    # U/S for head h directly in PSUM:
                                    #   DR dst-selection matmul of XR'' + identity matmul of xle
                                    pp = ps1.tile([128, C], fp32, tag="pp", bufs=2, name=f"ppe{h}_t{t}")
                                    for hf in range(HF):
                                        cl = h * C + hf * 512
                                        nc.tensor.matmul(
                                            pp[:, hf * 512:(hf + 1) * 512], eselw[:],
                                            xrdb8[:, 2 * pb:2 * pb + 2, cl:cl + 512],
                                            start=True, stop=False, perf_mode=DRM)
                                        nc.tensor.matmul(
                                            pp[:, hf * 512:(hf + 1) * 512], idm[:],
                                            xle[:, cl:cl + 512],
                                            start=False, stop=True)
                                    # scores: pos/neg relu accumulation off PSUM
                                    # (leaky = 0.8*relu(u) + 0.2*u; sum(u) rides the aug cols)
                                    np_h = npos[h]
                                    segs = [(0, np_h, 2 * h), (np_h, C - np_h, 2 * h + 1)]
                                    for off, ln, j in segs:
                                        if ln == 0:
                                            nc.vector.memset(eacc[:, j:j + 1], 0.0)
                                            continue
                                        if use_act:
                                            nc.scalar.activation(
                                                out=scrA[:, :ln], in_=pp[:, off:off + ln],
                                                func=AF.Relu, accum_out=eacc[:, j:j + 1])
                                        else:
                                            nc.vector.tensor_scalar(
                                                out=scrD[:, :ln], in0=pp[:, off:off + ln],
                                                scalar1=0.0, scalar2=0.0, op0=AT.max,
                                                op1=AT.add, accum_out=eacc[:, j:j + 1])
                                rdif = sb1.tile([128, H], fp32, tag="rdif")
                                nc.vector.tensor_tensor(
                                    out=rdif[:], in0=eacc[:, 0:2 * H:2],
                                    in1=eacc[:, 1:2 * H:2], op=AT.subtract)
                                adif = sb1.tile([128, H], fp32, tag="adif")
                                nc.vector.tensor_tensor(
                                    out=adif[:], in0=aug8[:, 0:2 * H:2],
                                    in1=aug8[:, 1:2 * H:2], op=AT.subtract)
                                nc.vector.scalar_tensor_tensor(
                                    out=esc_all[:, t2 * H:(t2 + 1) * H], in0=adif[:],
                                    scalar=float(AUGDIV * NEG_SLOPE / (1.0 - NEG_SLOPE)),
                                    in1=rdif[:], op0=AT.mult, op1=AT.add)
                            # one Exp per dst block (avoids per-tile act-table swaps)
                            nc.scalar.activation(
                                out=a_all[:, db * T_BLK * H:(db + 1) * T_BLK * H],
                                in_=esc_all[:], func=AF.Exp,
                                scale=float(S * (1.0 - NEG_SLOPE)))
                            # pass A: denominators
                            pden = ps2.tile([128, H], fp32, tag="psmall", bufs=2, name=f"pden_{db}")
                            for t2 in range(T_BLK):
                                t = db * T_BLK + t2
                                ee = sbe.tile([128, 128], fp32, tag="esel_et", name=f"eet_{db}_{t2}")
                                nc.vector.scalar_tensor_tensor(
                                    out=ee[:], in0=dstcp[:, t:t + 1].to_broadcast([128, 128]),
                                    scalar=float(-db * 128), in1=iotaF[:],
                                    op0=AT.add, op1=AT.is_equal)
                                esels.append(ee)
                                nc.tensor.matmul(
                                    pden[:], ee[:], a_all[:, t * H:(t + 1) * H],
                                    start=(t2 == 0), stop=(t2 == T_BLK - 1))
                            dtmp = sb2.tile([128, H], fp32, tag="dtmp")
                            nc.vector.tensor_scalar_add(out=dtmp[:], in0=pden[:], scalar1=1e-16)
                            nc.vector.reciprocal(out=denr[:, db * H:(db + 1) * H], in_=dtmp[:])
                            # pass B: alpha and selection weights
                            for t2 in range(T_BLK):
                                t = db * T_BLK + t2
                                esde = sb2.tile([128, 128], fp32, tag="esde", bufs=4)
                                nc.vector.scalar_tensor_tensor(
                                    out=esde[:], in0=drows[t2],
                                    scalar=float(-db * 128), in1=iotaP[:],
                                    op0=AT.add, op1=AT.is_equal)
                                pad = ps2.tile([128, H], fp32, tag="psmall", bufs=2, name=f"pad_{db}_{t2}")
                                nc.tensor.matmul(pad[:], esde[:], denr[:, db * H:(db + 1) * H],
                                                 start=True, stop=True)
                                alpha = sb2.tile([128, H], fp32, tag="alpha")
                                nc.vector.tensor_tensor(out=alpha[:], in0=a_all[:, t * H:(t + 1) * H],
                                                        in1=pad[:], op=AT.mult)
                                hsal = []
                                for h in range(H):
                                    sal = sb2.tile([128, 128], bf16, tag=f"sal{h}",
                                                   bufs=2 * T_BLK + 1, name=f"sal{h}_{db}_{t2}")
                                    nc.vector.tensor_tensor(
                                        out=sal[:], in0=esels[t2][:],
                                        in1=alpha[:, h:h + 1].to_broadcast([128, 128]),
                                        op=AT.mult)
                                    hsal.append(sal)
                                sals_all.append(hsal)
                            # transposed aggregation (xle-based: out = sum alpha*xl) + fc
                            NGRP = 4            # cc chunks per PSUM group (1 bank)
                            oaggs = []
                            for g in range(HC // 128 // NGRP):
                                pagg = ps2.tile([128, NGRP * 128], fp32, tag="pagg",
                                                bufs=1, name=f"pagg_{db}_{g}")
                                oagg = sb2.tile([128, NGRP * 128], bf16, tag="oagg", bufs=6,
                                                name=f"oagg_{db}_{g}")
                                for j in range(NGRP):
                                    cc = g * NGRP + j
                                    h = cc // (HC // 128 // H)
                                    for t2 in range(T_BLK):
                                        nc.tensor.matmul(
                                            pagg[:, j * 128:(j + 1) * 128],
                                            xles[t2][:, cc * 128:(cc + 1) * 128],
                                            sals_all[t2][h][:],
                                            start=(t2 == 0), stop=(t2 == T_BLK - 1))
                                if g % 2 == 0:
                                    nc.scalar.activation(out=oagg[:], in_=pagg[:], func=AF.Copy)
                                else:
                                    nc.vector.tensor_copy(out=oagg[:], in_=pagg[:])
                                oaggs.append(oagg)
                            pfc = ps2.tile([128, NCLS_P], fp32, tag="psmall", bufs=2, name=f"pfc_{db}")
                            for cc in range(HC // 128):
                                nc.tensor.matmul(
                                    pfc[:], oaggs[cc // NGRP][:, (cc % NGRP) * 128:(cc % NGRP + 1) * 128],
                                    wfs_r[:, cc, :],
                                    start=(cc == 0), stop=False)
                            nc.tensor.matmul(pfc[:], ones1[:], bf2[:], start=False, stop=True)
                            # softmax
                            negmax = sb2.tile([128, 1], fp32, tag="negmax")
                            nc.vector.tensor_reduce(out=negmax[:], in_=pfc[:],
                                                    axis=mybir.AxisListType.X,
                                                    op=AT.max, negate=True)
                            pexp = sb2.tile([128, NCLS_P], fp32, tag="pexp", bufs=2)
                            nc.scalar.activation(out=pexp[:], in_=pfc[:], func=AF.Exp,
                                                 bias=negmax[:, 0:1], scale=1.0)
                            ssum = sb2.tile([128, 1], fp32, tag="ssum")
                            nc.vector.tensor_reduce(out=ssum[:], in_=pexp[:],
                                                    axis=mybir.AxisListType.X, op=AT.add)
                            rs = sb2.tile([128, 1], fp32, tag="rs")
                            nc.vector.reciprocal(out=rs[:], in_=ssum[:])
                            hout = sb2.tile([128, NCLS_P], fp32, tag="hout", bufs=2)
                            nc.vector.scalar_tensor_tensor(
                                out=hout[:], in0=pexp[:], scalar=rs[:, 0:1], in1=pexp[:],
                                op0=AT.mult, op1=AT.bypass)
                            nc.sync.dma_start(out=d_out[db * 128:(db + 1) * 128, :], in_=hout[:])
                        if debug:
                            nc.sync.dma_start(out=d_dbga[:], in_=a_all[:])
                            nc.sync.dma_start(out=d_dbgx[:], in_=xrdb8[:])

    nc.compile()
    return nc


def kernel(**inputs):
    out_full = np.zeros((N, NCLS), np.float32)
    in_maps, dims = _prep(
        inputs["x"], inputs["edge_index"], inputs["Wl"], inputs["bl"],
        inputs["Wr"], inputs["br"], inputs["att"], inputs["bias"],
        inputs["Wf"], inputs["bf"])
    nc = _build(dims)
    from concourse.bass_utils import run_bass_kernel_spmd
    res = run_bass_kernel_spmd(nc, in_maps, core_ids=list(range(NCORES)))
    for k in range(NCORES):
        out_full[k * ND:(k + 1) * ND, :] = res.results[k]["out"][:ND, :NCLS]
    return out_full


# revision 22
# speedup vs baseline: 6.3198x; 1.0663x over previous
"""GATv2 + softmax head for nn_GATModel_Softmax_4535485465120 on 8 trn2 NeuronCores.

v2: fp8-DoubleRow projections + xle-based aggregation (no XR-transpose phase).

Strategy (graph/data parallel by dst node, fully local — no collectives):
  - Nodes partitioned into 8 ranges of 1000 dst nodes (one per core).
  - Host preps per core: x.T columns (fp8) for the core's unique src nodes and
    its dst nodes; weights with att-magnitude (clamped at max/64 per head)
    folded into Wl/Wr columns, permuted pos-att-first, scaled 1/S for fp8;
    Wf rows carry the inverse permutation, S/(3*att_eff) un-scaling and the
    head-mean.
  - Device pipeline per core:
      phase 0:  XR'' = xdst8 @ Wr8 via fp8 DoubleRow matmuls -> SBUF fp8
                (resident, pair-packed by dst block for DR reuse)
      phase 0.5: XLu'' = xu8 @ Wl8 (DR) -> HBM bf16 [UP, 3072]
      phase 1 (per dst block, per 128-edge tile): U/S accumulated directly in
                PSUM = DR-selection-matmul of XR'' + identity-matmul of
                gathered XLu'' rows; leaky-relu + sign-segmented accumulation
                on DVE reads PSUM directly; a = exp(S * esc) on ScalarE.
      phase 2:  denom/alpha via selection matmuls; agg^T = sum_e alpha *
                xle (selection matmul, bf16); logits = agg^T-chunks @ Wf_stack
                (+folded bias row); row softmax -> output.

kernel(**inputs) takes FULL inputs, shards internally, returns FULL [8000,460] f32.
"""

import numpy as np
import ml_dtypes

BF16 = ml_dtypes.bfloat16
E4M3 = ml_dtypes.float8_e4m3   # IEEE e4m3: max 240, has inf (matches mybir float8e4)
F8MAX = 240.0

# Problem constants (hardcoded per spec)
N = 8000
DIN = 1024
H = 3
C = 1024
HC = H * C          # 3072
NCLS = 460
NCLS_P = 512
NEG_SLOPE = 0.2
NCORES = 8
ND = N // NCORES    # 1000 dst nodes per core
NDP = 1024          # padded dst count per core
DB = NDP // 128     # 8 dst blocks per core
P = 128
KC = DIN // P       # 8 contraction chunks (4 DoubleRow pairs)
NB = 3              # n-chunks of 1024 in HC (one per head)
HF = 2              # 512-wide matmul halves per 1024 chunk
NAUG = 8            # aug columns: per-head pos/neg sums (6) + pad (2)
HCA = HC + NAUG     # 3080
AUGDIV = 64.0       # aug columns scaled by 1/64 to stay in fp8 range


def _prep(x, edge_index, Wl, bl, Wr, br, att, bias, Wf, bf):
    """Host-side preprocessing -> per-core input maps + static dims."""
    x = np.asarray(x, np.float32)
    ei = np.asarray(edge_index).astype(np.int64)
    Wl = np.asarray(Wl, np.float32)
    Wr = np.asarray(Wr, np.float32)
    bl = np.asarray(bl, np.float32)
    br = np.asarray(br, np.float32)
    att = np.asarray(att, np.float32)
    bias = np.asarray(bias, np.float32)
    Wf = np.asarray(Wf, np.float32)
    bf = np.asarray(bf, np.float32)

    assert np.all(bl == 0) and np.all(br == 0), \
        "nonzero bl/br not supported by this kernel build"

    loops = np.arange(N, dtype=np.int64)
    src_all = np.concatenate([ei[:, 0], loops])
    dst_all = np.concatenate([ei[:, 1], loops])

    # att folding: per head, column scale att_eff (clamped so fp8 columns
    # stay out of the subnormal range) and permutation pos-first
    absatt = np.abs(att)                       # [H, C]
    att_eff = np.maximum(absatt, absatt.max(1, keepdims=True) / 64.0)
    perm = np.zeros((H, C), np.int64)          # perm[h, newc] = origc
    npos = np.zeros(H, np.int64)
    for h in range(H):
        pos = np.nonzero(att[h] > 0)[0]
        neg = np.nonzero(att[h] <= 0)[0]
        perm[h] = np.concatenate([pos, neg])
        npos[h] = len(pos)

    # scaled/permuted projection weights  [DIN, HC]
    Wl_s = np.zeros((DIN, HC), np.float32)
    Wr_s = np.zeros((DIN, HC), np.float32)
    for h in range(H):
        sc = att_eff[h, perm[h]]               # [C]
        Wl_s[:, h * C:(h + 1) * C] = Wl[:, h * C + perm[h]] * sc
        Wr_s[:, h * C:(h + 1) * C] = Wr[:, h * C + perm[h]] * sc

    # global fp8 scale S: covers weight absmax and a 6-sigma bound on the
    # projected activations (xr'' must fit fp8 storage after the matmul)
    colnorm = max(np.linalg.norm(Wl_s, axis=0).max(),
                  np.linalg.norm(Wr_s, axis=0).max())
    S = float(max(np.abs(Wl_s).max() / F8MAX, np.abs(Wr_s).max() / F8MAX,
                  colnorm * 7.0 / F8MAX))

    def with_aug(W):
        W8 = np.clip(W / S, -F8MAX, F8MAX).astype(E4M3)
        Wa = np.zeros((DIN, HCA), E4M3)
        Wa[:, :HC] = W8
        W8f = W8.astype(np.float32)
        for h in range(H):
            np_h = int(npos[h])
            Wa[:, HC + 2 * h] = (W8f[:, h * C:h * C + np_h].sum(1) / AUGDIV).astype(E4M3)
            Wa[:, HC + 2 * h + 1] = (W8f[:, h * C + np_h:(h + 1) * C].sum(1) / AUGDIV).astype(E4M3)
        return Wa
    wl8 = with_aug(Wl_s)
    wr8 = with_aug(Wr_s)

    # final fc stack: logits = sum_h (agg''_h * S/(3*att_eff)) @ Wf  (+ bias@Wf + bf)
    Wfs = np.zeros((HC, NCLS_P), np.float32)
    for h in range(H):
        sc = S / (3.0 * att_eff[h, perm[h]])
        Wfs[h * C:(h + 1) * C, :NCLS] = Wf[perm[h]] * sc[:, None]
    bf2 = np.full((1, NCLS_P), -1e30, np.float32)
    bf2[0, :NCLS] = bias @ Wf + bf

    xT8 = np.clip(np.ascontiguousarray(x.T), -F8MAX, F8MAX).astype(E4M3)     # [DIN, N]

    # per-core edge grouping: edges (incl. self loops) by dst block
    cores = []
    tmax = 1
    for k in range(NCORES):
        lo, hi = k * ND, (k + 1) * ND
        m = (dst_all >= lo) & (dst_all < hi)
        s_k = src_all[m]
        dl_k = (dst_all[m] - lo).astype(np.int64)
        order = np.argsort(dl_k, kind="stable")
        s_k, dl_k = s_k[order], dl_k[order]
        blocks = []
        for db in range(DB):
            bm = (dl_k >= db * 128) & (dl_k < (db + 1) * 128)
            blocks.append((s_k[bm], dl_k[bm]))
            tmax = max(tmax, (len(blocks[-1][0]) + 127) // 128)
        cores.append(blocks)

    T_BLK = tmax
    E1T = DB * T_BLK
    E1P = E1T * 128

    iotaF = np.tile(np.arange(128, dtype=np.float32)[None, :], (128, 1))
    iotaP = np.tile(np.arange(128, dtype=np.float32)[:, None], (1, 128))
    ones1 = np.ones((1, 128), BF16)
    # unique srcs per core -> common padded tile count
    uniq = []
    for k in range(NCORES):
        srcs = np.concatenate([cores[k][db][0] for db in range(DB)])
        uniq.append(np.unique(srcs))
    UT = max((len(u) + 127) // 128 for u in uniq)
    UP = UT * 128
    in_maps = []
    for k in range(NCORES):
        srcslot = np.zeros(E1P, np.int64)
        real = np.zeros(E1P, bool)
        dstloc = np.full(E1P, -1.0, np.float32)
        for db in range(DB):
            s_k, dl_k = cores[k][db]
            base = db * T_BLK * 128
            srcslot[base:base + len(s_k)] = s_k
            real[base:base + len(s_k)] = True
            dstloc[base:base + len(s_k)] = dl_k.astype(np.float32)
        u = uniq[k]
        xuT = np.zeros((DIN, UP), E4M3)
        xuT[:, :len(u)] = xT8[:, u]
        srcloc = np.zeros((E1P, 1), np.int32)
        srcloc[real, 0] = np.searchsorted(u, srcslot[real]).astype(np.int32)
        xdstT = np.zeros((DIN, NDP), E4M3)
        xdstT[:, :ND] = xT8[:, k * ND:(k + 1) * ND]
        dst_col = np.ascontiguousarray(dstloc.reshape(E1T, 128).T)   # [128, E1T]
        src_col = np.ascontiguousarray(srcloc.reshape(E1T, 128).T)   # [128, E1T] int32
        dst_row = np.tile(dstloc[None, :], (128, 1))                 # [128, E1P]
        in_maps.append({
            "xuT": xuT,
            "srcloc": srcloc,
            "srccp": src_col,
            "xdstT": xdstT,
            "wl": wl8,
            "wr": wr8,
            "wfs": Wfs.astype(BF16),
            "bf2": bf2.astype(BF16),
            "dstcp": dst_col,
            "dstrow": dst_row,
            "iotaF": iotaF,
            "iotaP": iotaP,
            "ones1": ones1,
        })
    dims = {"T_BLK": T_BLK, "E1T": E1T, "E1P": E1P, "UT": UT,
            "npos": [int(v) for v in npos], "S": S}
    return in_maps, dims


def _build(dims, debug=False):
    """Trace the Bass/Tile program (identical for all cores)."""
    import concourse.bass as bass
    import concourse.mybir as mybir
    import concourse.tile as tile
    from concourse import bacc

    T_BLK, E1T, E1P = dims["T_BLK"], dims["E1T"], dims["E1P"]
    UT = dims["UT"]
    npos = dims["npos"]
    S = dims["S"]
    UP = UT * 128
    fp32 = mybir.dt.float32
    bf16 = mybir.dt.bfloat16
    fp8 = mybir.dt.float8e4
    AT = mybir.AluOpType
    AF = mybir.ActivationFunctionType
    DRM = mybir.MatmulPerfMode.DoubleRow

    nc = bacc.Bacc("TRN2", target_bir_lowering=False, debug=False)

    with tile.TileContext(nc) as tc:
        with tc.tile_pool(name="dram", bufs=1, space="DRAM") as dram:
            d_xuT = dram.tile([DIN, UP], fp8, kind="ExternalInput", name="xuT", uniquify=False)
            d_srcloc = dram.tile([E1P, 1], mybir.dt.int32, kind="ExternalInput", name="srcloc", uniquify=False)
            d_srccp = dram.tile([128, E1T], mybir.dt.int32, kind="ExternalInput", name="srccp", uniquify=False)
            d_xdstT = dram.tile([DIN, NDP], fp8, kind="ExternalInput", name="xdstT", uniquify=False)
            d_wl = dram.tile([DIN, HCA], fp8, kind="ExternalInput", name="wl", uniquify=False)
            d_wr = dram.tile([DIN, HCA], fp8, kind="ExternalInput", name="wr", uniquify=False)
            d_wfs = dram.tile([HC, NCLS_P], bf16, kind="ExternalInput", name="wfs", uniquify=False)
            d_bf2 = dram.tile([1, NCLS_P], bf16, kind="ExternalInput", name="bf2", uniquify=False)
            d_dstcp = dram.tile([128, E1T], fp32, kind="ExternalInput", name="dstcp", uniquify=False)
            d_dstrow = dram.tile([128, E1P], fp32, kind="ExternalInput", name="dstrow", uniquify=False)
            d_iotaF = dram.tile([128, 128], fp32, kind="ExternalInput", name="iotaF", uniquify=False)
            d_iotaP = dram.tile([128, 128], fp32, kind="ExternalInput", name="iotaP", uniquify=False)
            d_ones1 = dram.tile([1, 128], bf16, kind="ExternalInput", name="ones1", uniquify=False)
            d_out = dram.tile([NDP, NCLS_P], fp32, kind="ExternalOutput", name="out", uniquify=False)
            d_xlu = dram.tile([UP, HCA], bf16, name="xlu_i",
                              kind="ExternalOutput" if debug else "Internal",
                              uniquify=False)
            if debug:
                d_dbga = dram.tile([128, E1T * H], fp32, kind="ExternalOutput", name="dbg_a", uniquify=False)
                d_dbgx = dram.tile([128, DB, HCA], fp8, kind="ExternalOutput", name="dbg_xrd", uniquify=False)

            with tc.tile_pool(name="gsb", bufs=1) as gsb:
                # resident tensors
                dstcp = gsb.tile([128, E1T], fp32, name="dstcp_r")
                nc.sync.dma_start(out=dstcp[:], in_=d_dstcp[:])
                srccp = gsb.tile([128, E1T], mybir.dt.int32, name="srccp_r")
                nc.sync.dma_start(out=srccp[:], in_=d_srccp[:])
                iotaF = gsb.tile([128, 128], fp32, name="iotaF_r")
                nc.sync.dma_start(out=iotaF[:], in_=d_iotaF[:])
                iotaP = gsb.tile([128, 128], fp32, name="iotaP_r")
                nc.sync.dma_start(out=iotaP[:], in_=d_iotaP[:])
                ones1 = gsb.tile([1, 128], bf16, name="ones1_r")
                nc.sync.dma_start(out=ones1[:], in_=d_ones1[:])
                bf2 = gsb.tile([1, NCLS_P], bf16, name="bf2_r")
                nc.sync.dma_start(out=bf2[:], in_=d_bf2[:])
                a_all = gsb.tile([128, E1T * H], fp32, name="a_all_r")
                denr = gsb.tile([128, DB * H], fp32, name="denr_r")
                xrdb8 = gsb.tile([128, DB, HCA], fp8, name="xrdb8_r")
                idm = gsb.tile([128, 128], bf16, name="idm_r")
                nc.vector.scalar_tensor_tensor(
                    out=idm[:], in0=iotaP[:], scalar=0.0, in1=iotaF[:],
                    op0=AT.add, op1=AT.is_equal)

                # ---------------- phase 0: XR'' projection (fp8 DR) ----------------
                with tc.tile_pool(name="p01", bufs=1, space="PSUM") as ps1:
                    with tc.tile_pool(name="wpool", bufs=1) as wpool:
                        wmat = wpool.tile([128, KC, HCA], fp8, tag="wmat", bufs=1, name="wmat_r")
                        xdst8 = wpool.tile([128, KC, NDP], fp8, tag="xdst8", bufs=1, name="xdst8_r")
                        for kp in range(KC // 2):
                            nc.sync.dma_start(
                                out=xdst8[:, 2 * kp:2 * kp + 2, :],
                                in_=d_xdstT[kp * 256:(kp + 1) * 256, :]
                                    .rearrange("(kc p) n -> p kc n", p=128))
                            nc.sync.dma_start(
                                out=wmat[:, 2 * kp:2 * kp + 2, :],
                                in_=d_wr[kp * 256:(kp + 1) * 256, :]
                                    .rearrange("(kc p) n -> p kc n", p=128))
                        # phase 0 + 0.5 interleaved: XR'' block and XLu'' tiles
                        # alternate so PE fills each other's copy/DMA stalls
                        wmat2 = wpool.tile([128, KC, HCA], fp8, tag="wmat2", bufs=1, name="wmat_r2")
                        xu8 = wpool.tile([128, KC, UP], fp8, tag="xu8", bufs=1, name="xu8_r")
                        for kp in range(KC // 2):
                            nc.sync.dma_start(
                                out=wmat2[:, 2 * kp:2 * kp + 2, :],
                                in_=d_wl[kp * 256:(kp + 1) * 256, :]
                                    .rearrange("(kc p) n -> p kc n", p=128))
                            nc.sync.dma_start(
                                out=xu8[:, 2 * kp:2 * kp + 2, :],
                                in_=d_xuT[kp * 256:(kp + 1) * 256, :]
                                    .rearrange("(kc p) n -> p kc n", p=128))

                        def emit_xr(dc):
                            for nb in range(NB):
                                for hf in range(HF):
                                    ppu = ps1.tile([128, 512], fp32, tag="ppu",
                                                   bufs=4, name=f"pp{nb}_{hf}_c{dc}")
                                    for kp in range(KC // 2):
                                        nc.tensor.matmul(
                                            ppu[:],
                                            xdst8[:, 2 * kp:2 * kp + 2, dc * 128:(dc + 1) * 128],
                                            wmat[:, 2 * kp:2 * kp + 2, nb * C + hf * 512:nb * C + (hf + 1) * 512],
                                            start=(kp == 0), stop=(kp == KC // 2 - 1),
                                            perf_mode=DRM)
                                    nc.vector.tensor_copy(
                                        out=xrdb8[:, dc, nb * C + hf * 512:nb * C + (hf + 1) * 512],
                                        in_=ppu[:])
                            ppa = ps1.tile([128, NAUG], fp32, tag="ppaug", bufs=1, name=f"ppa_c{dc}")
                            for kp in range(KC // 2):
                                nc.tensor.matmul(
                                    ppa[:], xdst8[:, 2 * kp:2 * kp + 2, dc * 128:(dc + 1) * 128],
                                    wmat[:, 2 * kp:2 * kp + 2, HC:HCA],
                                    start=(kp == 0), stop=(kp == KC // 2 - 1),
                                    perf_mode=DRM)
                            nc.vector.tensor_copy(out=xrdb8[:, dc, HC:HCA], in_=ppa[:])

                        with tc.tile_pool(name="sb0", bufs=2) as sb0:
                            def emit_xlu(ut):
                                xl_sb = sb0.tile([128, HCA], bf16, tag="xl_sb", bufs=3)
                                for nb in range(NB):
                                    for hf in range(HF):
                                        ppu = ps1.tile([128, 512], fp32, tag="ppu",
                                                       bufs=4, name=f"ppu{nb}_{hf}_{ut}")
                                        for kp in range(KC // 2):
                                            nc.tensor.matmul(
                                                ppu[:],
                                                xu8[:, 2 * kp:2 * kp + 2, ut * 128:(ut + 1) * 128],
                                                wmat2[:, 2 * kp:2 * kp + 2, nb * C + hf * 512:nb * C + (hf + 1) * 512],
                                                start=(kp == 0), stop=(kp == KC // 2 - 1),
                                                perf_mode=DRM)
                                        cl = nb * C + hf * 512
                                        if nb == 0:
                                            nc.vector.tensor_copy(out=xl_sb[:, cl:cl + 512], in_=ppu[:])
                                        else:
                                            nc.scalar.activation(out=xl_sb[:, cl:cl + 512],
                                                                 in_=ppu[:], func=AF.Copy)
                                ppa = ps1.tile([128, NAUG], fp32, tag="ppaug", bufs=1, name=f"ppa_u{ut}")
                                for kp in range(KC // 2):
                                    nc.tensor.matmul(
                                        ppa[:], xu8[:, 2 * kp:2 * kp + 2, ut * 128:(ut + 1) * 128],
                                        wmat2[:, 2 * kp:2 * kp + 2, HC:HCA],
                                        start=(kp == 0), stop=(kp == KC // 2 - 1),
                                        perf_mode=DRM)
                                nc.scalar.activation(out=xl_sb[:, HC:HCA], in_=ppa[:], func=AF.Copy)
                                nc.sync.dma_start(out=d_xlu[ut * 128:(ut + 1) * 128, :], in_=xl_sb[:])

                            for dc in range(DB):
                                emit_xr(dc)
                            for ut_i in range(UT):
                                emit_xlu(ut_i)

                    # wfs resident (phase-0 weight slots are free now)
                    wfs_r = gsb.tile([128, HC // 128, NCLS_P], bf16, name="wfs_r")
                    nc.sync.dma_start(out=wfs_r[:], in_=d_wfs[:].rearrange("(cc p) n -> p cc n", p=128))

                    # ------- fused phases 1+2: per dst block -------
                    with (
                        tc.tile_pool(name="sb1", bufs=3) as sb1,
                        tc.tile_pool(name="sbe", bufs=T_BLK + 4) as sbe,
                        tc.tile_pool(name="p2", bufs=1, space="PSUM") as ps2,
                        tc.tile_pool(name="sb2", bufs=3) as sb2,
                    ):
                        for db in range(DB):
                            pb = db // 2
                            jact = db % 2
                            xles, esels, sals_all, drows = [], [], [], []
                            esc_all = sb1.tile([128, T_BLK * H], fp32, tag="esc_all",
                                               bufs=2, name=f"escall_{db}")
                            drow_db = sbe.tile([128, T_BLK * 128], fp32, tag="drow",
                                               bufs=2, name=f"drow_{db}")
                            nc.sync.dma_start(
                                out=drow_db[:],
                                in_=d_dstrow[:, db * T_BLK * 128:(db + 1) * T_BLK * 128])
                            for t2 in range(T_BLK):
                                t = db * T_BLK + t2
                                drow = drow_db[:, t2 * 128:(t2 + 1) * 128]
                                drows.append(drow)
                                # gather XL''[src] rows (bf16)
                                xle = sbe.tile([128, HCA], bf16, tag="xle", bufs=2 * T_BLK + 1, name=f"xle_{db}_{t2}")
                                nc.gpsimd.indirect_dma_start(
                                    out=xle[:], out_offset=None, in_=d_xlu[:],
                                    in_offset=bass.IndirectOffsetOnAxis(ap=srccp[:, t:t + 1], axis=0))
                                xles.append(xle)
                                # pair-packed 0/1 dst-selection for DoubleRow U-expansion
                                eselw = sb1.tile([128, 2, 128], fp8, tag="eselw")
                                nc.gpsimd.memset(eselw[:, 1 - jact, :], 0.0)
                                nc.vector.scalar_tensor_tensor(
                                    out=eselw[:, jact, :], in0=drow,
                                    scalar=float(-db * 128), in1=iotaP[:],
                                    op0=AT.add, op1=AT.is_equal)
                                eacc = sb1.tile([128, 4 * H], fp32, tag="eacc")
                                scrD = sb1.tile([128, C], fp32, tag="scrD", bufs=2)
                                scrA = sb1.tile([128, C], fp32, tag="scrA", bufs=2)
                                # aug sums first: frees the single-buffered aug bank
                                # early and overlaps the aug copy with head scores
                                ppa = ps1.tile([128, NAUG], fp32, tag="ppaug", bufs=1, name=f"ppa_t{t}")
                                nc.tensor.matmul(
                                    ppa[:], eselw[:], xrdb8[:, 2 * pb:2 * pb + 2, HC:HCA],
                                    start=True, stop=False, perf_mode=DRM)
                                nc.tensor.matmul(
                                    ppa[:], idm[:], xle[:, HC:HCA],
                                    start=False, stop=True)
                                aug8 = sb1.tile([128, NAUG], fp32, tag="aug8")
                                nc.scalar.activation(out=aug8[:], in_=ppa[:], func=AF.Copy)
                                for h in range(H):
                                    use_act = (h == 1) or (h == 2 and t2 % 2 == 0)
                                    np_h = npos[h]
                                    for hf in range(HF):
                                        # U/S for head h, 512-col half, in a 1-bank
                                        # PSUM group (4 in flight)
                                        ppu = ps1.tile([128, 512], fp32, tag="ppu",
                                                       bufs=4, name=f"ppe{h}_{hf}_t{t}")
                                        cl = h * C + hf * 512
                                        nc.tensor.matmul(
                                            ppu[:], eselw[:],
                                            xrdb8[:, 2 * pb:2 * pb + 2, cl:cl + 512],
                                            start=True, stop=False, perf_mode=DRM)
                                        nc.tensor.matmul(
                                            ppu[:], idm[:], xle[:, cl:cl + 512],
                                            start=False, stop=True)
                                        # scores: pos/neg relu accumulation off PSUM
                                        # (leaky = 0.8*relu(u)+0.2*u; sum(u) rides aug cols)
                                        lo, hi = hf * 512, hf * 512 + 512
                                        segs = []
                                        if np_h > lo:
                                            segs.append((lo, min(np_h, hi), 4 * h + 2 * hf))
                                        if np_h < hi:
                                            segs.append((max(np_h, lo), hi, 4 * h + 2 * hf + 1))
                                        done = set()
                                        for s0, s1, j in segs:
                                            ln = s1 - s0
                                            done.add(j)
                                            if use_act:
                                                nc.scalar.activation(
                                                    out=scrA[:, :ln], in_=ppu[:, s0 - lo:s1 - lo],
                                                    func=AF.Relu, accum_out=eacc[:, j:j + 1])
                                            else:
                                                nc.vector.tensor_scalar(
                                                    out=scrD[:, :ln], in0=ppu[:, s0 - lo:s1 - lo],
                                                    scalar1=0.0, scalar2=0.0, op0=AT.max,
                                                    op1=AT.add, accum_out=eacc[:, j:j + 1])
                                        for j in (4 * h + 2 * hf, 4 * h + 2 * hf + 1):
                                            if j not in done:
                                                nc.vector.memset(eacc[:, j:j + 1], 0.0)
                                rdif = sb1.tile([128, H], fp32, tag="rdif")
                                rtmp = sb1.tile([128, 2 * H], fp32, tag="rtmp")
                                nc.vector.tensor_tensor(
                                    out=rtmp[:], in0=eacc[:, 0:4 * H:2],
                                    in1=eacc[:, 1:4 * H:2], op=AT.subtract)
                                nc.vector.tensor_tensor(
                                    out=rdif[:], in0=rtmp[:, 0:2 * H:2],
                                    in1=rtmp[:, 1:2 * H:2], op=AT.add)
                                adif = sb1.tile([128, H], fp32, tag="adif")
                                nc.vector.tensor_tensor(
                                    out=adif[:], in0=aug8[:, 0:2 * H:2],
                                    in1=aug8[:, 1:2 * H:2], op=AT.subtract)
                                nc.vector.scalar_tensor_tensor(
                                    out=esc_all[:, t2 * H:(t2 + 1) * H], in0=adif[:],
                                    scalar=float(AUGDIV * NEG_SLOPE / (1.0 - NEG_SLOPE)),
                                    in1=rdif[:], op0=AT.mult, op1=AT.add)
                            # one Exp per dst block (avoids per-tile act-table swaps)
                            nc.scalar.activation(
                                out=a_all[:, db * T_BLK * H:(db + 1) * T_BLK * H],
                                in_=esc_all[:], func=AF.Exp,
                                scale=float(S * (1.0 - NEG_SLOPE)))
                            # pass A: denominators
                            pden = ps2.tile([128, H], fp32, tag="psmall", bufs=2, name=f"pden_{db}")
                            for t2 in range(T_BLK):
                                t = db * T_BLK + t2
                                ee = sbe.tile([128, 128], fp32, tag="esel_et", name=f"eet_{db}_{t2}")
                                nc.vector.scalar_tensor_tensor(
                                    out=ee[:], in0=dstcp[:, t:t + 1].to_broadcast([128, 128]),
                                    scalar=float(-db * 128), in1=iotaF[:],
                                    op0=AT.add, op1=AT.is_equal)
                                esels.append(ee)
                                nc.tensor.matmul(
                                    pden[:], ee[:], a_all[:, t * H:(t + 1) * H],
                                    start=(t2 == 0), stop=(t2 == T_BLK - 1))
                            dtmp = sb2.tile([128, H], fp32, tag="dtmp")
                            nc.vector.tensor_scalar_add(out=dtmp[:], in0=pden[:], scalar1=1e-16)
                            nc.vector.reciprocal(out=denr[:, db * H:(db + 1) * H], in_=dtmp[:])
                            # pass B: alpha and selection weights
                            for t2 in range(T_BLK):
                                t = db * T_BLK + t2
                                esde = sb2.tile([128, 128], fp32, tag="esde", bufs=4)
                                nc.vector.scalar_tensor_tensor(
                                    out=esde[:], in0=drows[t2],
                                    scalar=float(-db * 128), in1=iotaP[:],
                                    op0=AT.add, op1=AT.is_equal)
                                pad = ps2.tile([128, H], fp32, tag="psmall", bufs=2, name=f"pad_{db}_{t2}")
                                nc.tensor.matmul(pad[:], esde[:], denr[:, db * H:(db + 1) * H],
                                                 start=True, stop=True)
                                alpha = sb2.tile([128, H], fp32, tag="alpha")
                                nc.vector.tensor_tensor(out=alpha[:], in0=a_all[:, t * H:(t + 1) * H],
                                                        in1=pad[:], op=AT.mult)
                                hsal = []
                                for h in range(H):
                                    sal = sb2.tile([128, 128], bf16, tag=f"sal{h}",
                                                   bufs=2 * T_BLK + 1, name=f"sal{h}_{db}_{t2}")
                                    nc.vector.tensor_tensor(
                                        out=sal[:], in0=esels[t2][:],
                                        in1=alpha[:, h:h + 1].to_broadcast([128, 128]),
                                        op=AT.mult)
                                    hsal.append(sal)
                                sals_all.append(hsal)
                            # transposed aggregation (xle-based: out = sum alpha*xl) + fc
                            NGRP = 4            # cc chunks per PSUM group (1 bank)
                            oaggs = []
                            for g in range(HC // 128 // NGRP):
                                pagg = ps2.tile([128, NGRP * 128], fp32, tag="pagg",
                                                bufs=1, name=f"pagg_{db}_{g}")
                                oagg = sb2.tile([128, NGRP * 128], bf16, tag="oagg", bufs=6,
                                                name=f"oagg_{db}_{g}")
                                for j in range(NGRP):
                                    cc = g * NGRP + j
                                    h = cc // (HC // 128 // H)
                                    for t2 in range(T_BLK):
                                        nc.tensor.matmul(
                                            pagg[:, j * 128:(j + 1) * 128],
                                            xles[t2][:, cc * 128:(cc + 1) * 128],
                                            sals_all[t2][h][:],
                                            start=(t2 == 0), stop=(t2 == T_BLK - 1))
                                if g % 2 == 0:
                                    nc.scalar.activation(out=oagg[:], in_=pagg[:], func=AF.Copy)
                                else:
                                    nc.vector.tensor_copy(out=oagg[:], in_=pagg[:])
                                oaggs.append(oagg)
                            pfc = ps2.tile([128, NCLS_P], fp32, tag="psmall", bufs=2, name=f"pfc_{db}")
                            for cc in range(HC // 128):
                                nc.tensor.matmul(
                                    pfc[:], oaggs[cc // NGRP][:, (cc % NGRP) * 128:(cc % NGRP + 1) * 128],
                                    wfs_r[:, cc, :],
                                    start=(cc == 0), stop=False)
                            nc.tensor.matmul(pfc[:], ones1[:], bf2[:], start=False, stop=True)
                            # softmax
                            negmax = sb2.tile([128, 1], fp32, tag="negmax")
                            nc.vector.tensor_reduce(out=negmax[:], in_=pfc[:],
                                                    axis=mybir.AxisListType.X,
                                                    op=AT.max, negate=True)
                            pexp = sb2.tile([128, NCLS_P], fp32, tag="pexp", bufs=2)
                            nc.scalar.activation(out=pexp[:], in_=pfc[:], func=AF.Exp,
                                                 bias=negmax[:, 0:1], scale=1.0)
                            ssum = sb2.tile([128, 1], fp32, tag="ssum")
                            nc.vector.tensor_reduce(out=ssum[:], in_=pexp[:],
                                                    axis=mybir.AxisListType.X, op=AT.add)
                            rs = sb2.tile([128, 1], fp32, tag="rs")
                            nc.vector.reciprocal(out=rs[:], in_=ssum[:])
                            hout = sb2.tile([128, NCLS_P], fp32, tag="hout", bufs=2)
                            nc.vector.scalar_tensor_tensor(
                                out=hout[:], in0=pexp[:], scalar=rs[:, 0:1], in1=pexp[:],
                                op0=AT.mult, op1=AT.bypass)
                            nc.sync.dma_start(out=d_out[db * 128:(db + 1) * 128, :], in_=hout[:])
                        if debug:
                            nc.sync.dma_start(out=d_dbga[:], in_=a_all[:])
                            nc.sync.dma_start(out=d_dbgx[:], in_=xrdb8[:])

    nc.compile()
    return nc


def kernel(**inputs):
    out_full = np.zeros((N, NCLS), np.float32)
    in_maps, dims = _prep(
        inputs["x"], inputs["edge_index"], inputs["Wl"], inputs["bl"],
        inputs["Wr"], inputs["br"], inputs["att"], inputs["bias"],
        inputs["Wf"], inputs["bf"])
    nc = _build(dims)
    from concourse.bass_utils import run_bass_kernel_spmd
    res = run_bass_kernel_spmd(nc, in_maps, core_ids=list(range(NCORES)))
    for k in range(NCORES):
        out_full[k * ND:(k + 1) * ND, :] = res.results[k]["out"][:ND, :NCLS]
    return out_full


# revision 23
# speedup vs baseline: 6.3381x; 1.0029x over previous
"""GATv2 + softmax head for nn_GATModel_Softmax_4535485465120 on 8 trn2 NeuronCores.

v2: fp8-DoubleRow projections + xle-based aggregation (no XR-transpose phase).

Strategy (graph/data parallel by dst node, fully local — no collectives):
  - Nodes partitioned into 8 ranges of 1000 dst nodes (one per core).
  - Host preps per core: x.T columns (fp8) for the core's unique src nodes and
    its dst nodes; weights with att-magnitude (clamped at max/64 per head)
    folded into Wl/Wr columns, permuted pos-att-first, scaled 1/S for fp8;
    Wf rows carry the inverse permutation, S/(3*att_eff) un-scaling and the
    head-mean.
  - Device pipeline per core:
      phase 0:  XR'' = xdst8 @ Wr8 via fp8 DoubleRow matmuls -> SBUF fp8
                (resident, pair-packed by dst block for DR reuse)
      phase 0.5: XLu'' = xu8 @ Wl8 (DR) -> HBM bf16 [UP, 3072]
      phase 1 (per dst block, per 128-edge tile): U/S accumulated directly in
                PSUM = DR-selection-matmul of XR'' + identity-matmul of
                gathered XLu'' rows; leaky-relu + sign-segmented accumulation
                on DVE reads PSUM directly; a = exp(S * esc) on ScalarE.
      phase 2:  denom/alpha via selection matmuls; agg^T = sum_e alpha *
                xle (selection matmul, bf16); logits = agg^T-chunks @ Wf_stack
                (+folded bias row); row softmax -> output.

kernel(**inputs) takes FULL inputs, shards internally, returns FULL [8000,460] f32.
"""

import numpy as np
import ml_dtypes

BF16 = ml_dtypes.bfloat16
E4M3 = ml_dtypes.float8_e4m3   # IEEE e4m3: max 240, has inf (matches mybir float8e4)
F8MAX = 240.0

# Problem constants (hardcoded per spec)
N = 8000
DIN = 1024
H = 3
C = 1024
HC = H * C          # 3072
NCLS = 460
NCLS_P = 512
NEG_SLOPE = 0.2
NCORES = 8
ND = N // NCORES    # 1000 dst nodes per core
NDP = 1024          # padded dst count per core
DB = NDP // 128     # 8 dst blocks per core
P = 128
KC = DIN // P       # 8 contraction chunks (4 DoubleRow pairs)
NB = 3              # n-chunks of 1024 in HC (one per head)
HF = 2              # 512-wide matmul halves per 1024 chunk
NAUG = 8            # aug columns: per-head pos/neg sums (6) + pad (2)
HCA = HC + NAUG     # 3080
AUGDIV = 64.0       # aug columns scaled by 1/64 to stay in fp8 range


def _prep(x, edge_index, Wl, bl, Wr, br, att, bias, Wf, bf):
    """Host-side preprocessing -> per-core input maps + static dims."""
    x = np.asarray(x, np.float32)
    ei = np.asarray(edge_index).astype(np.int64)
    Wl = np.asarray(Wl, np.float32)
    Wr = np.asarray(Wr, np.float32)
    bl = np.asarray(bl, np.float32)
    br = np.asarray(br, np.float32)
    att = np.asarray(att, np.float32)
    bias = np.asarray(bias, np.float32)
    Wf = np.asarray(Wf, np.float32)
    bf = np.asarray(bf, np.float32)

    assert np.all(bl == 0) and np.all(br == 0), \
        "nonzero bl/br not supported by this kernel build"

    loops = np.arange(N, dtype=np.int64)
    src_all = np.concatenate([ei[:, 0], loops])
    dst_all = np.concatenate([ei[:, 1], loops])

    # att folding: per head, column scale att_eff (clamped so fp8 columns
    # stay out of the subnormal range) and permutation pos-first
    absatt = np.abs(att)                       # [H, C]
    att_eff = np.maximum(absatt, absatt.max(1, keepdims=True) / 64.0)
    perm = np.zeros((H, C), np.int64)          # perm[h, newc] = origc
    npos = np.zeros(H, np.int64)
    for h in range(H):
        pos = np.nonzero(att[h] > 0)[0]
        neg = np.nonzero(att[h] <= 0)[0]
        perm[h] = np.concatenate([pos, neg])
        npos[h] = len(pos)

    # scaled/permuted projection weights  [DIN, HC]
    Wl_s = np.zeros((DIN, HC), np.float32)
    Wr_s = np.zeros((DIN, HC), np.float32)
    for h in range(H):
        sc = att_eff[h, perm[h]]               # [C]
        Wl_s[:, h * C:(h + 1) * C] = Wl[:, h * C + perm[h]] * sc
        Wr_s[:, h * C:(h + 1) * C] = Wr[:, h * C + perm[h]] * sc

    # global fp8 scale S: covers weight absmax and a 6-sigma bound on the
    # projected activations (xr'' must fit fp8 storage after the matmul)
    colnorm = max(np.linalg.norm(Wl_s, axis=0).max(),
                  np.linalg.norm(Wr_s, axis=0).max())
    S = float(max(np.abs(Wl_s).max() / F8MAX, np.abs(Wr_s).max() / F8MAX,
                  colnorm * 7.0 / F8MAX))

    def with_aug(W):
        W8 = np.clip(W / S, -F8MAX, F8MAX).astype(E4M3)
        Wa = np.zeros((DIN, HCA), E4M3)
        Wa[:, :HC] = W8
        W8f = W8.astype(np.float32)
        for h in range(H):
            np_h = int(npos[h])
            Wa[:, HC + 2 * h] = (W8f[:, h * C:h * C + np_h].sum(1) / AUGDIV).astype(E4M3)
            Wa[:, HC + 2 * h + 1] = (W8f[:, h * C + np_h:(h + 1) * C].sum(1) / AUGDIV).astype(E4M3)
        return Wa
    wl8 = with_aug(Wl_s)
    wr8 = with_aug(Wr_s)

    # final fc stack: logits = sum_h (agg''_h * S/(3*att_eff)) @ Wf  (+ bias@Wf + bf)
    Wfs = np.zeros((HC, NCLS_P), np.float32)
    for h in range(H):
        sc = S / (3.0 * att_eff[h, perm[h]])
        Wfs[h * C:(h + 1) * C, :NCLS] = Wf[perm[h]] * sc[:, None]
    bf2 = np.full((1, NCLS_P), -1e30, np.float32)
    bf2[0, :NCLS] = bias @ Wf + bf

    xT8 = np.clip(np.ascontiguousarray(x.T), -F8MAX, F8MAX).astype(E4M3)     # [DIN, N]

    # per-core edge grouping: edges (incl. self loops) by dst block
    cores = []
    tmax = 1
    for k in range(NCORES):
        lo, hi = k * ND, (k + 1) * ND
        m = (dst_all >= lo) & (dst_all < hi)
        s_k = src_all[m]
        dl_k = (dst_all[m] - lo).astype(np.int64)
        order = np.argsort(dl_k, kind="stable")
        s_k, dl_k = s_k[order], dl_k[order]
        blocks = []
        for db in range(DB):
            bm = (dl_k >= db * 128) & (dl_k < (db + 1) * 128)
            blocks.append((s_k[bm], dl_k[bm]))
            tmax = max(tmax, (len(blocks[-1][0]) + 127) // 128)
        cores.append(blocks)

    T_BLK = tmax
    E1T = DB * T_BLK
    E1P = E1T * 128

    iotaF = np.tile(np.arange(128, dtype=np.float32)[None, :], (128, 1))
    iotaP = np.tile(np.arange(128, dtype=np.float32)[:, None], (1, 128))
    ones1 = np.ones((1, 128), BF16)
    # unique srcs per core -> common padded tile count
    uniq = []
    for k in range(NCORES):
        srcs = np.concatenate([cores[k][db][0] for db in range(DB)])
        uniq.append(np.unique(srcs))
    UT = max((len(u) + 127) // 128 for u in uniq)
    UP = UT * 128
    in_maps = []
    for k in range(NCORES):
        srcslot = np.zeros(E1P, np.int64)
        real = np.zeros(E1P, bool)
        dstloc = np.full(E1P, -1.0, np.float32)
        for db in range(DB):
            s_k, dl_k = cores[k][db]
            base = db * T_BLK * 128
            srcslot[base:base + len(s_k)] = s_k
            real[base:base + len(s_k)] = True
            dstloc[base:base + len(s_k)] = dl_k.astype(np.float32)
        u = uniq[k]
        xuT = np.zeros((DIN, UP), E4M3)
        xuT[:, :len(u)] = xT8[:, u]
        srcloc = np.zeros((E1P, 1), np.int32)
        srcloc[real, 0] = np.searchsorted(u, srcslot[real]).astype(np.int32)
        xdstT = np.zeros((DIN, NDP), E4M3)
        xdstT[:, :ND] = xT8[:, k * ND:(k + 1) * ND]
        dst_col = np.ascontiguousarray(dstloc.reshape(E1T, 128).T)   # [128, E1T]
        src_col = np.ascontiguousarray(srcloc.reshape(E1T, 128).T)   # [128, E1T] int32
        dst_row = np.tile(dstloc[None, :], (128, 1))                 # [128, E1P]
        in_maps.append({
            "xuT": xuT,
            "srcloc": srcloc,
            "srccp": src_col,
            "xdstT": xdstT,
            "wl": wl8,
            "wr": wr8,
            "wfs": Wfs.astype(BF16),
            "bf2": bf2.astype(BF16),
            "dstcp": dst_col,
            "dstrow": dst_row,
            "iotaF": iotaF,
            "iotaP": iotaP,
            "ones1": ones1,
        })
    dims = {"T_BLK": T_BLK, "E1T": E1T, "E1P": E1P, "UT": UT,
            "npos": [int(v) for v in npos], "S": S}
    return in_maps, dims


def _build(dims, debug=False):
    """Trace the Bass/Tile program (identical for all cores)."""
    import concourse.bass as bass
    import concourse.mybir as mybir
    import concourse.tile as tile
    from concourse import bacc

    T_BLK, E1T, E1P = dims["T_BLK"], dims["E1T"], dims["E1P"]
    UT = dims["UT"]
    npos = dims["npos"]
    S = dims["S"]
    UP = UT * 128
    fp32 = mybir.dt.float32
    bf16 = mybir.dt.bfloat16
    fp8 = mybir.dt.float8e4
    AT = mybir.AluOpType
    AF = mybir.ActivationFunctionType
    DRM = mybir.MatmulPerfMode.DoubleRow

    nc = bacc.Bacc("TRN2", target_bir_lowering=False, debug=False)

    with tile.TileContext(nc) as tc:
        with tc.tile_pool(name="dram", bufs=1, space="DRAM") as dram:
            d_xuT = dram.tile([DIN, UP], fp8, kind="ExternalInput", name="xuT", uniquify=False)
            d_srcloc = dram.tile([E1P, 1], mybir.dt.int32, kind="ExternalInput", name="srcloc", uniquify=False)
            d_srccp = dram.tile([128, E1T], mybir.dt.int32, kind="ExternalInput", name="srccp", uniquify=False)
            d_xdstT = dram.tile([DIN, NDP], fp8, kind="ExternalInput", name="xdstT", uniquify=False)
            d_wl = dram.tile([DIN, HCA], fp8, kind="ExternalInput", name="wl", uniquify=False)
            d_wr = dram.tile([DIN, HCA], fp8, kind="ExternalInput", name="wr", uniquify=False)
            d_wfs = dram.tile([HC, NCLS_P], bf16, kind="ExternalInput", name="wfs", uniquify=False)
            d_bf2 = dram.tile([1, NCLS_P], bf16, kind="ExternalInput", name="bf2", uniquify=False)
            d_dstcp = dram.tile([128, E1T], fp32, kind="ExternalInput", name="dstcp", uniquify=False)
            d_dstrow = dram.tile([128, E1P], fp32, kind="ExternalInput", name="dstrow", uniquify=False)
            d_iotaF = dram.tile([128, 128], fp32, kind="ExternalInput", name="iotaF", uniquify=False)
            d_iotaP = dram.tile([128, 128], fp32, kind="ExternalInput", name="iotaP", uniquify=False)
            d_ones1 = dram.tile([1, 128], bf16, kind="ExternalInput", name="ones1", uniquify=False)
            d_out = dram.tile([NDP, NCLS_P], fp32, kind="ExternalOutput", name="out", uniquify=False)
            d_xlu = dram.tile([UP, HCA], bf16, name="xlu_i",
                              kind="ExternalOutput" if debug else "Internal",
                              uniquify=False)
            if debug:
                d_dbga = dram.tile([128, E1T * H], fp32, kind="ExternalOutput", name="dbg_a", uniquify=False)
                d_dbgx = dram.tile([128, DB, HCA], fp8, kind="ExternalOutput", name="dbg_xrd", uniquify=False)

            with tc.tile_pool(name="gsb", bufs=1) as gsb:
                # resident tensors
                dstcp = gsb.tile([128, E1T], fp32, name="dstcp_r")
                nc.sync.dma_start(out=dstcp[:], in_=d_dstcp[:])
                srccp = gsb.tile([128, E1T], mybir.dt.int32, name="srccp_r")
                nc.sync.dma_start(out=srccp[:], in_=d_srccp[:])
                iotaF = gsb.tile([128, 128], fp32, name="iotaF_r")
                nc.sync.dma_start(out=iotaF[:], in_=d_iotaF[:])
                iotaP = gsb.tile([128, 128], fp32, name="iotaP_r")
                nc.sync.dma_start(out=iotaP[:], in_=d_iotaP[:])
                ones1 = gsb.tile([1, 128], bf16, name="ones1_r")
                nc.sync.dma_start(out=ones1[:], in_=d_ones1[:])
                bf2 = gsb.tile([1, NCLS_P], bf16, name="bf2_r")
                nc.sync.dma_start(out=bf2[:], in_=d_bf2[:])
                a_all = gsb.tile([128, E1T * H], fp32, name="a_all_r")
                denr = gsb.tile([128, DB * H], fp32, name="denr_r")
                xrdb8 = gsb.tile([128, DB, HCA], fp8, name="xrdb8_r")
                idm = gsb.tile([128, 128], bf16, name="idm_r")
                nc.vector.scalar_tensor_tensor(
                    out=idm[:], in0=iotaP[:], scalar=0.0, in1=iotaF[:],
                    op0=AT.add, op1=AT.is_equal)

                # ---------------- phase 0: XR'' projection (fp8 DR) ----------------
                with tc.tile_pool(name="p01", bufs=1, space="PSUM") as ps1:
                    with tc.tile_pool(name="wpool", bufs=1) as wpool:
                        wmat = wpool.tile([128, KC, HCA], fp8, tag="wmat", bufs=1, name="wmat_r")
                        xdst8 = wpool.tile([128, KC, NDP], fp8, tag="xdst8", bufs=1, name="xdst8_r")
                        for kp in range(KC // 2):
                            nc.sync.dma_start(
                                out=xdst8[:, 2 * kp:2 * kp + 2, :],
                                in_=d_xdstT[kp * 256:(kp + 1) * 256, :]
                                    .rearrange("(kc p) n -> p kc n", p=128))
                            nc.sync.dma_start(
                                out=wmat[:, 2 * kp:2 * kp + 2, :],
                                in_=d_wr[kp * 256:(kp + 1) * 256, :]
                                    .rearrange("(kc p) n -> p kc n", p=128))
                        # phase 0 + 0.5 interleaved: XR'' block and XLu'' tiles
                        # alternate so PE fills each other's copy/DMA stalls
                        wmat2 = wpool.tile([128, KC, HCA], fp8, tag="wmat2", bufs=1, name="wmat_r2")
                        xu8 = wpool.tile([128, KC, UP], fp8, tag="xu8", bufs=1, name="xu8_r")
                        for kp in range(KC // 2):
                            nc.sync.dma_start(
                                out=wmat2[:, 2 * kp:2 * kp + 2, :],
                                in_=d_wl[kp * 256:(kp + 1) * 256, :]
                                    .rearrange("(kc p) n -> p kc n", p=128))
                            nc.sync.dma_start(
                                out=xu8[:, 2 * kp:2 * kp + 2, :],
                                in_=d_xuT[kp * 256:(kp + 1) * 256, :]
                                    .rearrange("(kc p) n -> p kc n", p=128))

                        def emit_xr(dc):
                            for nb in range(NB):
                                for hf in range(HF):
                                    ppu = ps1.tile([128, 512], fp32, tag="ppu",
                                                   bufs=4, name=f"pp{nb}_{hf}_c{dc}")
                                    for kp in range(KC // 2):
                                        nc.tensor.matmul(
                                            ppu[:],
                                            xdst8[:, 2 * kp:2 * kp + 2, dc * 128:(dc + 1) * 128],
                                            wmat[:, 2 * kp:2 * kp + 2, nb * C + hf * 512:nb * C + (hf + 1) * 512],
                                            start=(kp == 0), stop=(kp == KC // 2 - 1),
                                            perf_mode=DRM)
                                    nc.vector.tensor_copy(
                                        out=xrdb8[:, dc, nb * C + hf * 512:nb * C + (hf + 1) * 512],
                                        in_=ppu[:])
                            ppa = ps1.tile([128, NAUG], fp32, tag="ppaug", bufs=1, name=f"ppa_c{dc}")
                            for kp in range(KC // 2):
                                nc.tensor.matmul(
                                    ppa[:], xdst8[:, 2 * kp:2 * kp + 2, dc * 128:(dc + 1) * 128],
                                    wmat[:, 2 * kp:2 * kp + 2, HC:HCA],
                                    start=(kp == 0), stop=(kp == KC // 2 - 1),
                                    perf_mode=DRM)
                            nc.vector.tensor_copy(out=xrdb8[:, dc, HC:HCA], in_=ppa[:])

                        with tc.tile_pool(name="sb0", bufs=2) as sb0:
                            def emit_xlu(ut):
                                xl_sb = sb0.tile([128, HCA], bf16, tag="xl_sb", bufs=3)
                                for nb in range(NB):
                                    for hf in range(HF):
                                        ppu = ps1.tile([128, 512], fp32, tag="ppu",
                                                       bufs=4, name=f"ppu{nb}_{hf}_{ut}")
                                        for kp in range(KC // 2):
                                            nc.tensor.matmul(
                                                ppu[:],
                                                xu8[:, 2 * kp:2 * kp + 2, ut * 128:(ut + 1) * 128],
                                                wmat2[:, 2 * kp:2 * kp + 2, nb * C + hf * 512:nb * C + (hf + 1) * 512],
                                                start=(kp == 0), stop=(kp == KC // 2 - 1),
                                                perf_mode=DRM)
                                        cl = nb * C + hf * 512
                                        if nb == 0:
                                            nc.vector.tensor_copy(out=xl_sb[:, cl:cl + 512], in_=ppu[:])
                                        else:
                                            nc.scalar.activation(out=xl_sb[:, cl:cl + 512],
                                                                 in_=ppu[:], func=AF.Copy)
                                ppa = ps1.tile([128, NAUG], fp32, tag="ppaug", bufs=1, name=f"ppa_u{ut}")
                                for kp in range(KC // 2):
                                    nc.tensor.matmul(
                                        ppa[:], xu8[:, 2 * kp:2 * kp + 2, ut * 128:(ut + 1) * 128],
                                        wmat2[:, 2 * kp:2 * kp + 2, HC:HCA],
                                        start=(kp == 0), stop=(kp == KC // 2 - 1),
                                        perf_mode=DRM)
                                nc.scalar.activation(out=xl_sb[:, HC:HCA], in_=ppa[:], func=AF.Copy)
                                nc.sync.dma_start(out=d_xlu[ut * 128:(ut + 1) * 128, :], in_=xl_sb[:])

                            for dc in range(DB):
                                emit_xr(dc)
                            for ut_i in range(UT):
                                emit_xlu(ut_i)

                    # wfs resident (phase-0 weight slots are free now)
                    wfs_r = gsb.tile([128, HC // 128, NCLS_P], bf16, name="wfs_r")
                    nc.sync.dma_start(out=wfs_r[:], in_=d_wfs[:].rearrange("(cc p) n -> p cc n", p=128))

                    # ------- fused phases 1+2: per dst block -------
                    with (
                        tc.tile_pool(name="sb1", bufs=3) as sb1,
                        tc.tile_pool(name="sbe", bufs=T_BLK + 4) as sbe,
                        tc.tile_pool(name="p2", bufs=1, space="PSUM") as ps2,
                        tc.tile_pool(name="sb2", bufs=3) as sb2,
                    ):
                        for db in range(DB):
                            pb = db // 2
                            jact = db % 2
                            xles, esels, sals_all, drows = [], [], [], []
                            esc_all = sb1.tile([128, T_BLK * H], fp32, tag="esc_all",
                                               bufs=2, name=f"escall_{db}")
                            drow_db = sbe.tile([128, T_BLK * 128], fp32, tag="drow",
                                               bufs=2, name=f"drow_{db}")
                            nc.sync.dma_start(
                                out=drow_db[:],
                                in_=d_dstrow[:, db * T_BLK * 128:(db + 1) * T_BLK * 128])
                            for t2 in range(T_BLK):
                                t = db * T_BLK + t2
                                drow = drow_db[:, t2 * 128:(t2 + 1) * 128]
                                drows.append(drow)
                                # gather XL''[src] rows (bf16)
                                xle = sbe.tile([128, HCA], bf16, tag="xle", bufs=2 * T_BLK + 1, name=f"xle_{db}_{t2}")
                                nc.gpsimd.indirect_dma_start(
                                    out=xle[:], out_offset=None, in_=d_xlu[:],
                                    in_offset=bass.IndirectOffsetOnAxis(ap=srccp[:, t:t + 1], axis=0))
                                xles.append(xle)
                                # pair-packed 0/1 dst-selection for DoubleRow U-expansion
                                eselw = sb1.tile([128, 2, 128], fp8, tag="eselw")
                                nc.gpsimd.memset(eselw[:, 1 - jact, :], 0.0)
                                nc.vector.scalar_tensor_tensor(
                                    out=eselw[:, jact, :], in0=drow,
                                    scalar=float(-db * 128), in1=iotaP[:],
                                    op0=AT.add, op1=AT.is_equal)
                                eacc = sb1.tile([128, 4 * H], fp32, tag="eacc")
                                scrD = sb1.tile([128, C], fp32, tag="scrD", bufs=2)
                                scrA = sb1.tile([128, C], fp32, tag="scrA", bufs=2)
                                # aug sums first: frees the single-buffered aug bank
                                # early and overlaps the aug copy with head scores
                                ppa = ps1.tile([128, NAUG], fp32, tag="ppaug", bufs=1, name=f"ppa_t{t}")
                                nc.tensor.matmul(
                                    ppa[:], eselw[:], xrdb8[:, 2 * pb:2 * pb + 2, HC:HCA],
                                    start=True, stop=False, perf_mode=DRM)
                                nc.tensor.matmul(
                                    ppa[:], idm[:], xle[:, HC:HCA],
                                    start=False, stop=True)
                                aug8 = sb1.tile([128, NAUG], fp32, tag="aug8")
                                nc.scalar.activation(out=aug8[:], in_=ppa[:], func=AF.Copy)
                                for h in range(H):
                                    use_act = (h == 1) or (h == 2 and t2 % 2 == 0)
                                    np_h = npos[h]
                                    for hf in range(HF):
                                        # U/S for head h, 512-col half, in a 1-bank
                                        # PSUM group (4 in flight)
                                        ppu = ps1.tile([128, 512], fp32, tag="ppu",
                                                       bufs=4, name=f"ppe{h}_{hf}_t{t}")
                                        cl = h * C + hf * 512
                                        nc.tensor.matmul(
                                            ppu[:], eselw[:],
                                            xrdb8[:, 2 * pb:2 * pb + 2, cl:cl + 512],
                                            start=True, stop=False, perf_mode=DRM)
                                        nc.tensor.matmul(
                                            ppu[:], idm[:], xle[:, cl:cl + 512],
                                            start=False, stop=True)
                                        # scores: pos/neg relu accumulation off PSUM
                                        # (leaky = 0.8*relu(u)+0.2*u; sum(u) rides aug cols)
                                        lo, hi = hf * 512, hf * 512 + 512
                                        segs = []
                                        if np_h > lo:
                                            segs.append((lo, min(np_h, hi), 4 * h + 2 * hf))
                                        if np_h < hi:
                                            segs.append((max(np_h, lo), hi, 4 * h + 2 * hf + 1))
                                        done = set()
                                        for s0, s1, j in segs:
                                            ln = s1 - s0
                                            done.add(j)
                                            if use_act:
                                                nc.scalar.activation(
                                                    out=scrA[:, :ln], in_=ppu[:, s0 - lo:s1 - lo],
                                                    func=AF.Relu, accum_out=eacc[:, j:j + 1])
                                            else:
                                                nc.vector.tensor_scalar(
                                                    out=scrD[:, :ln], in0=ppu[:, s0 - lo:s1 - lo],
                                                    scalar1=0.0, scalar2=0.0, op0=AT.max,
                                                    op1=AT.add, accum_out=eacc[:, j:j + 1])
                                        for j in (4 * h + 2 * hf, 4 * h + 2 * hf + 1):
                                            if j not in done:
                                                nc.vector.memset(eacc[:, j:j + 1], 0.0)
                                rdif = sb1.tile([128, H], fp32, tag="rdif")
                                rtmp = sb1.tile([128, 2 * H], fp32, tag="rtmp")
                                nc.gpsimd.tensor_tensor(
                                    out=rtmp[:], in0=eacc[:, 0:4 * H:2],
                                    in1=eacc[:, 1:4 * H:2], op=AT.subtract)
                                nc.gpsimd.tensor_tensor(
                                    out=rdif[:], in0=rtmp[:, 0:2 * H:2],
                                    in1=rtmp[:, 1:2 * H:2], op=AT.add)
                                adif = sb1.tile([128, H], fp32, tag="adif")
                                nc.gpsimd.tensor_tensor(
                                    out=adif[:], in0=aug8[:, 0:2 * H:2],
                                    in1=aug8[:, 1:2 * H:2], op=AT.subtract)
                                nc.vector.scalar_tensor_tensor(
                                    out=esc_all[:, t2 * H:(t2 + 1) * H], in0=adif[:],
                                    scalar=float(AUGDIV * NEG_SLOPE / (1.0 - NEG_SLOPE)),
                                    in1=rdif[:], op0=AT.mult, op1=AT.add)
                            # one Exp per dst block (avoids per-tile act-table swaps)
                            nc.scalar.activation(
                                out=a_all[:, db * T_BLK * H:(db + 1) * T_BLK * H],
                                in_=esc_all[:], func=AF.Exp,
                                scale=float(S * (1.0 - NEG_SLOPE)))
                            # pass A: denominators
                            pden = ps2.tile([128, H], fp32, tag="psmall", bufs=2, name=f"pden_{db}")
                            for t2 in range(T_BLK):
                                t = db * T_BLK + t2
                                ee = sbe.tile([128, 128], fp32, tag="esel_et", name=f"eet_{db}_{t2}")
                                nc.vector.scalar_tensor_tensor(
                                    out=ee[:], in0=dstcp[:, t:t + 1].to_broadcast([128, 128]),
                                    scalar=float(-db * 128), in1=iotaF[:],
                                    op0=AT.add, op1=AT.is_equal)
                                esels.append(ee)
                                nc.tensor.matmul(
                                    pden[:], ee[:], a_all[:, t * H:(t + 1) * H],
                                    start=(t2 == 0), stop=(t2 == T_BLK - 1))
                            dtmp = sb2.tile([128, H], fp32, tag="dtmp")
                            nc.vector.tensor_scalar_add(out=dtmp[:], in0=pden[:], scalar1=1e-16)
                            nc.vector.reciprocal(out=denr[:, db * H:(db + 1) * H], in_=dtmp[:])
                            # pass B: alpha and selection weights
                            for t2 in range(T_BLK):
                                t = db * T_BLK + t2
                                esde = sb2.tile([128, 128], fp32, tag="esde", bufs=4)
                                nc.vector.scalar_tensor_tensor(
                                    out=esde[:], in0=drows[t2],
                                    scalar=float(-db * 128), in1=iotaP[:],
                                    op0=AT.add, op1=AT.is_equal)
                                pad = ps2.tile([128, H], fp32, tag="psmall", bufs=2, name=f"pad_{db}_{t2}")
                                nc.tensor.matmul(pad[:], esde[:], denr[:, db * H:(db + 1) * H],
                                                 start=True, stop=True)
                                alpha = sb2.tile([128, H], fp32, tag="alpha")
                                nc.vector.tensor_tensor(out=alpha[:], in0=a_all[:, t * H:(t + 1) * H],
                                                        in1=pad[:], op=AT.mult)
                                hsal = []
                                for h in range(H):
                                    sal = sb2.tile([128, 128], bf16, tag=f"sal{h}",
                                                   bufs=2 * T_BLK + 1, name=f"sal{h}_{db}_{t2}")
                                    nc.vector.tensor_tensor(
                                        out=sal[:], in0=esels[t2][:],
                                        in1=alpha[:, h:h + 1].to_broadcast([128, 128]),
                                        op=AT.mult)
                                    hsal.append(sal)
                                sals_all.append(hsal)
                            # transposed aggregation (xle-based: out = sum alpha*xl) + fc
                            NGRP = 4            # cc chunks per PSUM group (1 bank)
                            oaggs = []
                            for g in range(HC // 128 // NGRP):
                                pagg = ps2.tile([128, NGRP * 128], fp32, tag="pagg",
                                                bufs=1, name=f"pagg_{db}_{g}")
                                oagg = sb2.tile([128, NGRP * 128], bf16, tag="oagg", bufs=6,
                                                name=f"oagg_{db}_{g}")
                                for j in range(NGRP):
                                    cc = g * NGRP + j
                                    h = cc // (HC // 128 // H)
                                    for t2 in range(T_BLK):
                                        nc.tensor.matmul(
                                            pagg[:, j * 128:(j + 1) * 128],
                                            xles[t2][:, cc * 128:(cc + 1) * 128],
                                            sals_all[t2][h][:],
                                            start=(t2 == 0), stop=(t2 == T_BLK - 1))
                                if g % 2 == 0:
                                    nc.scalar.activation(out=oagg[:], in_=pagg[:], func=AF.Copy)
                                else:
                                    nc.vector.tensor_copy(out=oagg[:], in_=pagg[:])
                                oaggs.append(oagg)
                            pfc = ps2.tile([128, NCLS_P], fp32, tag="psmall", bufs=2, name=f"pfc_{db}")
                            for cc in range(HC // 128):
                                nc.tensor.matmul(
                                    pfc[:], oaggs[cc // NGRP][:, (cc % NGRP) * 128:(cc % NGRP + 1) * 128],
                                    wfs_r[:, cc, :],
                                    start=(cc == 0), stop=False)
                            nc.tensor.matmul(pfc[:], ones1[:], bf2[:], start=False, stop=True)
                            # softmax
                            negmax = sb2.tile([128, 1], fp32, tag="negmax")
                            nc.vector.tensor_reduce(out=negmax[:], in_=pfc[:],
                                                    axis=mybir.AxisListType.X,
                                                    op=AT.max, negate=True)
                            pexp = sb2.tile([128, NCLS_P], fp32, tag="pexp", bufs=2)
                            nc.scalar.activation(out=pexp[:], in_=pfc[:], func=AF.Exp,
                                                 bias=negmax[:, 0:1], scale=1.0)
                            ssum = sb2.tile([128, 1], fp32, tag="ssum")
                            nc.vector.tensor_reduce(out=ssum[:], in_=pexp[:],
                                                    axis=mybir.AxisListType.X, op=AT.add)
                            rs = sb2.tile([128, 1], fp32, tag="rs")
                            nc.vector.reciprocal(out=rs[:], in_=ssum[:])
                            hout = sb2.tile([128, NCLS_P], fp32, tag="hout", bufs=2)
                            nc.vector.scalar_tensor_tensor(
                                out=hout[:], in0=pexp[:], scalar=rs[:, 0:1], in1=pexp[:],
                                op0=AT.mult, op1=AT.bypass)
                            nc.sync.dma_start(out=d_out[db * 128:(db + 1) * 128, :], in_=hout[:])
                        if debug:
                            nc.sync.dma_start(out=d_dbga[:], in_=a_all[:])
                            nc.sync.dma_start(out=d_dbgx[:], in_=xrdb8[:])

    nc.compile()
    return nc


def kernel(**inputs):
    out_full = np.zeros((N, NCLS), np.float32)
    in_maps, dims = _prep(
        inputs["x"], inputs["edge_index"], inputs["Wl"], inputs["bl"],
        inputs["Wr"], inputs["br"], inputs["att"], inputs["bias"],
        inputs["Wf"], inputs["bf"])
    nc = _build(dims)
    from concourse.bass_utils import run_bass_kernel_spmd
    res = run_bass_kernel_spmd(nc, in_maps, core_ids=list(range(NCORES)))
    for k in range(NCORES):
        out_full[k * ND:(k + 1) * ND, :] = res.results[k]["out"][:ND, :NCLS]
    return out_full
